# revision 1
# baseline (speedup 1.0000x reference)
"""Trainium2 Bass kernel v2 for nn_AudioEvent.

Per-core pipeline (batch-parallel over 8 cores):
  osc:  host-precomputed quadratic phase coeffs (per 128-sample half-segment)
        -> stage2 matmuls (f32r, exact) -> mod 1.0 (DVE/Pool) -> Sin (Act,
        big tiles, arg 2*pi*frac - pi; env weights pre-negated)
        -> env-folded select matmuls (L/R interp nodes as 32 cols) -> *W blend
        -> pairsum matmul -> osc in (slot,e)-partition layout
  noise: host-transposed frames -> windowed rDFT matmuls -> gaussian filter
        (built on-chip) -> combined irfft+overlap-add matmuls directly into
        the (slot,e) layout
  mix:  piecewise-linear via per-partition node scalars + ramp consts
  out:  bf16, per-slot DMAs, host casts to f32
"""
import os
import numpy as np
import ml_dtypes

B, NE, NH, SEQ, N, WS = 8, 16, 32, 64, 16384, 512
NYQ = 11025.0
MIN_F0 = np.float64(20.0 / NYQ)
MAX_F0 = np.float64(800.0 / NYQ)
F0_DIFF = MAX_F0 - MIN_F0
NROW = NE * 33            # 528
NFR = SEQ * NE            # 1024
NSL = 64                  # 63 interior slots + head/tail combo slot

_cache = {}


def _build_static():
    if "static" in _cache:
        return _cache["static"]
    # ---- quadratic coeffs of V = cumsum(interp weights) ----
    pos = (np.arange(N, dtype=np.float64) + 0.5) * (SEQ / N) - 0.5
    pos = np.clip(pos, 0.0, SEQ - 1)
    i0 = np.floor(pos).astype(np.int64)
    i1 = np.minimum(i0 + 1, SEQ - 1)
    w = pos - i0
    U = np.zeros((SEQ, N))
    U[i0, np.arange(N)] += 1.0 - w
    U[i1, np.arange(N)] += w
    V = np.cumsum(U, axis=1)
    W64 = np.zeros((SEQ, 512))
    for m in range(32):
        for hs in range(4):
            t0 = 512 * m + 128 * hs
            A = V[:, t0]
            C = (V[:, t0 + 2] - 2 * V[:, t0 + 1] + V[:, t0]) / 2
            Bc = V[:, t0 + 1] - V[:, t0] - C
            W64[:, 16 * m + 3 * hs + 0] = A
            W64[:, 16 * m + 3 * hs + 1] = Bc
            W64[:, 16 * m + 3 * hs + 2] = C

    # ---- stage2 basis: even/odd paired [32, 512] ----
    j = np.arange(128, dtype=np.float64)
    basis16 = np.zeros((16, 512), np.float32)
    for hs in range(4):
        basis16[3 * hs + 0, 128 * hs:128 * (hs + 1)] = 1.0
        basis16[3 * hs + 1, 128 * hs:128 * (hs + 1)] = j
        basis16[3 * hs + 2, 128 * hs:128 * (hs + 1)] = j * j
    basisE = np.zeros((128, 512), np.float32)
    basisO = np.zeros((128, 512), np.float32)
    for a in range(4):
        basisE[32 * a:32 * a + 16] = basis16
        basisO[32 * a + 16:32 * a + 32] = basis16
    # fp16 split basis: 16 logical rows -> (1, j, j2hi, j2lo) x 4 half-segs,
    # each logical row replicated at 3 split slots (4th slot zero pad);
    # row = 64*rep + 4*br + s, two 64-row replicas
    b16s = np.zeros((16, 512))
    for hs in range(4):
        sl = slice(128 * hs, 128 * (hs + 1))
        b16s[4 * hs + 0, sl] = 1.0
        b16s[4 * hs + 1, sl] = j
        j2h = np.float16(j * j).astype(np.float64)
        b16s[4 * hs + 2, sl] = j2h
        b16s[4 * hs + 3, sl] = j * j - j2h
    basis64 = np.zeros((128, 512), np.float16)
    for rep in range(2):
        for br in range(16):
            for sp in range(3):
                basis64[64 * rep + 4 * br + sp] = np.float16(b16s[br])

    # ---- DFT consts (win folded), 4 contraction groups ----
    t = np.arange(WS)
    f = np.arange(WS // 2 + 1)
    win = 0.5 - 0.5 * np.cos(2 * np.pi * t / WS)
    ang = 2 * np.pi * np.outer(t, f) / WS
    CwRe = np.cos(ang) * win[:, None]
    CwIm = -np.sin(ang) * win[:, None]
    wgt = np.full(WS // 2 + 1, 2.0)
    wgt[0] = 1.0
    wgt[-1] = 1.0
    ang2 = 2 * np.pi * np.outer(f, t) / WS
    DRe = wgt[:, None] * np.cos(ang2) / WS
    DIm = -wgt[:, None] * np.sin(ang2) / WS
    # groups: 0: re f0..127 | 1: re f128..255 | 2: im f1..128 | 3: im f129..255 + re256
    Cw = np.zeros((128, 2048))       # col = tc*512 + grp*128 + fcol
    Dc = np.zeros((128, 2048))       # col = grp*512 + tau
    freq4 = np.zeros((128, 4))
    for grp in range(4):
        if grp == 0:
            fidx, mats = np.arange(0, 128), CwRe
        elif grp == 1:
            fidx, mats = np.arange(128, 256), CwRe
        elif grp == 2:
            fidx, mats = np.arange(1, 129), CwIm
        else:
            fidx, mats = np.concatenate([np.arange(129, 256), [256]]), None
        for tc in range(4):
            trows = slice(128 * tc, 128 * (tc + 1))
            if grp < 3:
                Cw[:, tc * 512 + grp * 128: tc * 512 + grp * 128 + 128] = mats[trows][:, fidx]
            else:
                blockm = CwIm[trows][:, fidx[:-1]]
                Cw[:, tc * 512 + grp * 128: tc * 512 + grp * 128 + 127] = blockm
                Cw[:, tc * 512 + grp * 128 + 127] = CwRe[trows][:, 256]
        if grp < 3:
            Dc[:, grp * 512:(grp + 1) * 512] = (DRe if grp < 2 else DIm)[fidx]
            freq4[:, grp] = fidx / 256.0
        else:
            Dc[:127, grp * 512:(grp + 1) * 512] = DIm[fidx[:-1]]
            Dc[127, grp * 512:(grp + 1) * 512] = DRe[256]
            freq4[:127, grp] = fidx[:-1] / 256.0
            freq4[127, grp] = 1.0

    # ---- ramp consts ----
    wj = (np.arange(256) + 0.5) / 256.0
    Wc = np.zeros((128, 256))
    for p in range(128):
        Wc[p] = wj if (p // 16) % 2 == 1 else 1.0 - wj
    Wc63 = Wc.copy()
    Wc63[96:112] = np.concatenate([np.ones(128), np.zeros(128)])
    Wc63[112:128] = np.concatenate([np.zeros(128), np.ones(128)])
    W0m = np.tile(1.0 - wj, (128, 1))
    W1m = np.tile(wj, (128, 1))
    W0m63 = W0m.copy()
    W1m63 = W1m.copy()
    W0m63[112:128] = np.concatenate([np.ones(128), np.zeros(128)])
    W1m63[112:128] = np.concatenate([np.zeros(128), np.ones(128)])
    P = np.zeros((128, 64))
    for p in range(128):
        P[p, 16 * (p // 32) + p % 16] = 1.0

    bf = ml_dtypes.bfloat16
    static = dict(
        W64=W64, basis64=basis64, b16s=b16s,
        Cw=Cw.astype(bf), Dc=Dc.astype(bf), freq4=freq4.astype(np.float32),
        Wc=Wc.astype(bf), Wc63=Wc63.astype(bf),
        W0m=W0m.astype(bf), W1m=W1m.astype(bf),
        W0m63=W0m63.astype(bf), W1m63=W1m63.astype(bf),
        P=P.astype(bf),
    )
    _cache["static"] = static
    return static


def _build_nc():
    if "nc" in _cache:
        return _cache["nc"]
    from concourse import bacc
    import concourse.tile as tile
    from concourse import mybir
    from contextlib import ExitStack

    F32 = mybir.dt.float32
    F16 = mybir.dt.float16
    BF16 = mybir.dt.bfloat16
    AF = mybir.ActivationFunctionType
    OP = mybir.AluOpType
    PI = float(np.pi)

    nc = bacc.Bacc()
    # data params
    coefT7 = nc.declare_dram_parameter("coefT7", [128, 5 * 2048], F16, isOutput=False)
    selW = nc.declare_dram_parameter("selW", [128, 8 * 2048], BF16, isOutput=False)
    ovn = nc.declare_dram_parameter("ovn", [128, 16], F32, isOutput=False)
    nfT = nc.declare_dram_parameter("nfT", [512, 1024], BF16, isOutput=False)
    meanb = nc.declare_dram_parameter("meanb", [128, 1056], BF16, isOutput=False)
    c2b = nc.declare_dram_parameter("c2b", [128, 1056], BF16, isOutput=False)
    # const params
    basis64 = nc.declare_dram_parameter("basis64", [128, 512], F16, isOutput=False)
    coefT4s = nc.declare_dram_parameter("coefT4s", [128, 1024], F16, isOutput=False)
    Cw = nc.declare_dram_parameter("Cw", [128, 2048], BF16, isOutput=False)
    Dc = nc.declare_dram_parameter("Dc", [128, 2048], BF16, isOutput=False)
    freq4 = nc.declare_dram_parameter("freq4", [128, 4], F32, isOutput=False)
    Wc = nc.declare_dram_parameter("Wc", [128, 256], BF16, isOutput=False)
    Wc63 = nc.declare_dram_parameter("Wc63", [128, 256], BF16, isOutput=False)
    W0m = nc.declare_dram_parameter("W0m", [128, 256], BF16, isOutput=False)
    W1m = nc.declare_dram_parameter("W1m", [128, 256], BF16, isOutput=False)
    W0m63 = nc.declare_dram_parameter("W0m63", [128, 256], BF16, isOutput=False)
    W1m63 = nc.declare_dram_parameter("W1m63", [128, 256], BF16, isOutput=False)
    P = nc.declare_dram_parameter("P", [128, 64], BF16, isOutput=False)
    out = nc.declare_dram_parameter("out", [NE, N], BF16, isOutput=True)

    with tile.TileContext(nc) as tc, ExitStack() as ctx:
        cp = ctx.enter_context(tc.tile_pool(name="const", bufs=1))
        # noise consts first (phase A starts immediately)
        nfT_sb = [cp.tile([128, 1024], BF16, tag=f"nfT{i}", name=f"nfT{i}") for i in range(4)]
        for i in range(4):
            nc.sync.dma_start(nfT_sb[i][:], nfT[128 * i:128 * (i + 1), :])
        meanb_sb = cp.tile([128, 1056], BF16, tag="meanb")
        nc.sync.dma_start(meanb_sb[:], meanb[:])
        c2b_sb = cp.tile([128, 1056], BF16, tag="c2b")
        nc.sync.dma_start(c2b_sb[:], c2b[:])
        Cw_sb = cp.tile([128, 2048], BF16, tag="Cw")
        nc.sync.dma_start(Cw_sb[:], Cw[:])
        freq4_sb = cp.tile([128, 4], F32, tag="freq4")
        nc.sync.dma_start(freq4_sb[:], freq4[:])
        Dc_sb = cp.tile([128, 2048], BF16, tag="Dc")
        nc.sync.dma_start(Dc_sb[:], Dc[:])
        # phase B consts (queued behind; SP has slack)
        coefT7_sb = cp.tile([128, 5 * 2048], F16, tag="coefT7")
        for bb in range(5):
            nc.sync.dma_start(coefT7_sb[:, 2048 * bb:2048 * (bb + 1)],
                              coefT7[:, 2048 * bb:2048 * (bb + 1)])
        basis64_sb = cp.tile([128, 512], F16, tag="basis64")
        nc.sync.dma_start(basis64_sb[:], basis64[:])
        coefT4s_sb = cp.tile([128, 1024], F16, tag="coefT4s")
        nc.sync.dma_start(coefT4s_sb[:], coefT4s[:])
        selW_sb = cp.tile([128, 8 * 2048], BF16, tag="selW")
        for bb in range(8):
            nc.sync.dma_start(selW_sb[:, 2048 * bb:2048 * (bb + 1)],
                              selW[:, 2048 * bb:2048 * (bb + 1)])
        P_sb = cp.tile([128, 64], BF16, tag="P")
        nc.sync.dma_start(P_sb[:], P[:])
        Wc_sb = cp.tile([128, 256], BF16, tag="Wc")
        nc.sync.dma_start(Wc_sb[:], Wc[:])
        Wc63_sb = cp.tile([128, 256], BF16, tag="Wc63")
        nc.sync.dma_start(Wc63_sb[:], Wc63[:])
        W0m_sb = cp.tile([128, 256], BF16, tag="W0m")
        nc.sync.dma_start(W0m_sb[:], W0m[:])
        W1m_sb = cp.tile([128, 256], BF16, tag="W1m")
        nc.sync.dma_start(W1m_sb[:], W1m[:])
        W0m63_sb = cp.tile([128, 256], BF16, tag="W0m63")
        nc.sync.dma_start(W0m63_sb[:], W0m63[:])
        W1m63_sb = cp.tile([128, 256], BF16, tag="W1m63")
        nc.sync.dma_start(W1m63_sb[:], W1m63[:])
        ovn_sb = cp.tile([128, 16], F32, tag="ovn")
        nc.sync.dma_start(ovn_sb[:], ovn[:])
        c23b = cp.tile([128, 1], F32, tag="c23b")
        nc.vector.memset(c23b[:], float(2.0 ** 23))

        spec_sb = [cp.tile([128, 1056], BF16, tag=f"spec{g}", name=f"spec{g}") for g in range(4)]
        noise_sb = [cp.tile([128, 256], BF16, tag=f"nz{t}", name=f"nz{t}") for t in range(8)]

        # ============ Phase A: noise ============
        with ExitStack() as actx:
            fpool = actx.enter_context(tc.tile_pool(name="fp", bufs=2))
            psA = actx.enter_context(tc.tile_pool(name="psA", bufs=2, space="PSUM"))
            psNZ = actx.enter_context(tc.tile_pool(name="psNZ", bufs=2, space="PSUM"))
            for g in range(4):
                nc.gpsimd.memset(spec_sb[g][:, 0:16], 0.0)
                nc.gpsimd.memset(spec_sb[g][:, 1040:1056], 0.0)
            # build filter then immediately rfft+mult for each group
            for g in range(4):
                d = fpool.tile([128, 1056], BF16, tag="fd")
                nc.gpsimd.tensor_scalar(d[:], meanb_sb[:], freq4_sb[:, g:g + 1], None, OP.subtract)
                d2 = fpool.tile([128, 1056], BF16, tag="fd2")
                nc.gpsimd.tensor_tensor(d2[:], d[:], d[:], OP.mult)
                m2 = fpool.tile([128, 1056], BF16, tag="fm2")
                nc.gpsimd.tensor_tensor(m2[:], d2[:], c2b_sb[:], OP.mult)
                filt = fpool.tile([128, 1056], BF16, tag=f"filt{g}")
                nc.scalar.activation(filt[:], m2[:], AF.Exp)
                for h in range(2):
                    fr_sl = slice(512 * h, 512 * (h + 1))
                    sp = psA.tile([128, 512], F32, tag="rf")
                    for tcx in range(4):
                        nc.tensor.matmul(sp[:],
                                         Cw_sb[:, tcx * 512 + g * 128: tcx * 512 + g * 128 + 128],
                                         nfT_sb[tcx][:, fr_sl],
                                         start=(tcx == 0), stop=(tcx == 3))
                    srf = fpool.tile([128, 512], BF16, tag="srf")
                    nc.scalar.copy(srf[:], sp[:])
                    nc.gpsimd.tensor_tensor(spec_sb[g][:, 16 + 512 * h:16 + 512 * (h + 1)],
                                            srf[:], filt[:, 16 + 512 * h:16 + 512 * (h + 1)],
                                            OP.mult)
            # irfft + overlap-add into (slot, e) layout
            for t in range(8):
                pz = psNZ.tile([128, 256], F32, tag="nzps")
                base = 16 + 16 * (8 * t)          # spec col of fr(slot=8t)
                nslots = 7 if t == 7 else 8
                ncols = 16 * nslots
                first = True
                for g in range(4):
                    gD = Dc_sb[:, 512 * g: 512 * (g + 1)]
                    sW = spec_sb[g]
                    # A: y[fr(s), 128+j] -> [:, 0:128]
                    nc.tensor.matmul(pz[0:ncols, 0:128], sW[:, base:base + ncols],
                                     gD[:, 128:256], start=first, stop=False,
                                     skip_group_check=True)
                    first = False
                    # C: y[fr(s-1), 384+j] -> [:, 0:128]
                    nc.tensor.matmul(pz[0:ncols, 0:128], sW[:, base - 16:base - 16 + ncols],
                                     gD[:, 384:512], start=False, stop=False,
                                     skip_group_check=True)
                    # B: y[fr(s+1), j] -> [:, 128:256]
                    nc.tensor.matmul(pz[0:ncols, 128:256], sW[:, base + 16:base + 16 + ncols],
                                     gD[:, 0:128], start=False, stop=False,
                                     skip_group_check=True)
                    # D: y[fr(s), 256+j] -> [:, 128:256]
                    nc.tensor.matmul(pz[0:ncols, 128:256], sW[:, base:base + ncols],
                                     gD[:, 256:384], start=False,
                                     stop=(t < 7 and g == 3),
                                     skip_group_check=True)
                if t == 7:
                    b63 = 16 + 16 * 63
                    pz63 = psNZ.tile([32, 256], F32, tag="nz63")
                    for g in range(4):
                        gD = Dc_sb[:, 512 * g: 512 * (g + 1)]
                        sW = spec_sb[g]
                        # head: y[fr(0), j] (+ zero-pad tail term via pad cols)
                        nc.tensor.matmul(pz63[0:16, 0:128], sW[:, 16:32],
                                         gD[:, 0:128], start=(g == 0), stop=False,
                                         skip_group_check=True, tile_position=(0, 0))
                        # tail: y[fr(63), 128+j] + y[fr(62), 384+j]
                        nc.tensor.matmul(pz63[0:16, 128:256], sW[:, b63:b63 + 16],
                                         gD[:, 128:256], start=False, stop=False,
                                         skip_group_check=True, tile_position=(0, 0))
                        last = (g == 3)
                        nc.tensor.matmul(pz63[0:16, 128:256], sW[:, b63 - 16:b63],
                                         gD[:, 384:512], start=False, stop=last,
                                         skip_group_check=True, tile_position=(0, 0))
                    nc.scalar.copy(noise_sb[t][0:112, :], pz[0:112, :])
                    nz63s = fpool.tile([16, 256], BF16, tag="nz63s")
                    nc.scalar.copy(nz63s[:], pz63[0:16, :])
                    nc.sync.dma_start(noise_sb[t][112:128, :], nz63s[:])
                else:
                    nc.scalar.copy(noise_sb[t][:], pz[:])

        # ============ Phase B: oscillator bank ============
        stp = ctx.enter_context(tc.tile_pool(name="st", bufs=2))
        st4p = ctx.enter_context(tc.tile_pool(name="st4", bufs=2))
        php = ctx.enter_context(tc.tile_pool(name="phi", bufs=3))
        ph4p = ctx.enter_context(tc.tile_pool(name="phi4", bufs=2))
        awp = ctx.enter_context(tc.tile_pool(name="aw", bufs=2))
        rtp = ctx.enter_context(tc.tile_pool(name="rt", bufs=3))
        fin = ctx.enter_context(tc.tile_pool(name="fin", bufs=2))
        psPH = ctx.enter_context(tc.tile_pool(name="psPH", bufs=2, space="PSUM"))
        psP4 = ctx.enter_context(tc.tile_pool(name="psP4", bufs=1, space="PSUM"))
        psSel = ctx.enter_context(tc.tile_pool(name="psSel", bufs=2, space="PSUM"))
        psSel3 = ctx.enter_context(tc.tile_pool(name="psSel3", bufs=2, space="PSUM"))
        psOsc = ctx.enter_context(tc.tile_pool(name="psOsc", bufs=1, space="PSUM"))

        st_tiles = {}   # (b, m) -> tile ; kept for current & previous span
        st4_tiles = {}
        mod_ctr = [0]

        C23 = float(2.0 ** 23)

        def do_mod(dst_ap, src_ap):
            """dst = frac-reduced phase in [-0.5, 0.5] via +-2^23 round trick.
            Rotates across engine assignments to balance load."""
            r = mod_ctr[0] % 8
            mod_ctr[0] += 1
            n_p = src_ap.partition_size()
            n_f = src_ap.free_size()
            yt = rtp.tile([128, 512], F32, tag="yt", name="yt")
            if r < 4:
                # X: Act yt (psum+2^23 -> sbuf, rounds at f32 write)
                nc.scalar.activation(yt[0:n_p, 0:n_f], src_ap, AF.Identity,
                                     bias=c23b[0:n_p, 0:1])
            else:
                # W: DVE yt
                nc.vector.tensor_scalar(yt[0:n_p, 0:n_f], src_ap,
                                        C23, None, OP.add)
            kt = rtp.tile([128, 512], F32, tag="kt", name="kt")
            nc.gpsimd.tensor_scalar(kt[0:n_p, 0:n_f], yt[0:n_p, 0:n_f],
                                    C23, None, OP.subtract)
            nc.vector.tensor_tensor(dst_ap, src_ap, kt[0:n_p, 0:n_f], OP.subtract)

        def emit_span(m):
            """stage2 + mod for span m; each sin deferred one b-block so its
            reduce chain drains while the next block's work occupies Act."""
            pend = None
            for b in range(4):
                phi = php.tile([128, 2048], F32, tag="phi", name=f"phi{b}")
                for k in range(4):
                    mm = 4 * m + k
                    p2 = 64 * (mm % 2)
                    cb = 2048 * b + (mm // 2) * 128
                    pp = psPH.tile([128, 512], F32, tag="ph")
                    nc.tensor.matmul(pp[:],
                                     coefT7_sb[p2:p2 + 64, cb:cb + 128],
                                     basis64_sb[p2:p2 + 64, :], start=True, stop=True,
                                     skip_group_check=True, tile_position=(p2, 0))
                    do_mod(phi[:, 512 * k:512 * (k + 1)], pp[:])
                if pend is not None:
                    pb, pphi = pend
                    st = stp.tile([128, 2048], BF16, tag=f"st{pb}", name=f"st{pb}_{m}")
                    nc.scalar.activation(st[:], pphi[:], AF.Sin, scale=2 * PI)
                    st_tiles[(pb, m)] = st
                pend = (b, phi)
            # block 4 (g 512..527), 4 chunks packed at 32-aligned bases
            phi4 = ph4p.tile([128, 512], F32, tag="phi4")
            pp4 = psP4.tile([128, 512], F32, tag="ph4")
            for kp in range(2):
                nc.tensor.matmul(pp4[64 * kp:64 * kp + 64, :],
                                 coefT4s_sb[:, 64 * (2 * m + kp):64 * (2 * m + kp) + 64],
                                 basis64_sb[:], start=True, stop=True,
                                 skip_group_check=True, tile_position=(0, 64 * kp))
            do_mod(phi4[:], pp4[:])
            pb, pphi = pend
            stl = stp.tile([128, 2048], BF16, tag=f"st{pb}", name=f"st{pb}_l{m}")
            nc.scalar.activation(stl[:], pphi[:], AF.Sin, scale=2 * PI)
            st_tiles[(pb, m)] = stl
            st4 = st4p.tile([128, 512], BF16, tag="st4")
            nc.scalar.activation(st4[:], phi4[:], AF.Sin, scale=2 * PI)
            st4_tiles[m] = st4

        def sel_windows(slot):
            """windows into st spans for slot; returns list of (m, lo, hi, psum_lo)."""
            t0 = 128 + 256 * slot
            m = t0 // 2048
            lo = t0 - 2048 * m
            if lo + 256 <= 2048:
                return [(m, lo, lo + 256, 0)]
            return [(m, lo, 2048, 0), (m + 1, 0, lo + 256 - 2048, 2048 - lo)]

        def sel4_windows(slot):
            """block4 windows: (span, chunk-in-span, lo, hi, psum_lo)."""
            t0 = 128 + 256 * slot
            mm = t0 // 512
            lo = t0 - 512 * mm
            if lo + 256 <= 512:
                return [(mm // 4, mm % 4, lo, lo + 256, 0)]
            mm2 = mm + 1
            return [(mm // 4, mm % 4, lo, 512, 0),
                    (mm2 // 4, mm2 % 4, 0, lo + 256 - 512, 512 - lo)]

        A_tiles = {}

        def emit_slot(slot):
            at4 = slot // 4
            if at4 not in A_tiles:
                A_tiles[at4] = (psSel.tile([128, 256], F32, tag="A", name=f"A{at4}"),
                               psSel3.tile([32, 256], F32, tag="A3", name=f"A3{at4}"))
            A, A3 = A_tiles[at4]
            sl4 = slot % 4
            first = [True]

            def outA(plo, ln):
                if sl4 < 3:
                    return A[32 * sl4:32 * sl4 + 32, plo:plo + ln], 32 * sl4
                return A3[0:32, plo:plo + ln], 0

            def mmA2(plo, ln, lhsT, lbase, rhs, last=False):
                dst, ob = outA(plo, ln)
                nc.tensor.matmul(dst, lhsT, rhs, start=first[0], stop=last,
                                 skip_group_check=True, tile_position=(lbase, ob))
                first[0] = False

            if slot < 63:
                cws = sel_windows(slot)
                c4s = sel4_windows(slot)
                nmm = 4 * len(cws) + len(c4s)
                i = 0
                for b in range(4):
                    for (m, lo, hi, plo) in cws:
                        i += 1
                        mmA2(plo, hi - lo,
                             selW_sb[:, 2048 * b + 32 * slot: 2048 * b + 32 * slot + 32], 0,
                             st_tiles[(b, m)][:, lo:hi], last=(i == nmm))
                for (m, k, lo, hi, plo) in c4s:
                    i += 1
                    mmA2(plo, hi - lo,
                         selW_sb[:, 2048 * (4 + k) + 32 * slot: 2048 * (4 + k) + 32 * slot + 32], 0,
                         st4_tiles[m][:, lo:hi], last=(i == nmm))
            else:
                # head (L cols -> [:,0:128], span-0 windows saved in stHT) and
                # tail (R cols -> [:,128:256], live span-7 tiles)
                for b in range(4):
                    mmA2(0, 128,
                         selW_sb[:, 2048 * b + 32 * 63: 2048 * b + 32 * 63 + 32], 0,
                         stHT[:, 128 * b:128 * (b + 1)], last=False)
                    mmA2(128, 128,
                         selW_sb[:, 2048 * b + 32 * 63: 2048 * b + 32 * 63 + 32], 0,
                         st_tiles[(b, 7)][:, 1920:2048], last=False)
                mmA2(0, 128,
                     selW_sb[:, 2048 * 4 + 32 * 63: 2048 * 4 + 32 * 63 + 32], 0,
                     stHT[:, 512:640], last=False)
                mmA2(128, 128,
                     selW_sb[:, 2048 * 7 + 32 * 63: 2048 * 7 + 32 * 63 + 32], 0,
                     st4_tiles[7][:, 384:512], last=True)

        osc_tiles = {}

        def emit_atile_done(at4):
            """A-tile at4 complete -> AW mult + pairsum into osc_ps."""
            A, A3 = A_tiles.pop(at4)
            aw = awp.tile([128, 256], BF16, tag="aw")
            wc = Wc63_sb if at4 == 15 else Wc_sb
            nc.vector.tensor_tensor(aw[0:96, :], A[0:96, :], wc[0:96, :], OP.mult)
            nc.vector.tensor_tensor(aw[96:128, :], A3[0:32, :], wc[96:128, :], OP.mult)
            t = at4 // 2
            if t not in osc_tiles:
                osc_tiles[t] = psOsc.tile([128, 256], F32, tag="osc", name=f"osc{t}")
            nc.tensor.matmul(osc_tiles[t][64 * (at4 % 2):64 * (at4 % 2) + 64, :],
                             P_sb[:], aw[:], start=True, stop=True,
                             skip_group_check=True, tile_position=(0, 64 * (at4 % 2)))

        def emit_combine(t):
            osc = osc_tiles.pop(t)
            w0 = W0m63_sb if t == 7 else W0m_sb
            w1 = W1m63_sb if t == 7 else W1m_sb
            mixa = fin.tile([128, 256], BF16, tag="mixa")
            nc.gpsimd.tensor_scalar(mixa[:], w0[:], ovn_sb[:, 2 * t:2 * t + 1], None, OP.mult)
            mixb = fin.tile([128, 256], BF16, tag="mixb")
            nc.gpsimd.tensor_scalar(mixb[:], w1[:], ovn_sb[:, 2 * t + 1:2 * t + 2], None, OP.mult)
            mixT = fin.tile([128, 256], BF16, tag="mixT")
            nc.gpsimd.tensor_tensor(mixT[:], mixa[:], mixb[:], OP.add)
            d = fin.tile([128, 256], BF16, tag="d")
            nc.vector.tensor_tensor(d[:], osc[:], noise_sb[t][:], OP.subtract)
            mres = fin.tile([128, 256], BF16, tag="mres")
            nc.gpsimd.tensor_tensor(mres[:], d[:], mixT[:], OP.mult)
            r = fin.tile([128, 256], BF16, tag="r")
            nc.gpsimd.tensor_tensor(r[:], mres[:], noise_sb[t][:], OP.add)
            # out DMAs per slot
            for sl in range(8):
                slot = 8 * t + sl
                if slot < 63:
                    t0 = 128 + 256 * slot
                    nc.sync.dma_start(out[:, t0:t0 + 256], r[16 * sl:16 * (sl + 1), :])
                else:
                    nc.sync.dma_start(out[:, 0:128], r[112:128, 0:128])
                    nc.sync.dma_start(out[:, 16256:16384], r[112:128, 128:256])

        # drive: spans 0..7; selects trail one span behind
        stHT = cp.tile([128, 640], BF16, tag="stHT")
        emit_span(0)
        for b in range(4):
            nc.gpsimd.tensor_copy(stHT[:, 128 * b:128 * (b + 1)],
                                  st_tiles[(b, 0)][:, 0:128])
        nc.gpsimd.tensor_copy(stHT[:, 512:640], st4_tiles[0][:, 0:128])
        for m in range(1, 8):
            emit_span(m)
            for slot in range(8 * (m - 1), 8 * m):
                emit_slot(slot)
                if slot % 4 == 3:
                    emit_atile_done(slot // 4)
                if slot % 8 == 7:
                    emit_combine(slot // 8)
            # free previous-previous span tiles
            for b in range(4):
                st_tiles.pop((b, m - 2), None)
            st4_tiles.pop(m - 2, None)
        for slot in range(56, 64):
            emit_slot(slot)
            if slot % 4 == 3:
                emit_atile_done(slot // 4)
            if slot % 8 == 7:
                emit_combine(slot // 8)
    nc.finalize()
    _cache["nc"] = nc
    return nc


def _host_prep(inputs):
    st = _build_static()
    bf = ml_dtypes.bfloat16
    f0 = np.clip(np.asarray(inputs["f0"], np.float64), -0.5, 0.5)
    f0b = np.asarray(inputs["f0_baselines"], np.float64)
    erb = (0.108 * (f0b * NYQ) + 24.7) / NYQ
    f0v = np.clip(f0b + f0 * erb, 0.0, 1.0)
    f0n = MIN_F0 + f0v * F0_DIFF                                    # (B,16,64)
    hfact = np.concatenate([[1.0], np.arange(2, 2 + NH)])
    frq = (f0n[:, :, None, :] * hfact[None, None, :, None] * 0.5)   # (B,16,33,64)
    frq = frq.reshape(B, NROW, SEQ)

    coef = np.einsum("bgs,sc->bgc", frq, st["W64"])                 # (B,528,512) f64
    # logical per-chunk coef rows br = 4*hs + {A,B,C,C}; W64 col = 16m+3hs+k
    clog = np.zeros((B, NROW, 32, 16))                              # (B,g,chunk,br)
    for m in range(32):
        for hs in range(4):
            base = 16 * m + 3 * hs
            clog[:, :, m, 4 * hs + 0] = coef[:, :, base + 0]
            clog[:, :, m, 4 * hs + 1] = coef[:, :, base + 1]
            clog[:, :, m, 4 * hs + 2] = coef[:, :, base + 2]
            clog[:, :, m, 4 * hs + 3] = coef[:, :, base + 2]
    # 3-way fp16 split
    h0 = clog.astype(np.float16).astype(np.float64)
    h1 = (clog - h0).astype(np.float16).astype(np.float64)
    h2 = (clog - h0 - h1).astype(np.float16)
    splits = [h0.astype(np.float16), h1.astype(np.float16), h2]
    # coefT7[b][64*(m%2) + 4*br + s, bblk*2048 + (m//2)*128 + g]
    coefT7 = np.zeros((B, 128, 5 * 2048), np.float16)
    for b5 in range(4):
        blkg = slice(128 * b5, 128 * (b5 + 1))
        for m in range(32):
            for sp in range(3):
                rows = 64 * (m % 2) + 4 * np.arange(16) + sp
                cols = b5 * 2048 + (m // 2) * 128
                coefT7[:, rows, cols:cols + 128] = \
                    splits[sp][:, blkg, m, :].transpose(0, 2, 1)
    # block-4 span-packed: [128, 16 blocks x 64]
    coefT4s = np.zeros((B, 128, 1024), np.float16)
    for m8 in range(8):
        for kp in range(2):
            blk = 2 * m8 + kp
            for klocal in range(2):
                ch = 4 * m8 + 2 * kp + klocal
                for sp in range(3):
                    rows = 64 * klocal + 4 * np.arange(16) + sp
                    cols = 64 * blk + 32 * klocal
                    coefT4s[:, rows, cols:cols + 16] = \
                        splits[sp][:, 512:528, ch, :].transpose(0, 2, 1)

    oe = np.clip(np.asarray(inputs["osc_env"], np.float64), 0, 1)   # (B,16,64)
    he = np.clip(np.asarray(inputs["harm_env"], np.float64), 0, 1)  # (B,16,32,64)
    env_node = np.zeros((B, NROW, SEQ))
    env_node[:, 0::33, :] = oe
    for o in range(1, 33):
        env_node[:, o::33, :] = oe * he[:, :, o - 1, :]
    selWh = np.zeros((B, 128, 8 * 2048), np.float32)
    eidx = np.arange(NROW) // 33
    for b5 in range(5):
        gl = 128 if b5 < 4 else 16
        for glo in range(gl):
            g = 128 * b5 + glo
            e = eidx[g]
            for side in range(2):
                cols = 2048 * b5 + 32 * np.arange(64) + 16 * side + e
                nodes = np.minimum(np.arange(64) + side, 63)
                if side == 0:
                    nodes = np.concatenate([np.arange(63), [0]])
                else:
                    nodes = np.concatenate([np.arange(1, 64), [63]])
                selWh[:, glo, cols] = env_node[:, g, nodes]
    # b4 variants: slot-block (4+k) has env rows only at 32k..32k+16
    for k in range(1, 4):
        selWh[:, 32 * k:32 * k + 16, 2048 * (4 + k):2048 * (5 + k)] = \
            selWh[:, 0:16, 2048 * 4:2048 * 5]
    selWh = selWh.astype(bf)

    ov = np.clip(np.asarray(inputs["overall_env"], np.float64), 0, 1)  # (B,16,64)
    ovn = np.zeros((B, 128, 16), np.float32)
    for t in range(8):
        for sl in range(8):
            slot = 8 * t + sl
            p = slice(16 * sl, 16 * (sl + 1))
            if slot < 63:
                ovn[:, p, 2 * t] = ov[:, :, slot]
                ovn[:, p, 2 * t + 1] = ov[:, :, min(slot + 1, 63)]
            else:
                ovn[:, p, 2 * t] = ov[:, :, 0]
                ovn[:, p, 2 * t + 1] = ov[:, :, 63]

    nf = np.asarray(inputs["noise_frames"], np.float32)             # (B,16,64,512)
    nfT = np.ascontiguousarray(
        nf.transpose(0, 3, 2, 1).reshape(B, WS, NFR)).astype(bf)    # [ws, s*16+e]

    nstd = np.clip(np.asarray(inputs["noise_std"], np.float64), 1e-12, 1.0) * F0_DIFF
    mean_fr = f0n.transpose(0, 2, 1).reshape(B, NFR)                # fr = s*16+e
    c2_fr = -0.5 / nstd.transpose(0, 2, 1).reshape(B, NFR) ** 2
    meanb = np.zeros((B, 128, 1056), np.float32)
    c2b = np.zeros((B, 128, 1056), np.float32)
    meanb[:, :, 16:1040] = mean_fr[:, None, :]
    c2b[:, :, 16:1040] = c2_fr[:, None, :]

    per_core = []
    for b in range(B):
        per_core.append(dict(
            coefT7=coefT7[b], coefT4s=coefT4s[b], selW=selWh[b], ovn=ovn[b], nfT=nfT[b],
            meanb=meanb[b].astype(bf), c2b=c2b[b].astype(bf),
            basis64=st["basis64"],
            Cw=st["Cw"], Dc=st["Dc"], freq4=st["freq4"],
            Wc=st["Wc"], Wc63=st["Wc63"], W0m=st["W0m"], W1m=st["W1m"],
            W0m63=st["W0m63"], W1m63=st["W1m63"], P=st["P"],
        ))
    return per_core


def kernel(**inputs):
    from concourse.bass_utils import run_bass_kernel_spmd
    per_core = _host_prep(inputs)
    nc = _build_nc()
    trace = bool(os.environ.get("BASS_PROFILE"))
    res = run_bass_kernel_spmd(nc, per_core, list(range(B)), trace=trace)
    if trace and res.exec_time_ns is not None:
        print(f"HW exec time: {res.exec_time_ns} ns")
    out = np.stack([np.asarray(r["out"], np.float32) for r in res.results])
    return out



# revision 35
# speedup vs baseline: 1.8030x; 1.8030x over previous
"""Trainium2 Bass kernel v3 for nn_AudioEvent.

Per-core pipeline (batch-parallel over 8 cores):
  harmonics: host quadratic phase coeffs (A pre-reduced mod 1 so |phase|<256)
        -> stage2 matmuls (f16 3-split, exact) -> frac via dual-op round
        (kt=(phi+2^23)-2^23 in one DVE op, bf16) -> PE -I matmul accumulates
        -kt into psum -> Sin from psum (Act) -> env-folded select matmuls
        (512 harmonics = exactly 4 contraction blocks) -> ramp mult ->
        pairsum matmul -> osc psum in (slot,e) layout
  fundamentals: computed directly in (slot,e)-partition layout [128,256]
        per t-tile: tiny quad-coef matmul -> frac -> Sin -> env interp via
        dual-op tensor_scalar with per-partition node scalars
  noise: windowed rDFT matmuls -> gaussian filter -> irfft+overlap-add
        matmuls into (slot,e) layout
  mix:  dual-op piecewise-linear; out [128,2048] bf16, host unshuffles
"""
import os
import numpy as np
import ml_dtypes

B, NE, NH, SEQ, N, WS = 8, 16, 32, 64, 16384, 512
NYQ = 11025.0
MIN_F0 = np.float64(20.0 / NYQ)
MAX_F0 = np.float64(800.0 / NYQ)
F0_DIFF = MAX_F0 - MIN_F0
NHRM = 512                # 16 events x 32 harmonics
NFR = SEQ * NE            # 1024
C23 = float(1.5 * 2.0 ** 23)  # round magic: x+C in [2^23,2^24) ulp-1 zone

_cache = {}

# per-block frac variant: v2 = DVE kt + PE sub; v3 = Act yt + Pool kt2 + PE sub;
# v1 = DVE kt + DVE sub (sbuf fr, block sin)
_VAR = []
for _i in range(32):
    if _i in (2, 6, 10, 14, 18, 22, 26, 30) and not os.environ.get("ALL_V2"):
        _VAR.append("v3")
    else:
        _VAR.append("v2")
# pairs (block, q=1) computed DVE-sub + sbuf-sin instead of PE -I + psum-sin
_V1P = {4 * m + 1 for m in range(8)}


def _quad_halfseg(V):
    """Per 128-sample half-segment quadratic coefs of V (cumsum weights).
    V: [SEQ, N] -> (A, Bc, C) each [SEQ, 128]."""
    H = N // 128
    A = V[:, 0::128][:, :H]
    C = (V[:, 2::128][:, :H] - 2 * V[:, 1::128][:, :H] + A) / 2
    Bc = V[:, 1::128][:, :H] - A - C
    return A, Bc, C


def _build_static():
    if "static" in _cache:
        return _cache["static"]
    # ---- interp weight cumsum V and quadratic coeffs ----
    pos = (np.arange(N, dtype=np.float64) + 0.5) * (SEQ / N) - 0.5
    pos = np.clip(pos, 0.0, SEQ - 1)
    i0 = np.floor(pos).astype(np.int64)
    i1 = np.minimum(i0 + 1, SEQ - 1)
    w = pos - i0
    U = np.zeros((SEQ, N))
    U[i0, np.arange(N)] += 1.0 - w
    U[i1, np.arange(N)] += w
    V = np.cumsum(U, axis=1)
    # harmonic chunk coefs: W64[s, 16m+3hs+k] (k=A,B,C) for chunk (m,hs)
    W64 = np.zeros((SEQ, 512))
    for m in range(32):
        for hs in range(4):
            t0 = 512 * m + 128 * hs
            A = V[:, t0]
            C = (V[:, t0 + 2] - 2 * V[:, t0 + 1] + V[:, t0]) / 2
            Bc = V[:, t0 + 1] - V[:, t0] - C
            W64[:, 16 * m + 3 * hs + 0] = A
            W64[:, 16 * m + 3 * hs + 1] = Bc
            W64[:, 16 * m + 3 * hs + 2] = C
    # fund: per half-seg quad coefs
    Aq, Bq, Cq = _quad_halfseg(V)          # [SEQ, 128]

    # ---- stage2 basis (f16 split): rows 64*rep + 4*br + sp ----
    j = np.arange(128, dtype=np.float64)
    b16s = np.zeros((16, 512))
    for hs in range(4):
        sl = slice(128 * hs, 128 * (hs + 1))
        b16s[4 * hs + 0, sl] = 1.0
        b16s[4 * hs + 1, sl] = j
        j2h = np.float16(j * j).astype(np.float64)
        b16s[4 * hs + 2, sl] = j2h
        b16s[4 * hs + 3, sl] = j * j - j2h
    basis64 = np.zeros((128, 512), np.float16)
    for rep in range(2):
        for br in range(16):
            for sp in range(3):
                basis64[64 * rep + 4 * br + sp] = np.float16(b16s[br])

    # ---- fund basis: rows 12*hh + 3*bf + sp; block-diag pair [64, 512] ----
    fbasis1 = np.zeros((32, 256), np.float16)
    for hh in range(2):
        sl = slice(128 * hh, 128 * (hh + 1))
        j2h = np.float16(j * j).astype(np.float64)
        for sp in range(3):
            fbasis1[12 * hh + 0 + sp, sl] = 1.0
            fbasis1[12 * hh + 3 + sp, sl] = np.float16(j)
            fbasis1[12 * hh + 6 + sp, sl] = np.float16(j2h)
            fbasis1[12 * hh + 9 + sp, sl] = np.float16(j * j - j2h)
    fbasis = np.zeros((64, 512), np.float16)
    fbasis[0:32, 0:256] = fbasis1
    fbasis[32:64, 256:512] = fbasis1

    # ---- DFT consts (win folded), 4 contraction groups ----
    t = np.arange(WS)
    f = np.arange(WS // 2 + 1)
    win = 0.5 - 0.5 * np.cos(2 * np.pi * t / WS)
    ang = 2 * np.pi * np.outer(t, f) / WS
    CwRe = np.cos(ang) * win[:, None]
    CwIm = -np.sin(ang) * win[:, None]
    wgt = np.full(WS // 2 + 1, 2.0)
    wgt[0] = 1.0
    wgt[-1] = 1.0
    ang2 = 2 * np.pi * np.outer(f, t) / WS
    DRe = wgt[:, None] * np.cos(ang2) / WS
    DIm = -wgt[:, None] * np.sin(ang2) / WS
    Cw = np.zeros((128, 2048))       # col = tc*512 + grp*128 + fcol
    Dc = np.zeros((128, 2048))       # col = grp*512 + tau
    freq4 = np.zeros((128, 4))
    for grp in range(4):
        if grp == 0:
            fidx, mats = np.arange(0, 128), CwRe
        elif grp == 1:
            fidx, mats = np.arange(128, 256), CwRe
        elif grp == 2:
            fidx, mats = np.arange(1, 129), CwIm
        else:
            fidx, mats = np.concatenate([np.arange(129, 256), [256]]), None
        for tc in range(4):
            trows = slice(128 * tc, 128 * (tc + 1))
            if grp < 3:
                Cw[:, tc * 512 + grp * 128: tc * 512 + grp * 128 + 128] = mats[trows][:, fidx]
            else:
                blockm = CwIm[trows][:, fidx[:-1]]
                Cw[:, tc * 512 + grp * 128: tc * 512 + grp * 128 + 127] = blockm
                Cw[:, tc * 512 + grp * 128 + 127] = CwRe[trows][:, 256]
        if grp < 3:
            Dc[:, grp * 512:(grp + 1) * 512] = (DRe if grp < 2 else DIm)[fidx]
            freq4[:, grp] = fidx / 256.0
        else:
            Dc[:127, grp * 512:(grp + 1) * 512] = DIm[fidx[:-1]]
            Dc[127, grp * 512:(grp + 1) * 512] = DRe[256]
            freq4[:127, grp] = fidx[:-1] / 256.0
            freq4[127, grp] = 1.0

    # ---- ramp consts ----
    wj = (np.arange(256) + 0.5) / 256.0
    Wc = np.zeros((128, 256))
    for p in range(128):
        Wc[p] = wj if (p // 16) % 2 == 1 else 1.0 - wj
    Wc63 = Wc.copy()
    Wc63[96:112] = np.concatenate([np.ones(128), np.zeros(128)])
    Wc63[112:128] = np.concatenate([np.zeros(128), np.ones(128)])
    W1m = np.tile(wj, (128, 1))
    W1m63 = W1m.copy()
    W1m63[112:128] = np.concatenate([np.zeros(128), np.ones(128)])
    P = np.zeros((128, 64))
    for p in range(128):
        P[p, 16 * (p // 32) + p % 16] = 1.0
    negI = (-np.eye(128)).astype(np.float64)

    bf = ml_dtypes.bfloat16
    static = dict(
        W64=W64, basis64=basis64, Aq=Aq, Bq=Bq, Cq=Cq, fbasis=fbasis,
        Cw=Cw.astype(bf), Dc=Dc.astype(bf), freq4=freq4.astype(np.float32),
        Wc=Wc.astype(bf), Wc63=Wc63.astype(bf),
        W1m=W1m.astype(bf), W1m63=W1m63.astype(bf),
        P=P.astype(bf), negI=negI.astype(bf),
    )
    _cache["static"] = static
    return static


def _split3(x):
    """3-way fp16 split of float64 array: returns list of 3 fp16 arrays."""
    h0 = x.astype(np.float16).astype(np.float64)
    h1 = (x - h0).astype(np.float16).astype(np.float64)
    h2 = (x - h0 - h1).astype(np.float16)
    return [h0.astype(np.float16), h1.astype(np.float16), h2]


def _build_nc():
    if "nc" in _cache:
        return _cache["nc"]
    from concourse import bacc
    import concourse.tile as tile
    from concourse import mybir
    from contextlib import ExitStack

    F32 = mybir.dt.float32
    F16 = mybir.dt.float16
    BF16 = mybir.dt.bfloat16
    AF = mybir.ActivationFunctionType
    OP = mybir.AluOpType
    PI = float(np.pi)

    nc = bacc.Bacc()
    nc._dbg = {}
    # data params
    coefT7 = nc.declare_dram_parameter("coefT7", [128, 16 * 640], F16, isOutput=False)
    selW = nc.declare_dram_parameter("selW", [128, 4 * 2048], BF16, isOutput=False)
    envsc = nc.declare_dram_parameter("envsc", [128, 32], F32, isOutput=False)
    fcoef = nc.declare_dram_parameter("fcoef", [64, 512], F16, isOutput=False)
    nfT = nc.declare_dram_parameter("nfT", [512, 1024], BF16, isOutput=False)
    meanb = nc.declare_dram_parameter("meanb", [128, 1056], BF16, isOutput=False)
    c2b = nc.declare_dram_parameter("c2b", [128, 1056], BF16, isOutput=False)
    # const params
    basis64 = nc.declare_dram_parameter("basis64", [128, 512], F16, isOutput=False)
    fbasis = nc.declare_dram_parameter("fbasis", [64, 512], F16, isOutput=False)
    Cw = nc.declare_dram_parameter("Cw", [128, 2048], BF16, isOutput=False)
    Dc = nc.declare_dram_parameter("Dc", [128, 2048], BF16, isOutput=False)
    freq4 = nc.declare_dram_parameter("freq4", [128, 4], F32, isOutput=False)
    Wc = nc.declare_dram_parameter("Wc", [128, 256], BF16, isOutput=False)
    Wc63 = nc.declare_dram_parameter("Wc63", [128, 256], BF16, isOutput=False)
    W1m = nc.declare_dram_parameter("W1m", [128, 256], BF16, isOutput=False)
    W1m63 = nc.declare_dram_parameter("W1m63", [128, 256], BF16, isOutput=False)
    P = nc.declare_dram_parameter("P", [128, 64], BF16, isOutput=False)
    negI = nc.declare_dram_parameter("negI", [128, 128], BF16, isOutput=False)
    out = nc.declare_dram_parameter("out", [128, 2048], BF16, isOutput=True)

    with tile.TileContext(nc) as tc, ExitStack() as ctx:
        cp = ctx.enter_context(tc.tile_pool(name="const", bufs=1))

        # ---- const DMAs (SP queue), ordered by first need ----
        meanb_sb = cp.tile([128, 1056], BF16, tag="meanb")
        nc.gpsimd.dma_start(meanb_sb[:], meanb[:])
        c2b_sb = cp.tile([128, 1056], BF16, tag="c2b")
        nc.gpsimd.dma_start(c2b_sb[:], c2b[:])
        basis64_sb = cp.tile([128, 512], F16, tag="basis64")
        nc.sync.dma_start(basis64_sb[:], basis64[:])
        freq4_sb = cp.tile([128, 4], F32, tag="freq4")
        nc.sync.dma_start(freq4_sb[:], freq4[:])
        coefT7_sb = cp.tile([128, 16 * 640], F16, tag="coefT7")
        nc.sync.dma_start(coefT7_sb[:, 0:1280], coefT7[:, 0:1280])
        negI_sb = cp.tile([128, 128], BF16, tag="negI")
        nc.sync.dma_start(negI_sb[:], negI[:])
        nc.sync.dma_start(coefT7_sb[:, 1280:2560], coefT7[:, 1280:2560])
        fbasis_sb = cp.tile([64, 512], F16, tag="fbasis")
        nc.sync.dma_start(fbasis_sb[:], fbasis[:])
        fcoef_sb = cp.tile([64, 512], F16, tag="fcoef")
        nc.sync.dma_start(fcoef_sb[:], fcoef[:])
        W1m_sb = cp.tile([128, 256], BF16, tag="W1m")
        nc.sync.dma_start(W1m_sb[:], W1m[:])
        W1m63_sb = cp.tile([128, 256], BF16, tag="W1m63")
        nc.sync.dma_start(W1m63_sb[:], W1m63[:])
        P_sb = cp.tile([128, 64], BF16, tag="P")
        nc.sync.dma_start(P_sb[:], P[:])
        envsc_sb = cp.tile([128, 32], F32, tag="envsc")
        nc.sync.dma_start(envsc_sb[:], envsc[:])
        Wc_sb = cp.tile([128, 256], BF16, tag="Wc")
        nc.sync.dma_start(Wc_sb[:], Wc[:])
        Wc63_sb = cp.tile([128, 256], BF16, tag="Wc63")
        nc.sync.dma_start(Wc63_sb[:], Wc63[:])
        # selW in slot-quarters so early selects unblock sooner
        selW_sb = cp.tile([128, 4 * 2048], BF16, tag="selW")
        for b4 in range(4):           # slots 0..15
            nc.sync.dma_start(selW_sb[:, 2048 * b4:2048 * b4 + 512],
                              selW[:, 2048 * b4:2048 * b4 + 512])
        nfT_sb = [cp.tile([128, 1024], BF16, tag=f"nfT{i}", name=f"nfT{i}") for i in range(4)]
        for i in range(4):
            nc.sync.dma_start(nfT_sb[i][:], nfT[128 * i:128 * (i + 1), :])
        Cw_sb = cp.tile([128, 2048], BF16, tag="Cw")
        nc.sync.dma_start(Cw_sb[:], Cw[:])
        Dc_sb = cp.tile([128, 2048], BF16, tag="Dc")
        nc.sync.dma_start(Dc_sb[:], Dc[:])
        for cch in (2, 3):
            nc.sync.dma_start(coefT7_sb[:, 1280 * cch:1280 * (cch + 1)],
                              coefT7[:, 1280 * cch:1280 * (cch + 1)])
        for b4 in range(4):           # slots 16..39
            nc.sync.dma_start(selW_sb[:, 2048 * b4 + 512:2048 * b4 + 1280],
                              selW[:, 2048 * b4 + 512:2048 * b4 + 1280])
        for cch in (4, 5):
            nc.sync.dma_start(coefT7_sb[:, 1280 * cch:1280 * (cch + 1)],
                              coefT7[:, 1280 * cch:1280 * (cch + 1)])
        for b4 in range(4):           # slots 40..63
            nc.sync.dma_start(selW_sb[:, 2048 * b4 + 1280:2048 * b4 + 2048],
                              selW[:, 2048 * b4 + 1280:2048 * b4 + 2048])
        for cch in (6, 7):
            nc.sync.dma_start(coefT7_sb[:, 1280 * cch:1280 * (cch + 1)],
                              coefT7[:, 1280 * cch:1280 * (cch + 1)])
        c23b = cp.tile([128, 1], F32, tag="c23b")
        nc.vector.memset(c23b[:], C23)

        spec_sb = [cp.tile([128, 1056], BF16, tag=f"spec{g}", name=f"spec{g}") for g in range(4)]
        noise_sb = [cp.tile([128, 256], BF16, tag=f"nz{t}", name=f"nz{t}") for t in range(8)]
        st4f = [cp.tile([128, 512], BF16, tag=f"st4fp{u}", name=f"st4fp{u}") for u in range(4)]
        stHT = cp.tile([128, 512], BF16, tag="stHT")

        # ---- pools ----
        php = ctx.enter_context(tc.tile_pool(name="phi", bufs=2))      # v1 fr tiles
        stp = ctx.enter_context(tc.tile_pool(name="st", bufs=3))
        rtp = ctx.enter_context(tc.tile_pool(name="rt", bufs=3))       # kt bf16 pairs
        ytp = ctx.enter_context(tc.tile_pool(name="yt", bufs=2))       # v3 yt f32 pairs
        awp = ctx.enter_context(tc.tile_pool(name="aw", bufs=2))
        fin = ctx.enter_context(tc.tile_pool(name="fin", bufs=2))
        fpool = ctx.enter_context(tc.tile_pool(name="fp", bufs=1))
        # PSUM: ph ring 3x[128,1024] (6 banks) + A-pair (1) + osc-pair (1)
        psPH = ctx.enter_context(tc.tile_pool(name="psPH", bufs=3, space="PSUM"))
        psSel = ctx.enter_context(tc.tile_pool(name="psSel", bufs=1, space="PSUM"))
        psOsc = ctx.enter_context(tc.tile_pool(name="psOsc", bufs=1, space="PSUM"))

        st_tiles = {}

        # ============ noise filter pre-pass (Act: exps before any sin) ======
        for g in range(4):
            nc.gpsimd.memset(spec_sb[g][:, 0:16], 0.0)
            nc.gpsimd.memset(spec_sb[g][:, 1040:1056], 0.0)
        filt_t = [None] * 4
        dts = {}
        for g in (0, 1):
            d = fpool.tile([128, 1056], BF16, tag=f"fdt{g}", name=f"fd{g}")
            nc.gpsimd.tensor_scalar(d[:], meanb_sb[:], freq4_sb[:, g:g + 1], None,
                                    OP.subtract)
            dts[g] = d
        for g in (2, 3):
            d = fpool.tile([128, 1056], BF16, tag=f"fdt{g}", name=f"fd{g}")
            nc.vector.tensor_scalar(d[:], meanb_sb[:], freq4_sb[:, g:g + 1], None,
                                    OP.subtract)
            dts[g] = d
        # PE p-state warm-up: dummy accumulations while DMAs stream in
        warm = psPH.tile([128, 1024], F32, tag="ph", name="warm")
        for i in range(14):
            nc.tensor.matmul(warm[:, 0:512], basis64_sb[0:64, 0:128],
                             basis64_sb[0:64, :], start=(i == 0), stop=(i == 13),
                             skip_group_check=True, tile_position=(0, 0))
        dd = {}
        for g in (0, 1):
            d2 = fpool.tile([128, 1056], BF16, tag=f"fd2t{g}", name=f"fd2{g}")
            nc.gpsimd.tensor_tensor(d2[:], dts[g][:], dts[g][:], OP.mult)
            dd[g] = d2
        for g in (2, 3, 0, 1):
            if g in (2, 3):
                d2 = fpool.tile([128, 1056], BF16, tag=f"fd2t{g}", name=f"fd2{g}")
                nc.vector.tensor_tensor(d2[:], dts[g][:], dts[g][:], OP.mult)
            else:
                d2 = dd[g]
            m2 = fpool.tile([128, 1056], BF16, tag=f"fm2t{g}", name=f"fm2{g}")
            nc.vector.tensor_tensor(m2[:], d2[:], c2b_sb[:], OP.mult)
            filt = fpool.tile([128, 1056], BF16, tag=f"filt{g}", name=f"filt{g}")
            nc.scalar.activation(filt[:], m2[:], AF.Exp)
            filt_t[g] = filt

        # ============ span machinery (pair-granular, software-pipelined) ====
        pair_state = {}

        def emit_F(m, b, q):
            """stage2 matmuls + round (kt) for pair q of block (m, b)."""
            var = _VAR[4 * m + b]
            if var == "v2" and q == 1 and (4 * m + b) in _V1P:
                var = "v1p"
            if q == 0:
                st = stp.tile([128, 2048], BF16, tag=f"st{b}", name=f"st{b}_{m}")
                st_tiles[(b, m)] = st
            pp = psPH.tile([128, 1024], F32, tag="ph", name=f"pp{m}_{b}_{q}")
            for h in range(2):
                k = 2 * q + h
                mm = 4 * m + k
                p2 = 64 * (mm % 2)
                cb = (mm // 2) * 640 + b * 128
                nc.tensor.matmul(pp[:, 512 * h:512 * (h + 1)],
                                 coefT7_sb[p2:p2 + 64, cb:cb + 128],
                                 basis64_sb[p2:p2 + 64, :], start=True, stop=True,
                                 skip_group_check=True, tile_position=(p2, 0))
            kt = rtp.tile([128, 1024], BF16, tag="kt", name=f"kt{m}_{b}_{q}")
            if var == "v3":
                yt = ytp.tile([128, 1024], F32, tag="yt", name=f"yt{m}_{b}_{q}")
                nc.scalar.activation(yt[:], pp[:], AF.Identity, bias=c23b[:, 0:1])
                nc.gpsimd.tensor_scalar(kt[:], yt[:], C23, None, OP.subtract)
            else:
                nc.vector.tensor_scalar(kt[:], pp[:], C23, C23, OP.add, OP.subtract)
            pair_state[(b, q)] = (var, pp, kt)

        def emit_D(m, b, q):
            """-I accumulate + Sin (or DVE sub for v1p) for pair q."""
            var, pp, kt = pair_state.pop((b, q))
            st = st_tiles[(b, m)]
            if var == "v1p":
                fr = php.tile([128, 1024], F32, tag="fr", name=f"fr{b}_{m}_{q}")
                nc.vector.tensor_tensor(fr[:], pp[:], kt[:], OP.subtract)
                nc.scalar.activation(st[:, 1024 * q:1024 * (q + 1)], fr[:],
                                     AF.Sin, scale=2 * PI)
            else:
                for h in range(2):
                    nc.tensor.matmul(pp[:, 512 * h:512 * (h + 1)], negI_sb[:],
                                     kt[:, 512 * h:512 * (h + 1)],
                                     start=False, stop=True,
                                     skip_group_check=True, tile_position=(0, 0))
                nc.scalar.activation(st[:, 1024 * q:1024 * (q + 1)], pp[:],
                                     AF.Sin, scale=2 * PI)

        def span_sched(m):
            ops = []
            pend = []
            for b in range(4):
                for q in range(2):
                    ops.append(("F", b, q))
                    pend.append((b, q))
                    if len(pend) > 2:
                        ops.append(("D",) + pend.pop(0))
            while pend:
                ops.append(("D",) + pend.pop(0))
            return ops

        def emit_fund_pair(u):
            pf = psPH.tile([128, 1024], F32, tag="ph", name=f"fph{u}")
            nc.tensor.matmul(pf[:, 0:512], fcoef_sb[:, 128 * u:128 * (u + 1)],
                             fbasis_sb[:], start=True, stop=True,
                             skip_group_check=True, tile_position=(0, 0))
            kt = rtp.tile([128, 1024], BF16, tag="kt", name=f"fkt{u}")
            nc.vector.tensor_scalar(kt[:, 0:512], pf[:, 0:512], C23, C23,
                                    OP.add, OP.subtract)
            nc.tensor.matmul(pf[:, 0:512], negI_sb[:], kt[:, 0:512],
                             start=False, stop=True, skip_group_check=True,
                             tile_position=(0, 0))
            nc.scalar.activation(st4f[u][:], pf[:, 0:512], AF.Sin, scale=2 * PI)

        def sel_windows(slot):
            t0 = 128 + 256 * slot
            m = t0 // 2048
            lo = t0 - 2048 * m
            if lo + 256 <= 2048:
                return [(m, lo, lo + 256, 0)]
            return [(m, lo, 2048, 0), (m + 1, 0, lo + 256 - 2048, 2048 - lo)]

        A_pairs = {}

        def emit_slot(slot):
            at4 = slot // 4
            v = at4 // 2
            if v not in A_pairs:
                A_pairs[v] = psSel.tile([128, 512], F32, tag="A", name=f"Ap{v}")
            ao = 256 * (at4 % 2)
            A = A_pairs[v]
            sl4 = slot % 4
            first = [True]
            if slot < 63:
                cws = sel_windows(slot)
                nmm = 4 * len(cws)
                i = 0
                for b in range(4):
                    for (mw, lo, hi, plo) in cws:
                        i += 1
                        nc.tensor.matmul(
                            A[32 * sl4:32 * sl4 + 32, ao + plo:ao + plo + hi - lo],
                            selW_sb[:, 2048 * b + 32 * slot: 2048 * b + 32 * slot + 32],
                            st_tiles[(b, mw)][:, lo:hi],
                            start=first[0], stop=(i == nmm),
                            skip_group_check=True, tile_position=(0, 32 * sl4))
                        first[0] = False
            else:
                for b in range(4):
                    nc.tensor.matmul(
                        A[96:128, ao:ao + 128],
                        selW_sb[:, 2048 * b + 32 * 63: 2048 * b + 32 * 63 + 32],
                        stHT[:, 128 * b:128 * (b + 1)],
                        start=(b == 0), stop=False,
                        skip_group_check=True, tile_position=(0, 96))
                    nc.tensor.matmul(
                        A[96:128, ao + 128:ao + 256],
                        selW_sb[:, 2048 * b + 32 * 63: 2048 * b + 32 * 63 + 32],
                        st_tiles[(b, 7)][:, 1920:2048],
                        start=False, stop=(b == 3),
                        skip_group_check=True, tile_position=(0, 96))
            if slot % 4 == 3:
                emit_atile_done(at4)

        osc_pairs = {}

        def emit_atile_done(at4):
            v = at4 // 2
            A = A_pairs[v]
            ao = 256 * (at4 % 2)
            if at4 % 2 == 1:
                A_pairs.pop(v)
            aw = awp.tile([128, 256], BF16, tag="aw")
            wc = Wc63_sb if at4 == 15 else Wc_sb
            nc.vector.tensor_tensor(aw[:], A[:, ao:ao + 256], wc[:], OP.mult)
            t = at4 // 2
            u = t // 2
            if u not in osc_pairs:
                osc_pairs[u] = psOsc.tile([128, 512], F32, tag="osc", name=f"oscp{u}")
            oo = 256 * (t % 2)
            nc.tensor.matmul(
                osc_pairs[u][64 * (at4 % 2):64 * (at4 % 2) + 64, oo:oo + 256],
                P_sb[:], aw[:], start=True, stop=True,
                skip_group_check=True, tile_position=(0, 64 * (at4 % 2)))

        def emit_rfft(g, h):
            fr_sl = slice(512 * h, 512 * (h + 1))
            sp = psPH.tile([128, 1024], F32, tag="ph", name=f"rf{g}_{h}")
            for tcx in range(4):
                nc.tensor.matmul(sp[:, 0:512],
                                 Cw_sb[:, tcx * 512 + g * 128: tcx * 512 + g * 128 + 128],
                                 nfT_sb[tcx][:, fr_sl],
                                 start=(tcx == 0), stop=(tcx == 3))
            nc.vector.tensor_tensor(spec_sb[g][:, 16 + 512 * h:16 + 512 * (h + 1)],
                                    sp[:, 0:512], filt_t[g][:, 16 + 512 * h:16 + 512 * (h + 1)],
                                    OP.mult)

        def emit_irfft(t):
            pz = psPH.tile([128, 1024], F32, tag="ph", name=f"nzps{t}")
            base = 16 + 16 * (8 * t)
            nslots = 7 if t == 7 else 8
            ncols = 16 * nslots
            for g in range(4):
                gD = Dc_sb[:, 512 * g: 512 * (g + 1)]
                sW = spec_sb[g]
                nc.tensor.matmul(pz[0:ncols, 0:256], sW[:, base:base + ncols],
                                 gD[:, 128:384], start=(g == 0), stop=False,
                                 skip_group_check=True)
                nc.tensor.matmul(pz[0:ncols, 0:128], sW[:, base - 16:base - 16 + ncols],
                                 gD[:, 384:512], start=False, stop=False,
                                 skip_group_check=True)
                nc.tensor.matmul(pz[0:ncols, 128:256], sW[:, base + 16:base + 16 + ncols],
                                 gD[:, 0:128], start=False,
                                 stop=(t < 7 and g == 3),
                                 skip_group_check=True)
            if t == 7:
                b63 = 16 + 16 * 63
                pz63 = psPH.tile([32, 512], F32, tag="ph", name="nz63")
                for g in range(4):
                    gD = Dc_sb[:, 512 * g: 512 * (g + 1)]
                    sW = spec_sb[g]
                    nc.tensor.matmul(pz63[0:16, 0:128], sW[:, 16:32],
                                     gD[:, 0:128], start=(g == 0), stop=False,
                                     skip_group_check=True, tile_position=(0, 0))
                    nc.tensor.matmul(pz63[0:16, 128:256], sW[:, b63:b63 + 16],
                                     gD[:, 128:256], start=False, stop=False,
                                     skip_group_check=True, tile_position=(0, 0))
                    nc.tensor.matmul(pz63[0:16, 128:256], sW[:, b63 - 16:b63],
                                     gD[:, 384:512], start=False, stop=(g == 3),
                                     skip_group_check=True, tile_position=(0, 0))
                nc.vector.tensor_copy(noise_sb[t][0:112, :], pz[0:112, 0:256])
                nz63s = fpool.tile([16, 256], BF16, tag="nz63s")
                nc.vector.tensor_copy(nz63s[:], pz63[0:16, 0:256])
                nc.sync.dma_start(noise_sb[t][112:128, :], nz63s[:])
            else:
                nc.vector.tensor_copy(noise_sb[t][:], pz[:, 0:256])

        def emit_combine(t):
            u = t // 2
            osc = osc_pairs[u][:, 256 * (t % 2):256 * (t % 2) + 256]
            if t % 2 == 1:
                osc_pairs.pop(u)
            w1 = W1m63_sb if t == 7 else W1m_sb
            mixT = fin.tile([128, 256], BF16, tag="mixT")
            nc.gpsimd.tensor_scalar(mixT[:], w1[:], envsc_sb[:, 4 * t + 1:4 * t + 2],
                                    envsc_sb[:, 4 * t + 0:4 * t + 1], OP.mult, OP.add)
            mixF = fin.tile([128, 256], BF16, tag="mixF")
            nc.gpsimd.tensor_scalar(mixF[:], w1[:], envsc_sb[:, 4 * t + 3:4 * t + 4],
                                    envsc_sb[:, 4 * t + 2:4 * t + 3], OP.mult, OP.add)
            fc = fin.tile([128, 256], BF16, tag="fc")
            nc.gpsimd.tensor_tensor(fc[:], st4f[t // 2][:, 256 * (t % 2):256 * (t % 2) + 256], mixF[:], OP.mult)
            d = fin.tile([128, 256], BF16, tag="d")
            nc.vector.tensor_tensor(d[:], osc, noise_sb[t][:], OP.subtract)
            d2 = fin.tile([128, 256], BF16, tag="d2")
            nc.gpsimd.tensor_tensor(d2[:], d[:], fc[:], OP.add)
            mres = fin.tile([128, 256], BF16, tag="mres")
            nc.gpsimd.tensor_tensor(mres[:], d2[:], mixT[:], OP.mult)
            r = fin.tile([128, 256], BF16, tag="r")
            nc._dbg[f"r{t}"] = r
            nc.gpsimd.tensor_tensor(r[:], mres[:], noise_sb[t][:], OP.add)
            nc.sync.dma_start(out[:, 256 * t:256 * (t + 1)], r[:])

        # ============ interleaved drive (global pipelined stream) ============
        def mkslot(sl):
            return lambda: emit_slot(sl)

        def stht_copy():
            for b in range(4):
                nc.gpsimd.tensor_copy(stHT[:, 128 * b:128 * (b + 1)],
                                      st_tiles[(b, 0)][:, 0:128])

        fillers = {m: [] for m in range(8)}
        fillers[0] = [stht_copy]
        fillers[1] = [lambda: emit_rfft(0, 0), lambda: emit_rfft(0, 1),
                      lambda: emit_rfft(1, 0), lambda: emit_rfft(1, 1),
                      mkslot(0), lambda: emit_rfft(2, 0), mkslot(1),
                      lambda: emit_rfft(2, 1), mkslot(2), lambda: emit_rfft(3, 0),
                      mkslot(3), lambda: emit_rfft(3, 1), mkslot(4),
                      lambda: emit_fund_pair(0), mkslot(5),
                      mkslot(6), mkslot(7), lambda: emit_irfft(0)]
        for m in range(2, 8):
            base_s = 8 * (m - 1)
            fl = [(lambda t: lambda: emit_combine(t))(m - 2)]
            if m in (3, 5, 7):
                fl.append((lambda u: lambda: emit_fund_pair(u))(m // 2))
            fl += [mkslot(base_s + i) for i in range(6)]
            fl.append((lambda t: lambda: emit_irfft(t))(m - 1))
            fl += [mkslot(base_s + 6), mkslot(base_s + 7)]
            fillers[m] = fl

        allp = [(m, b, q) for m in range(8) for b in range(4) for q in range(2)]
        ops = []
        pend = []
        for pr in allp:
            ops.append(("F",) + pr)
            pend.append(pr)
            if len(pend) > 2:
                ops.append(("D",) + pend.pop(0))
        while pend:
            ops.append(("D",) + pend.pop(0))

        from collections import Counter
        cnt = Counter(op[1] for op in ops)
        seen = Counter()
        fidx = {m: 0 for m in fillers}
        fm = [0]

        def pump():
            # strictly ordered filler groups; pace group fm by span-fm progress
            while fm[0] < 8:
                fl = fillers[fm[0]]
                if fidx[fm[0]] >= len(fl):
                    if seen[fm[0]] >= cnt[fm[0]]:
                        fm[0] += 1
                        continue
                    break
                frac = seen[fm[0]] / cnt[fm[0]]
                if (fidx[fm[0]] + 1) / len(fl) <= frac or seen[fm[0]] >= cnt[fm[0]]:
                    fl[fidx[fm[0]]]()
                    fidx[fm[0]] += 1
                else:
                    break

        for op in ops:
            if op[0] == "F":
                emit_F(op[1], op[2], op[3])
            else:
                emit_D(op[1], op[2], op[3])
            seen[op[1]] += 1
            pump()
        while fm[0] < 8:
            fl = fillers[fm[0]]
            if fidx[fm[0]] < len(fl):
                fl[fidx[fm[0]]]()
                fidx[fm[0]] += 1
            else:
                fm[0] += 1
        # epilogue: sel(7) + final combines (slots 56+ need all span-7 sins
        # emitted first; engine parallelism still overlaps them with the
        # last Act sins)
        emit_slot(56)
        emit_slot(57)
        emit_irfft(7)
        emit_slot(58)
        emit_slot(59)
        emit_combine(6)
        for sl in range(60, 64):
            emit_slot(sl)
        emit_combine(7)
    nc.finalize()
    _cache["nc"] = nc
    return nc


def _host_prep(inputs):
    st = _build_static()
    bf = ml_dtypes.bfloat16
    f0 = np.clip(np.asarray(inputs["f0"], np.float64), -0.5, 0.5)
    f0b = np.asarray(inputs["f0_baselines"], np.float64)
    erb = (0.108 * (f0b * NYQ) + 24.7) / NYQ
    f0v = np.clip(f0b + f0 * erb, 0.0, 1.0)
    f0n = MIN_F0 + f0v * F0_DIFF                                    # (B,16,64)

    # ---- harmonic rows: g' = 32*e + (o-1), o in 1..32, hfact = o+1 ----
    hfact = np.arange(2, 2 + NH, dtype=np.float64)                  # [2..33]
    frq = (f0n[:, :, None, :] * hfact[None, None, :, None] * 0.5)   # (B,16,32,64)
    frq = frq.reshape(B, NHRM, SEQ)

    coef = np.einsum("bgs,sc->bgc", frq, st["W64"])                 # (B,512,512)
    # reduce A (offset) and B (slope) coefs mod 1: basis funcs (1, j, j2hi,
    # j2lo) take integer values, so shifting A/B by integers moves the phase
    # by whole turns and leaves sin unchanged while keeping |phi| < 256.
    for hs in range(4):
        coef[:, :, (3 * hs + 0)::16] %= 1.0
        coef[:, :, (3 * hs + 1)::16] %= 1.0
    clog = np.zeros((B, NHRM, 32, 16))
    for m in range(32):
        for hs in range(4):
            base = 16 * m + 3 * hs
            clog[:, :, m, 4 * hs + 0] = coef[:, :, base + 0]
            clog[:, :, m, 4 * hs + 1] = coef[:, :, base + 1]
            clog[:, :, m, 4 * hs + 2] = coef[:, :, base + 2]
            clog[:, :, m, 4 * hs + 3] = coef[:, :, base + 2]
    splits = _split3(clog)
    # coefT7[64*(m%2) + 4*br + sp, (m//2)*640 + b*128 + glo]
    coefT7 = np.zeros((B, 128, 16 * 640), np.float16)
    for b4 in range(4):
        blkg = slice(128 * b4, 128 * (b4 + 1))
        for m in range(32):
            for sp in range(3):
                rows = 64 * (m % 2) + 4 * np.arange(16) + sp
                cols = (m // 2) * 640 + b4 * 128
                coefT7[:, rows, cols:cols + 128] = \
                    splits[sp][:, blkg, m, :].transpose(0, 2, 1)

    # ---- fund coefs in (sl,e) layout ----
    f0n_f = f0n * 0.5                                               # (B,16,64) turns
    fA = np.einsum("bes,sh->beh", f0n_f, st["Aq"])                  # (B,16,128)
    fB = np.einsum("bes,sh->beh", f0n_f, st["Bq"])
    fC = np.einsum("bes,sh->beh", f0n_f, st["Cq"])
    fA %= 1.0
    sA, sB, sC = _split3(fA), _split3(fB), _split3(fC)
    fcoef = np.zeros((B, 64, 512), np.float16)
    for t in range(8):
        ro = 32 * (t % 2)
        for sl in range(8):
            slot = 8 * t + sl
            p = 128 * (t // 2) + 16 * sl + np.arange(16)            # col index (per e)
            for hh in range(2):
                h = (1 + 2 * slot + hh) if slot < 63 else (0 if hh == 0 else 127)
                for sp in range(3):
                    fcoef[:, ro + 12 * hh + 0 + sp, p] = sA[sp][:, :, h]
                    fcoef[:, ro + 12 * hh + 3 + sp, p] = sB[sp][:, :, h]
                    fcoef[:, ro + 12 * hh + 6 + sp, p] = sC[sp][:, :, h]
                    fcoef[:, ro + 12 * hh + 9 + sp, p] = sC[sp][:, :, h]

    # ---- select weights (harmonics only) ----
    oe = np.clip(np.asarray(inputs["osc_env"], np.float64), 0, 1)   # (B,16,64)
    he = np.clip(np.asarray(inputs["harm_env"], np.float64), 0, 1)  # (B,16,32,64)
    env_node = (oe[:, :, None, :] * he).reshape(B, NHRM, SEQ)       # g'=32e+(o-1)
    selWh = np.zeros((B, 128, 4 * 2048), np.float32)
    eidx = np.arange(NHRM) // 32
    nodesL = np.concatenate([np.arange(63), [0]])
    nodesR = np.concatenate([np.arange(1, 64), [63]])
    for b4 in range(4):
        for glo in range(128):
            g = 128 * b4 + glo
            e = eidx[g]
            colsL = 2048 * b4 + 32 * np.arange(64) + e
            colsR = 2048 * b4 + 32 * np.arange(64) + 16 + e
            selWh[:, glo, colsL] = env_node[:, g, nodesL]
            selWh[:, glo, colsR] = env_node[:, g, nodesR]
    selWh = selWh.astype(bf)

    # ---- mix + fund env scalars ----
    ov = np.clip(np.asarray(inputs["overall_env"], np.float64), 0, 1)  # (B,16,64)
    envsc = np.zeros((B, 128, 32), np.float32)
    for t in range(8):
        for sl in range(8):
            slot = 8 * t + sl
            p = slice(16 * sl, 16 * (sl + 1))
            L, R = (slot, slot + 1) if slot < 63 else (0, 63)
            envsc[:, p, 4 * t + 0] = ov[:, :, L]
            envsc[:, p, 4 * t + 1] = ov[:, :, R] - ov[:, :, L]
            envsc[:, p, 4 * t + 2] = oe[:, :, L]
            envsc[:, p, 4 * t + 3] = oe[:, :, R] - oe[:, :, L]

    nf = np.asarray(inputs["noise_frames"], np.float32)             # (B,16,64,512)
    nfT = np.ascontiguousarray(
        nf.transpose(0, 3, 2, 1).reshape(B, WS, NFR)).astype(bf)    # [ws, s*16+e]

    nstd = np.clip(np.asarray(inputs["noise_std"], np.float64), 1e-12, 1.0) * F0_DIFF
    mean_fr = f0n.transpose(0, 2, 1).reshape(B, NFR)                # fr = s*16+e
    c2_fr = -0.5 / nstd.transpose(0, 2, 1).reshape(B, NFR) ** 2
    meanb = np.zeros((B, 128, 1056), np.float32)
    c2b = np.zeros((B, 128, 1056), np.float32)
    meanb[:, :, 16:1040] = mean_fr[:, None, :]
    c2b[:, :, 16:1040] = c2_fr[:, None, :]

    per_core = []
    for b in range(B):
        per_core.append(dict(
            coefT7=coefT7[b], selW=selWh[b], envsc=envsc[b], fcoef=fcoef[b],
            nfT=nfT[b], meanb=meanb[b].astype(bf), c2b=c2b[b].astype(bf),
            basis64=st["basis64"], fbasis=st["fbasis"],
            Cw=st["Cw"], Dc=st["Dc"], freq4=st["freq4"],
            Wc=st["Wc"], Wc63=st["Wc63"], W1m=st["W1m"], W1m63=st["W1m63"],
            P=st["P"], negI=st["negI"],
        ))
    return per_core


def _unshuffle(o):
    """[128, 2048] (sl,e)x(t,j) -> [16, 16384]."""
    full = np.zeros((NE, N), np.float32)
    for t in range(8):
        for sl in range(8):
            slot = 8 * t + sl
            r = o[16 * sl:16 * (sl + 1), 256 * t:256 * (t + 1)]
            if slot < 63:
                full[:, 128 + 256 * slot:128 + 256 * slot + 256] = r
            else:
                full[:, 0:128] = r[:, 0:128]
                full[:, 16256:16384] = r[:, 128:256]
    return full


def kernel(**inputs):
    from concourse.bass_utils import run_bass_kernel_spmd
    per_core = _host_prep(inputs)
    nc = _build_nc()
    trace = bool(os.environ.get("BASS_PROFILE"))
    res = run_bass_kernel_spmd(nc, per_core, list(range(B)), trace=trace)
    if trace and res.exec_time_ns is not None:
        print(f"HW exec time: {res.exec_time_ns} ns")
    out = np.stack([_unshuffle(np.asarray(r["out"], np.float32))
                    for r in res.results])
    return out


# revision 43
# speedup vs baseline: 1.8095x; 1.0036x over previous
"""Trainium2 Bass kernel v3 for nn_AudioEvent.

Per-core pipeline (batch-parallel over 8 cores):
  harmonics: host quadratic phase coeffs (A pre-reduced mod 1 so |phase|<256)
        -> stage2 matmuls (f16 3-split, exact) -> frac via dual-op round
        (kt=(phi+2^23)-2^23 in one DVE op, bf16) -> PE -I matmul accumulates
        -kt into psum -> Sin from psum (Act) -> env-folded select matmuls
        (512 harmonics = exactly 4 contraction blocks) -> ramp mult ->
        pairsum matmul -> osc psum in (slot,e) layout
  fundamentals: computed directly in (slot,e)-partition layout [128,256]
        per t-tile: tiny quad-coef matmul -> frac -> Sin -> env interp via
        dual-op tensor_scalar with per-partition node scalars
  noise: windowed rDFT matmuls -> gaussian filter -> irfft+overlap-add
        matmuls into (slot,e) layout
  mix:  dual-op piecewise-linear; out [128,2048] bf16, host unshuffles
"""
import os
import numpy as np
import ml_dtypes

B, NE, NH, SEQ, N, WS = 8, 16, 32, 64, 16384, 512
NYQ = 11025.0
MIN_F0 = np.float64(20.0 / NYQ)
MAX_F0 = np.float64(800.0 / NYQ)
F0_DIFF = MAX_F0 - MIN_F0
NHRM = 512                # 16 events x 32 harmonics
NFR = SEQ * NE            # 1024
C23 = float(1.5 * 2.0 ** 23)  # round magic: x+C in [2^23,2^24) ulp-1 zone

_cache = {}

# per-block frac variant: v2 = DVE kt + PE sub; v3 = Act yt + Pool kt2 + PE sub;
# v1 = DVE kt + DVE sub (sbuf fr, block sin)
_VAR = []
for _i in range(32):
    if _i in (2, 6, 10, 14, 18, 22, 26, 30) and not os.environ.get("ALL_V2"):
        _VAR.append("v3")
    else:
        _VAR.append("v2")
# pairs (block, q=1) computed DVE-sub + sbuf-sin instead of PE -I + psum-sin
_V1P = {4 * m + 1 for m in range(8)}


def _quad_halfseg(V):
    """Per 128-sample half-segment quadratic coefs of V (cumsum weights).
    V: [SEQ, N] -> (A, Bc, C) each [SEQ, 128]."""
    H = N // 128
    A = V[:, 0::128][:, :H]
    C = (V[:, 2::128][:, :H] - 2 * V[:, 1::128][:, :H] + A) / 2
    Bc = V[:, 1::128][:, :H] - A - C
    return A, Bc, C


def _build_static():
    if "static" in _cache:
        return _cache["static"]
    # ---- interp weight cumsum V and quadratic coeffs ----
    pos = (np.arange(N, dtype=np.float64) + 0.5) * (SEQ / N) - 0.5
    pos = np.clip(pos, 0.0, SEQ - 1)
    i0 = np.floor(pos).astype(np.int64)
    i1 = np.minimum(i0 + 1, SEQ - 1)
    w = pos - i0
    U = np.zeros((SEQ, N))
    U[i0, np.arange(N)] += 1.0 - w
    U[i1, np.arange(N)] += w
    V = np.cumsum(U, axis=1)
    # harmonic chunk coefs: W64[s, 16m+3hs+k] (k=A,B,C) for chunk (m,hs)
    W64 = np.zeros((SEQ, 512))
    for m in range(32):
        for hs in range(4):
            t0 = 512 * m + 128 * hs
            A = V[:, t0]
            C = (V[:, t0 + 2] - 2 * V[:, t0 + 1] + V[:, t0]) / 2
            Bc = V[:, t0 + 1] - V[:, t0] - C
            W64[:, 16 * m + 3 * hs + 0] = A
            W64[:, 16 * m + 3 * hs + 1] = Bc
            W64[:, 16 * m + 3 * hs + 2] = C
    # fund: per half-seg quad coefs
    Aq, Bq, Cq = _quad_halfseg(V)          # [SEQ, 128]

    # ---- stage2 basis (f16 split): rows 64*rep + 4*br + sp ----
    j = np.arange(128, dtype=np.float64)
    b16s = np.zeros((16, 512))
    for hs in range(4):
        sl = slice(128 * hs, 128 * (hs + 1))
        b16s[4 * hs + 0, sl] = 1.0
        b16s[4 * hs + 1, sl] = j
        j2h = np.float16(j * j).astype(np.float64)
        b16s[4 * hs + 2, sl] = j2h
        b16s[4 * hs + 3, sl] = j * j - j2h
    basis64 = np.zeros((128, 512), np.float16)
    for rep in range(2):
        for br in range(16):
            for sp in range(3):
                basis64[64 * rep + 4 * br + sp] = np.float16(b16s[br])

    # ---- fund basis: rows 12*hh + 3*bf + sp; block-diag pair [64, 512] ----
    fbasis1 = np.zeros((32, 256), np.float16)
    for hh in range(2):
        sl = slice(128 * hh, 128 * (hh + 1))
        j2h = np.float16(j * j).astype(np.float64)
        for sp in range(3):
            fbasis1[12 * hh + 0 + sp, sl] = 1.0
            fbasis1[12 * hh + 3 + sp, sl] = np.float16(j)
            fbasis1[12 * hh + 6 + sp, sl] = np.float16(j2h)
            fbasis1[12 * hh + 9 + sp, sl] = np.float16(j * j - j2h)
    fbasis = np.zeros((64, 512), np.float16)
    fbasis[0:32, 0:256] = fbasis1
    fbasis[32:64, 256:512] = fbasis1

    # ---- DFT consts (win folded), 4 contraction groups ----
    t = np.arange(WS)
    f = np.arange(WS // 2 + 1)
    win = 0.5 - 0.5 * np.cos(2 * np.pi * t / WS)
    ang = 2 * np.pi * np.outer(t, f) / WS
    CwRe = np.cos(ang) * win[:, None]
    CwIm = -np.sin(ang) * win[:, None]
    wgt = np.full(WS // 2 + 1, 2.0)
    wgt[0] = 1.0
    wgt[-1] = 1.0
    ang2 = 2 * np.pi * np.outer(f, t) / WS
    DRe = wgt[:, None] * np.cos(ang2) / WS
    DIm = -wgt[:, None] * np.sin(ang2) / WS
    Cw = np.zeros((128, 2048))       # col = tc*512 + grp*128 + fcol
    Dc = np.zeros((128, 2048))       # col = grp*512 + tau
    freq4 = np.zeros((128, 4))
    for grp in range(4):
        if grp == 0:
            fidx, mats = np.arange(0, 128), CwRe
        elif grp == 1:
            fidx, mats = np.arange(128, 256), CwRe
        elif grp == 2:
            fidx, mats = np.arange(1, 129), CwIm
        else:
            fidx, mats = np.concatenate([np.arange(129, 256), [256]]), None
        for tc in range(4):
            trows = slice(128 * tc, 128 * (tc + 1))
            if grp < 3:
                Cw[:, tc * 512 + grp * 128: tc * 512 + grp * 128 + 128] = mats[trows][:, fidx]
            else:
                blockm = CwIm[trows][:, fidx[:-1]]
                Cw[:, tc * 512 + grp * 128: tc * 512 + grp * 128 + 127] = blockm
                Cw[:, tc * 512 + grp * 128 + 127] = CwRe[trows][:, 256]
        if grp < 3:
            Dc[:, grp * 512:(grp + 1) * 512] = (DRe if grp < 2 else DIm)[fidx]
            freq4[:, grp] = fidx / 256.0
        else:
            Dc[:127, grp * 512:(grp + 1) * 512] = DIm[fidx[:-1]]
            Dc[127, grp * 512:(grp + 1) * 512] = DRe[256]
            freq4[:127, grp] = fidx[:-1] / 256.0
            freq4[127, grp] = 1.0

    # ---- ramp consts ----
    wj = (np.arange(256) + 0.5) / 256.0
    Wc = np.zeros((128, 256))
    for p in range(128):
        Wc[p] = wj if (p // 16) % 2 == 1 else 1.0 - wj
    Wc63 = Wc.copy()
    Wc63[96:112] = np.concatenate([np.ones(128), np.zeros(128)])
    Wc63[112:128] = np.concatenate([np.zeros(128), np.ones(128)])
    W1m = np.tile(wj, (128, 1))
    W1m63 = W1m.copy()
    W1m63[112:128] = np.concatenate([np.zeros(128), np.ones(128)])
    P = np.zeros((128, 64))
    for p in range(128):
        P[p, 16 * (p // 32) + p % 16] = 1.0
    negI = (-np.eye(128)).astype(np.float64)

    bf = ml_dtypes.bfloat16
    static = dict(
        W64=W64, basis64=basis64, Aq=Aq, Bq=Bq, Cq=Cq, fbasis=fbasis,
        Cw=Cw.astype(bf), Dc=Dc.astype(bf), freq4=freq4.astype(np.float32),
        Wc=Wc.astype(bf), Wc63=Wc63.astype(bf),
        W1m=W1m.astype(bf), W1m63=W1m63.astype(bf),
        P=P.astype(bf), negI=negI.astype(bf),
    )
    _cache["static"] = static
    return static


def _split3(x):
    """3-way fp16 split of float64 array: returns list of 3 fp16 arrays."""
    h0 = x.astype(np.float16).astype(np.float64)
    h1 = (x - h0).astype(np.float16).astype(np.float64)
    h2 = (x - h0 - h1).astype(np.float16)
    return [h0.astype(np.float16), h1.astype(np.float16), h2]


def _build_nc():
    if "nc" in _cache:
        return _cache["nc"]
    from concourse import bacc
    import concourse.tile as tile
    from concourse import mybir
    from contextlib import ExitStack

    F32 = mybir.dt.float32
    F16 = mybir.dt.float16
    BF16 = mybir.dt.bfloat16
    AF = mybir.ActivationFunctionType
    OP = mybir.AluOpType
    PI = float(np.pi)

    nc = bacc.Bacc()
    nc._dbg = {}
    # data params
    coefT7 = nc.declare_dram_parameter("coefT7", [128, 16 * 640], F16, isOutput=False)
    selW = nc.declare_dram_parameter("selW", [128, 4 * 2048], BF16, isOutput=False)
    envsc = nc.declare_dram_parameter("envsc", [128, 32], F32, isOutput=False)
    fcoef = nc.declare_dram_parameter("fcoef", [64, 512], F16, isOutput=False)
    nfT = nc.declare_dram_parameter("nfT", [512, 1024], BF16, isOutput=False)
    meanb = nc.declare_dram_parameter("meanb", [128, 1056], BF16, isOutput=False)
    c2b = nc.declare_dram_parameter("c2b", [128, 1056], BF16, isOutput=False)
    # const params
    basis64 = nc.declare_dram_parameter("basis64", [128, 512], F16, isOutput=False)
    fbasis = nc.declare_dram_parameter("fbasis", [64, 512], F16, isOutput=False)
    Cw = nc.declare_dram_parameter("Cw", [128, 2048], BF16, isOutput=False)
    Dc = nc.declare_dram_parameter("Dc", [128, 2048], BF16, isOutput=False)
    freq4 = nc.declare_dram_parameter("freq4", [128, 4], F32, isOutput=False)
    Wc = nc.declare_dram_parameter("Wc", [128, 256], BF16, isOutput=False)
    Wc63 = nc.declare_dram_parameter("Wc63", [128, 256], BF16, isOutput=False)
    W1m = nc.declare_dram_parameter("W1m", [128, 256], BF16, isOutput=False)
    W1m63 = nc.declare_dram_parameter("W1m63", [128, 256], BF16, isOutput=False)
    P = nc.declare_dram_parameter("P", [128, 64], BF16, isOutput=False)
    negI = nc.declare_dram_parameter("negI", [128, 128], BF16, isOutput=False)
    out = nc.declare_dram_parameter("out", [128, 2048], BF16, isOutput=True)

    with tile.TileContext(nc) as tc, ExitStack() as ctx:
        cp = ctx.enter_context(tc.tile_pool(name="const", bufs=1))

        # ---- const DMAs (SP queue), ordered by first need ----
        meanb_sb = cp.tile([128, 1056], BF16, tag="meanb")
        nc.gpsimd.dma_start(meanb_sb[:], meanb[:])
        c2b_sb = cp.tile([128, 1056], BF16, tag="c2b")
        nc.gpsimd.dma_start(c2b_sb[:], c2b[:])
        basis64_sb = cp.tile([128, 512], F16, tag="basis64")
        nc.sync.dma_start(basis64_sb[:], basis64[:])
        freq4_sb = cp.tile([128, 4], F32, tag="freq4")
        nc.sync.dma_start(freq4_sb[:], freq4[:])
        coefT7_sb = cp.tile([128, 16 * 640], F16, tag="coefT7")
        nc.sync.dma_start(coefT7_sb[:, 0:1280], coefT7[:, 0:1280])
        negI_sb = cp.tile([128, 128], BF16, tag="negI")
        nc.sync.dma_start(negI_sb[:], negI[:])
        nc.sync.dma_start(coefT7_sb[:, 1280:2560], coefT7[:, 1280:2560])
        fbasis_sb = cp.tile([64, 512], F16, tag="fbasis")
        nc.sync.dma_start(fbasis_sb[:], fbasis[:])
        fcoef_sb = cp.tile([64, 512], F16, tag="fcoef")
        nc.sync.dma_start(fcoef_sb[:], fcoef[:])
        W1m_sb = cp.tile([128, 256], BF16, tag="W1m")
        nc.sync.dma_start(W1m_sb[:], W1m[:])
        W1m63_sb = cp.tile([128, 256], BF16, tag="W1m63")
        nc.sync.dma_start(W1m63_sb[:], W1m63[:])
        P_sb = cp.tile([128, 64], BF16, tag="P")
        nc.sync.dma_start(P_sb[:], P[:])
        envsc_sb = cp.tile([128, 32], F32, tag="envsc")
        nc.sync.dma_start(envsc_sb[:], envsc[:])
        Wc_sb = cp.tile([128, 256], BF16, tag="Wc")
        nc.sync.dma_start(Wc_sb[:], Wc[:])
        Wc63_sb = cp.tile([128, 256], BF16, tag="Wc63")
        nc.sync.dma_start(Wc63_sb[:], Wc63[:])
        # selW in slot-quarters so early selects unblock sooner
        selW_sb = cp.tile([128, 4 * 2048], BF16, tag="selW")
        for b4 in range(4):           # slots 0..15
            nc.sync.dma_start(selW_sb[:, 2048 * b4:2048 * b4 + 512],
                              selW[:, 2048 * b4:2048 * b4 + 512])
        nfT_sb = [cp.tile([128, 1024], BF16, tag=f"nfT{i}", name=f"nfT{i}") for i in range(4)]
        for i in range(4):
            nc.sync.dma_start(nfT_sb[i][:], nfT[128 * i:128 * (i + 1), :])
        Cw_sb = cp.tile([128, 2048], BF16, tag="Cw")
        nc.sync.dma_start(Cw_sb[:], Cw[:])
        Dc_sb = cp.tile([128, 2048], BF16, tag="Dc")
        nc.sync.dma_start(Dc_sb[:], Dc[:])
        for cch in (2, 3):
            nc.sync.dma_start(coefT7_sb[:, 1280 * cch:1280 * (cch + 1)],
                              coefT7[:, 1280 * cch:1280 * (cch + 1)])
        for b4 in range(4):           # slots 16..39
            nc.sync.dma_start(selW_sb[:, 2048 * b4 + 512:2048 * b4 + 1280],
                              selW[:, 2048 * b4 + 512:2048 * b4 + 1280])
        for cch in (4, 5):
            nc.sync.dma_start(coefT7_sb[:, 1280 * cch:1280 * (cch + 1)],
                              coefT7[:, 1280 * cch:1280 * (cch + 1)])
        for b4 in range(4):           # slots 40..63
            nc.sync.dma_start(selW_sb[:, 2048 * b4 + 1280:2048 * b4 + 2048],
                              selW[:, 2048 * b4 + 1280:2048 * b4 + 2048])
        for cch in (6, 7):
            nc.sync.dma_start(coefT7_sb[:, 1280 * cch:1280 * (cch + 1)],
                              coefT7[:, 1280 * cch:1280 * (cch + 1)])
        c23b = cp.tile([128, 1], F32, tag="c23b")
        nc.vector.memset(c23b[:], C23)

        spec_sb = [cp.tile([128, 1056], BF16, tag=f"spec{g}", name=f"spec{g}") for g in range(4)]
        noise_sb = [cp.tile([128, 256], BF16, tag=f"nz{t}", name=f"nz{t}") for t in range(8)]
        st4f = [cp.tile([128, 512], BF16, tag=f"st4fp{u}", name=f"st4fp{u}") for u in range(4)]
        stHT = cp.tile([128, 512], BF16, tag="stHT")

        # ---- pools ----
        php = ctx.enter_context(tc.tile_pool(name="phi", bufs=2))      # v1 fr tiles
        stp = ctx.enter_context(tc.tile_pool(name="st", bufs=3))
        rtp = ctx.enter_context(tc.tile_pool(name="rt", bufs=4))       # kt bf16 pairs
        ytp = ctx.enter_context(tc.tile_pool(name="yt", bufs=2))       # v3 yt f32 pairs
        awp = ctx.enter_context(tc.tile_pool(name="aw", bufs=2))
        fin = ctx.enter_context(tc.tile_pool(name="fin", bufs=2))
        fpool = ctx.enter_context(tc.tile_pool(name="fp", bufs=1))
        # PSUM: ph ring 3x[128,1024] (6 banks) + A-pair (1) + osc-pair (1)
        psPH = ctx.enter_context(tc.tile_pool(name="psPH", bufs=3, space="PSUM"))
        psSel = ctx.enter_context(tc.tile_pool(name="psSel", bufs=1, space="PSUM"))
        psOsc = ctx.enter_context(tc.tile_pool(name="psOsc", bufs=1, space="PSUM"))

        st_tiles = {}

        # ============ noise filter pre-pass (Act: exps before any sin) ======
        for g in range(4):
            nc.gpsimd.memset(spec_sb[g][:, 0:16], 0.0)
            nc.gpsimd.memset(spec_sb[g][:, 1040:1056], 0.0)
        filt_t = [None] * 4
        dts = {}
        for g in (0, 1):
            d = fpool.tile([128, 1056], BF16, tag=f"fdt{g}", name=f"fd{g}")
            nc.gpsimd.tensor_scalar(d[:], meanb_sb[:], freq4_sb[:, g:g + 1], None,
                                    OP.subtract)
            dts[g] = d
        for g in (2, 3):
            d = fpool.tile([128, 1056], BF16, tag=f"fdt{g}", name=f"fd{g}")
            nc.vector.tensor_scalar(d[:], meanb_sb[:], freq4_sb[:, g:g + 1], None,
                                    OP.subtract)
            dts[g] = d
        # PE p-state warm-up: dummy accumulations while DMAs stream in
        warm = psPH.tile([128, 1024], F32, tag="ph", name="warm")
        for i in range(18):
            nc.tensor.matmul(warm[:, 0:512], basis64_sb[0:64, 0:128],
                             basis64_sb[0:64, :], start=(i == 0), stop=(i == 17),
                             skip_group_check=True, tile_position=(0, 0))
        dd = {}
        for g in (0, 1):
            d2 = fpool.tile([128, 1056], BF16, tag=f"fd2t{g}", name=f"fd2{g}")
            nc.gpsimd.tensor_tensor(d2[:], dts[g][:], dts[g][:], OP.mult)
            dd[g] = d2
        for g in (2, 3, 0, 1):
            if g in (2, 3):
                d2 = fpool.tile([128, 1056], BF16, tag=f"fd2t{g}", name=f"fd2{g}")
                nc.vector.tensor_tensor(d2[:], dts[g][:], dts[g][:], OP.mult)
            else:
                d2 = dd[g]
            m2 = fpool.tile([128, 1056], BF16, tag=f"fm2t{g}", name=f"fm2{g}")
            nc.vector.tensor_tensor(m2[:], d2[:], c2b_sb[:], OP.mult)
            filt = fpool.tile([128, 1056], BF16, tag=f"filt{g}", name=f"filt{g}")
            nc.scalar.activation(filt[:], m2[:], AF.Exp)
            filt_t[g] = filt

        # ============ span machinery (pair-granular, software-pipelined) ====
        pair_state = {}

        def emit_F(m, b, q):
            """stage2 matmuls + round (kt) for pair q of block (m, b)."""
            var = _VAR[4 * m + b]
            if var == "v2" and q == 1 and (4 * m + b) in _V1P:
                var = "v1p"
            if q == 0:
                st = stp.tile([128, 2048], BF16, tag=f"st{b}", name=f"st{b}_{m}")
                st_tiles[(b, m)] = st
            pp = psPH.tile([128, 1024], F32, tag="ph", name=f"pp{m}_{b}_{q}")
            for h in range(2):
                k = 2 * q + h
                mm = 4 * m + k
                p2 = 64 * (mm % 2)
                cb = (mm // 2) * 640 + b * 128
                nc.tensor.matmul(pp[:, 512 * h:512 * (h + 1)],
                                 coefT7_sb[p2:p2 + 64, cb:cb + 128],
                                 basis64_sb[p2:p2 + 64, :], start=True, stop=True,
                                 skip_group_check=True, tile_position=(p2, 0))
            kt = rtp.tile([128, 1024], BF16, tag="kt", name=f"kt{m}_{b}_{q}")
            if var == "v3":
                yt = ytp.tile([128, 1024], F32, tag="yt", name=f"yt{m}_{b}_{q}")
                nc.scalar.activation(yt[:], pp[:], AF.Identity, bias=c23b[:, 0:1])
                nc.gpsimd.tensor_scalar(kt[:], yt[:], C23, None, OP.subtract)
            else:
                nc.vector.tensor_scalar(kt[:], pp[:], C23, C23, OP.add, OP.subtract)
            pair_state[(b, q)] = (var, pp, kt)

        def emit_D(m, b, q):
            """-I accumulate + Sin (or DVE sub for v1p) for pair q."""
            var, pp, kt = pair_state.pop((b, q))
            st = st_tiles[(b, m)]
            if var == "v1p":
                fr = php.tile([128, 1024], F32, tag="fr", name=f"fr{b}_{m}_{q}")
                nc.vector.tensor_tensor(fr[:], pp[:], kt[:], OP.subtract)
                nc.scalar.activation(st[:, 1024 * q:1024 * (q + 1)], fr[:],
                                     AF.Sin, scale=2 * PI)
            else:
                for h in range(2):
                    nc.tensor.matmul(pp[:, 512 * h:512 * (h + 1)], negI_sb[:],
                                     kt[:, 512 * h:512 * (h + 1)],
                                     start=False, stop=True,
                                     skip_group_check=True, tile_position=(0, 0))
                nc.scalar.activation(st[:, 1024 * q:1024 * (q + 1)], pp[:],
                                     AF.Sin, scale=2 * PI)

        def span_sched(m):
            ops = []
            pend = []
            for b in range(4):
                for q in range(2):
                    ops.append(("F", b, q))
                    pend.append((b, q))
                    if len(pend) > 2:
                        ops.append(("D",) + pend.pop(0))
            while pend:
                ops.append(("D",) + pend.pop(0))
            return ops

        def emit_fund_pair(u):
            pf = psPH.tile([128, 1024], F32, tag="ph", name=f"fph{u}")
            nc.tensor.matmul(pf[:, 0:512], fcoef_sb[:, 128 * u:128 * (u + 1)],
                             fbasis_sb[:], start=True, stop=True,
                             skip_group_check=True, tile_position=(0, 0))
            kt = rtp.tile([128, 1024], BF16, tag="kt", name=f"fkt{u}")
            nc.vector.tensor_scalar(kt[:, 0:512], pf[:, 0:512], C23, C23,
                                    OP.add, OP.subtract)
            nc.tensor.matmul(pf[:, 0:512], negI_sb[:], kt[:, 0:512],
                             start=False, stop=True, skip_group_check=True,
                             tile_position=(0, 0))
            nc.scalar.activation(st4f[u][:], pf[:, 0:512], AF.Sin, scale=2 * PI)

        def sel_windows(slot):
            t0 = 128 + 256 * slot
            m = t0 // 2048
            lo = t0 - 2048 * m
            if lo + 256 <= 2048:
                return [(m, lo, lo + 256, 0)]
            return [(m, lo, 2048, 0), (m + 1, 0, lo + 256 - 2048, 2048 - lo)]

        A_pairs = {}

        def emit_slot(slot):
            at4 = slot // 4
            v = at4 // 2
            if v not in A_pairs:
                A_pairs[v] = psSel.tile([128, 512], F32, tag="A", name=f"Ap{v}")
            ao = 256 * (at4 % 2)
            A = A_pairs[v]
            sl4 = slot % 4
            first = [True]
            if slot < 63:
                cws = sel_windows(slot)
                nmm = 4 * len(cws)
                i = 0
                for b in range(4):
                    for (mw, lo, hi, plo) in cws:
                        i += 1
                        nc.tensor.matmul(
                            A[32 * sl4:32 * sl4 + 32, ao + plo:ao + plo + hi - lo],
                            selW_sb[:, 2048 * b + 32 * slot: 2048 * b + 32 * slot + 32],
                            st_tiles[(b, mw)][:, lo:hi],
                            start=first[0], stop=(i == nmm),
                            skip_group_check=True, tile_position=(0, 32 * sl4))
                        first[0] = False
            else:
                for b in range(4):
                    nc.tensor.matmul(
                        A[96:128, ao:ao + 128],
                        selW_sb[:, 2048 * b + 32 * 63: 2048 * b + 32 * 63 + 32],
                        stHT[:, 128 * b:128 * (b + 1)],
                        start=(b == 0), stop=False,
                        skip_group_check=True, tile_position=(0, 96))
                    nc.tensor.matmul(
                        A[96:128, ao + 128:ao + 256],
                        selW_sb[:, 2048 * b + 32 * 63: 2048 * b + 32 * 63 + 32],
                        st_tiles[(b, 7)][:, 1920:2048],
                        start=False, stop=(b == 3),
                        skip_group_check=True, tile_position=(0, 96))
            if slot % 4 == 3:
                emit_atile_done(at4)

        osc_pairs = {}

        def emit_atile_done(at4):
            v = at4 // 2
            A = A_pairs[v]
            ao = 256 * (at4 % 2)
            if at4 % 2 == 1:
                A_pairs.pop(v)
            aw = awp.tile([128, 256], BF16, tag="aw")
            wc = Wc63_sb if at4 == 15 else Wc_sb
            nc.vector.tensor_tensor(aw[:], A[:, ao:ao + 256], wc[:], OP.mult)
            t = at4 // 2
            u = t // 2
            if u not in osc_pairs:
                osc_pairs[u] = psOsc.tile([128, 512], F32, tag="osc", name=f"oscp{u}")
            oo = 256 * (t % 2)
            nc.tensor.matmul(
                osc_pairs[u][64 * (at4 % 2):64 * (at4 % 2) + 64, oo:oo + 256],
                P_sb[:], aw[:], start=True, stop=True,
                skip_group_check=True, tile_position=(0, 64 * (at4 % 2)))

        def emit_rfft(g, h):
            fr_sl = slice(512 * h, 512 * (h + 1))
            sp = psPH.tile([128, 1024], F32, tag="ph", name=f"rf{g}_{h}")
            for tcx in range(4):
                nc.tensor.matmul(sp[:, 0:512],
                                 Cw_sb[:, tcx * 512 + g * 128: tcx * 512 + g * 128 + 128],
                                 nfT_sb[tcx][:, fr_sl],
                                 start=(tcx == 0), stop=(tcx == 3))
            nc.vector.tensor_tensor(spec_sb[g][:, 16 + 512 * h:16 + 512 * (h + 1)],
                                    sp[:, 0:512], filt_t[g][:, 16 + 512 * h:16 + 512 * (h + 1)],
                                    OP.mult)

        def emit_irfft(t):
            pz = psPH.tile([128, 1024], F32, tag="ph", name=f"nzps{t}")
            base = 16 + 16 * (8 * t)
            nslots = 7 if t == 7 else 8
            ncols = 16 * nslots
            for g in range(4):
                gD = Dc_sb[:, 512 * g: 512 * (g + 1)]
                sW = spec_sb[g]
                nc.tensor.matmul(pz[0:ncols, 0:256], sW[:, base:base + ncols],
                                 gD[:, 128:384], start=(g == 0), stop=False,
                                 skip_group_check=True)
                nc.tensor.matmul(pz[0:ncols, 0:128], sW[:, base - 16:base - 16 + ncols],
                                 gD[:, 384:512], start=False, stop=False,
                                 skip_group_check=True)
                nc.tensor.matmul(pz[0:ncols, 128:256], sW[:, base + 16:base + 16 + ncols],
                                 gD[:, 0:128], start=False,
                                 stop=(t < 7 and g == 3),
                                 skip_group_check=True)
            if t == 7:
                b63 = 16 + 16 * 63
                pz63 = psPH.tile([32, 512], F32, tag="ph", name="nz63")
                for g in range(4):
                    gD = Dc_sb[:, 512 * g: 512 * (g + 1)]
                    sW = spec_sb[g]
                    nc.tensor.matmul(pz63[0:16, 0:128], sW[:, 16:32],
                                     gD[:, 0:128], start=(g == 0), stop=False,
                                     skip_group_check=True, tile_position=(0, 0))
                    nc.tensor.matmul(pz63[0:16, 128:256], sW[:, b63:b63 + 16],
                                     gD[:, 128:256], start=False, stop=False,
                                     skip_group_check=True, tile_position=(0, 0))
                    nc.tensor.matmul(pz63[0:16, 128:256], sW[:, b63 - 16:b63],
                                     gD[:, 384:512], start=False, stop=(g == 3),
                                     skip_group_check=True, tile_position=(0, 0))
                nc.vector.tensor_copy(noise_sb[t][0:112, :], pz[0:112, 0:256])
                nz63s = fpool.tile([16, 256], BF16, tag="nz63s")
                nc.vector.tensor_copy(nz63s[:], pz63[0:16, 0:256])
                nc.sync.dma_start(noise_sb[t][112:128, :], nz63s[:])
            else:
                nc.vector.tensor_copy(noise_sb[t][:], pz[:, 0:256])

        def emit_combine(t):
            u = t // 2
            osc = osc_pairs[u][:, 256 * (t % 2):256 * (t % 2) + 256]
            if t % 2 == 1:
                osc_pairs.pop(u)
            w1 = W1m63_sb if t == 7 else W1m_sb
            mixT = fin.tile([128, 256], BF16, tag="mixT")
            nc.gpsimd.tensor_scalar(mixT[:], w1[:], envsc_sb[:, 4 * t + 1:4 * t + 2],
                                    envsc_sb[:, 4 * t + 0:4 * t + 1], OP.mult, OP.add)
            mixF = fin.tile([128, 256], BF16, tag="mixF")
            nc.gpsimd.tensor_scalar(mixF[:], w1[:], envsc_sb[:, 4 * t + 3:4 * t + 4],
                                    envsc_sb[:, 4 * t + 2:4 * t + 3], OP.mult, OP.add)
            fc = fin.tile([128, 256], BF16, tag="fc")
            nc.gpsimd.tensor_tensor(fc[:], st4f[t // 2][:, 256 * (t % 2):256 * (t % 2) + 256], mixF[:], OP.mult)
            d = fin.tile([128, 256], BF16, tag="d")
            nc.vector.tensor_tensor(d[:], osc, noise_sb[t][:], OP.subtract)
            d2 = fin.tile([128, 256], BF16, tag="d2")
            nc.gpsimd.tensor_tensor(d2[:], d[:], fc[:], OP.add)
            mres = fin.tile([128, 256], BF16, tag="mres")
            nc.gpsimd.tensor_tensor(mres[:], d2[:], mixT[:], OP.mult)
            r = fin.tile([128, 256], BF16, tag="r")
            nc._dbg[f"r{t}"] = r
            nc.gpsimd.tensor_tensor(r[:], mres[:], noise_sb[t][:], OP.add)
            nc.sync.dma_start(out[:, 256 * t:256 * (t + 1)], r[:])

        # ============ interleaved drive (global pipelined stream) ============
        def mkslot(sl):
            return lambda: emit_slot(sl)

        def stht_copy():
            for b in range(4):
                nc.gpsimd.tensor_copy(stHT[:, 128 * b:128 * (b + 1)],
                                      st_tiles[(b, 0)][:, 0:128])

        fillers = {m: [] for m in range(8)}
        fillers[0] = [stht_copy]
        fillers[1] = [lambda: emit_rfft(0, 0), lambda: emit_rfft(0, 1),
                      lambda: emit_rfft(1, 0), lambda: emit_rfft(1, 1),
                      mkslot(0), lambda: emit_rfft(2, 0), mkslot(1),
                      lambda: emit_rfft(2, 1), mkslot(2), lambda: emit_rfft(3, 0),
                      mkslot(3), lambda: emit_rfft(3, 1), mkslot(4),
                      lambda: emit_fund_pair(0), mkslot(5),
                      mkslot(6), mkslot(7), lambda: emit_irfft(0)]
        for m in range(2, 8):
            base_s = 8 * (m - 1)
            fl = [(lambda t: lambda: emit_combine(t))(m - 2)]
            if m in (3, 5, 7):
                fl.append((lambda u: lambda: emit_fund_pair(u))(m // 2))
            fl += [mkslot(base_s + i) for i in range(6)]
            fl.append((lambda t: lambda: emit_irfft(t))(m - 1))
            fl += [mkslot(base_s + 6), mkslot(base_s + 7)]
            fillers[m] = fl

        allp = [(m, b, q) for m in range(8) for b in range(4) for q in range(2)]
        ops = []
        pend = []
        for pr in allp:
            ops.append(("F",) + pr)
            pend.append(pr)
            if len(pend) > 3:
                ops.append(("D",) + pend.pop(0))
        while pend:
            ops.append(("D",) + pend.pop(0))

        from collections import Counter
        cnt = Counter(op[1] for op in ops)
        seen = Counter()
        fidx = {m: 0 for m in fillers}
        fm = [0]

        def pump():
            # strictly ordered filler groups; pace group fm by span-fm progress
            while fm[0] < 8:
                fl = fillers[fm[0]]
                if fidx[fm[0]] >= len(fl):
                    if seen[fm[0]] >= cnt[fm[0]]:
                        fm[0] += 1
                        continue
                    break
                frac = seen[fm[0]] / cnt[fm[0]]
                if (fidx[fm[0]] + 1) / len(fl) <= frac or seen[fm[0]] >= cnt[fm[0]]:
                    fl[fidx[fm[0]]]()
                    fidx[fm[0]] += 1
                else:
                    break

        for op in ops:
            if op[0] == "F":
                emit_F(op[1], op[2], op[3])
            else:
                emit_D(op[1], op[2], op[3])
            seen[op[1]] += 1
            pump()
        while fm[0] < 8:
            fl = fillers[fm[0]]
            if fidx[fm[0]] < len(fl):
                fl[fidx[fm[0]]]()
                fidx[fm[0]] += 1
            else:
                fm[0] += 1
        # epilogue: sel(7) + final combines (slots 56+ need all span-7 sins
        # emitted first; engine parallelism still overlaps them with the
        # last Act sins)
        emit_slot(56)
        emit_slot(57)
        emit_irfft(7)
        emit_slot(58)
        emit_slot(59)
        emit_combine(6)
        for sl in range(60, 64):
            emit_slot(sl)
        emit_combine(7)
    nc.finalize()
    _cache["nc"] = nc
    return nc


def _host_prep(inputs):
    st = _build_static()
    bf = ml_dtypes.bfloat16
    f0 = np.clip(np.asarray(inputs["f0"], np.float64), -0.5, 0.5)
    f0b = np.asarray(inputs["f0_baselines"], np.float64)
    erb = (0.108 * (f0b * NYQ) + 24.7) / NYQ
    f0v = np.clip(f0b + f0 * erb, 0.0, 1.0)
    f0n = MIN_F0 + f0v * F0_DIFF                                    # (B,16,64)

    # ---- harmonic rows: g' = 32*e + (o-1), o in 1..32, hfact = o+1 ----
    hfact = np.arange(2, 2 + NH, dtype=np.float64)                  # [2..33]
    frq = (f0n[:, :, None, :] * hfact[None, None, :, None] * 0.5)   # (B,16,32,64)
    frq = frq.reshape(B, NHRM, SEQ)

    coef = np.einsum("bgs,sc->bgc", frq, st["W64"])                 # (B,512,512)
    # reduce A (offset) and B (slope) coefs mod 1: basis funcs (1, j, j2hi,
    # j2lo) take integer values, so shifting A/B by integers moves the phase
    # by whole turns and leaves sin unchanged while keeping |phi| < 256.
    for hs in range(4):
        coef[:, :, (3 * hs + 0)::16] %= 1.0
        coef[:, :, (3 * hs + 1)::16] %= 1.0
    clog = np.zeros((B, NHRM, 32, 16))
    for m in range(32):
        for hs in range(4):
            base = 16 * m + 3 * hs
            clog[:, :, m, 4 * hs + 0] = coef[:, :, base + 0]
            clog[:, :, m, 4 * hs + 1] = coef[:, :, base + 1]
            clog[:, :, m, 4 * hs + 2] = coef[:, :, base + 2]
            clog[:, :, m, 4 * hs + 3] = coef[:, :, base + 2]
    splits = _split3(clog)
    # coefT7[64*(m%2) + 4*br + sp, (m//2)*640 + b*128 + glo]
    coefT7 = np.zeros((B, 128, 16 * 640), np.float16)
    for b4 in range(4):
        blkg = slice(128 * b4, 128 * (b4 + 1))
        for m in range(32):
            for sp in range(3):
                rows = 64 * (m % 2) + 4 * np.arange(16) + sp
                cols = (m // 2) * 640 + b4 * 128
                coefT7[:, rows, cols:cols + 128] = \
                    splits[sp][:, blkg, m, :].transpose(0, 2, 1)

    # ---- fund coefs in (sl,e) layout ----
    f0n_f = f0n * 0.5                                               # (B,16,64) turns
    fA = np.einsum("bes,sh->beh", f0n_f, st["Aq"])                  # (B,16,128)
    fB = np.einsum("bes,sh->beh", f0n_f, st["Bq"])
    fC = np.einsum("bes,sh->beh", f0n_f, st["Cq"])
    fA %= 1.0
    sA, sB, sC = _split3(fA), _split3(fB), _split3(fC)
    fcoef = np.zeros((B, 64, 512), np.float16)
    for t in range(8):
        ro = 32 * (t % 2)
        for sl in range(8):
            slot = 8 * t + sl
            p = 128 * (t // 2) + 16 * sl + np.arange(16)            # col index (per e)
            for hh in range(2):
                h = (1 + 2 * slot + hh) if slot < 63 else (0 if hh == 0 else 127)
                for sp in range(3):
                    fcoef[:, ro + 12 * hh + 0 + sp, p] = sA[sp][:, :, h]
                    fcoef[:, ro + 12 * hh + 3 + sp, p] = sB[sp][:, :, h]
                    fcoef[:, ro + 12 * hh + 6 + sp, p] = sC[sp][:, :, h]
                    fcoef[:, ro + 12 * hh + 9 + sp, p] = sC[sp][:, :, h]

    # ---- select weights (harmonics only) ----
    oe = np.clip(np.asarray(inputs["osc_env"], np.float64), 0, 1)   # (B,16,64)
    he = np.clip(np.asarray(inputs["harm_env"], np.float64), 0, 1)  # (B,16,32,64)
    env_node = (oe[:, :, None, :] * he).reshape(B, NHRM, SEQ)       # g'=32e+(o-1)
    selWh = np.zeros((B, 128, 4 * 2048), np.float32)
    eidx = np.arange(NHRM) // 32
    nodesL = np.concatenate([np.arange(63), [0]])
    nodesR = np.concatenate([np.arange(1, 64), [63]])
    for b4 in range(4):
        for glo in range(128):
            g = 128 * b4 + glo
            e = eidx[g]
            colsL = 2048 * b4 + 32 * np.arange(64) + e
            colsR = 2048 * b4 + 32 * np.arange(64) + 16 + e
            selWh[:, glo, colsL] = env_node[:, g, nodesL]
            selWh[:, glo, colsR] = env_node[:, g, nodesR]
    selWh = selWh.astype(bf)

    # ---- mix + fund env scalars ----
    ov = np.clip(np.asarray(inputs["overall_env"], np.float64), 0, 1)  # (B,16,64)
    envsc = np.zeros((B, 128, 32), np.float32)
    for t in range(8):
        for sl in range(8):
            slot = 8 * t + sl
            p = slice(16 * sl, 16 * (sl + 1))
            L, R = (slot, slot + 1) if slot < 63 else (0, 63)
            envsc[:, p, 4 * t + 0] = ov[:, :, L]
            envsc[:, p, 4 * t + 1] = ov[:, :, R] - ov[:, :, L]
            envsc[:, p, 4 * t + 2] = oe[:, :, L]
            envsc[:, p, 4 * t + 3] = oe[:, :, R] - oe[:, :, L]

    nf = np.asarray(inputs["noise_frames"], np.float32)             # (B,16,64,512)
    nfT = np.ascontiguousarray(
        nf.transpose(0, 3, 2, 1).reshape(B, WS, NFR)).astype(bf)    # [ws, s*16+e]

    nstd = np.clip(np.asarray(inputs["noise_std"], np.float64), 1e-12, 1.0) * F0_DIFF
    mean_fr = f0n.transpose(0, 2, 1).reshape(B, NFR)                # fr = s*16+e
    c2_fr = -0.5 / nstd.transpose(0, 2, 1).reshape(B, NFR) ** 2
    meanb = np.zeros((B, 128, 1056), np.float32)
    c2b = np.zeros((B, 128, 1056), np.float32)
    meanb[:, :, 16:1040] = mean_fr[:, None, :]
    c2b[:, :, 16:1040] = c2_fr[:, None, :]

    per_core = []
    for b in range(B):
        per_core.append(dict(
            coefT7=coefT7[b], selW=selWh[b], envsc=envsc[b], fcoef=fcoef[b],
            nfT=nfT[b], meanb=meanb[b].astype(bf), c2b=c2b[b].astype(bf),
            basis64=st["basis64"], fbasis=st["fbasis"],
            Cw=st["Cw"], Dc=st["Dc"], freq4=st["freq4"],
            Wc=st["Wc"], Wc63=st["Wc63"], W1m=st["W1m"], W1m63=st["W1m63"],
            P=st["P"], negI=st["negI"],
        ))
    return per_core


def _unshuffle(o):
    """[128, 2048] (sl,e)x(t,j) -> [16, 16384]."""
    full = np.zeros((NE, N), np.float32)
    for t in range(8):
        for sl in range(8):
            slot = 8 * t + sl
            r = o[16 * sl:16 * (sl + 1), 256 * t:256 * (t + 1)]
            if slot < 63:
                full[:, 128 + 256 * slot:128 + 256 * slot + 256] = r
            else:
                full[:, 0:128] = r[:, 0:128]
                full[:, 16256:16384] = r[:, 128:256]
    return full


def kernel(**inputs):
    from concourse.bass_utils import run_bass_kernel_spmd
    per_core = _host_prep(inputs)
    nc = _build_nc()
    trace = bool(os.environ.get("BASS_PROFILE"))
    res = run_bass_kernel_spmd(nc, per_core, list(range(B)), trace=trace)
    if trace and res.exec_time_ns is not None:
        print(f"HW exec time: {res.exec_time_ns} ns")
    out = np.stack([_unshuffle(np.asarray(r["out"], np.float32))
                    for r in res.results])
    return out


# revision 47
# speedup vs baseline: 1.8167x; 1.0040x over previous
"""Trainium2 Bass kernel v3 for nn_AudioEvent.

Per-core pipeline (batch-parallel over 8 cores):
  harmonics: host quadratic phase coeffs (A pre-reduced mod 1 so |phase|<256)
        -> stage2 matmuls (f16 3-split, exact) -> frac via dual-op round
        (kt=(phi+2^23)-2^23 in one DVE op, bf16) -> PE -I matmul accumulates
        -kt into psum -> Sin from psum (Act) -> env-folded select matmuls
        (512 harmonics = exactly 4 contraction blocks) -> ramp mult ->
        pairsum matmul -> osc psum in (slot,e) layout
  fundamentals: computed directly in (slot,e)-partition layout [128,256]
        per t-tile: tiny quad-coef matmul -> frac -> Sin -> env interp via
        dual-op tensor_scalar with per-partition node scalars
  noise: windowed rDFT matmuls -> gaussian filter -> irfft+overlap-add
        matmuls into (slot,e) layout
  mix:  dual-op piecewise-linear; out [128,2048] bf16, host unshuffles
"""
import os
import numpy as np
import ml_dtypes

B, NE, NH, SEQ, N, WS = 8, 16, 32, 64, 16384, 512
NYQ = 11025.0
MIN_F0 = np.float64(20.0 / NYQ)
MAX_F0 = np.float64(800.0 / NYQ)
F0_DIFF = MAX_F0 - MIN_F0
NHRM = 512                # 16 events x 32 harmonics
NFR = SEQ * NE            # 1024
C23 = float(1.5 * 2.0 ** 23)  # round magic: x+C in [2^23,2^24) ulp-1 zone

_cache = {}

# per-block frac variant: v2 = DVE kt + PE sub; v3 = Act yt + Pool kt2 + PE sub;
# v1 = DVE kt + DVE sub (sbuf fr, block sin)
_VAR = []
for _i in range(32):
    if _i in (2, 6, 10, 14, 18, 22, 26, 30) and not os.environ.get("ALL_V2"):
        _VAR.append("v3")
    else:
        _VAR.append("v2")
# pairs (block, q=1) computed DVE-sub + sbuf-sin instead of PE -I + psum-sin
_V1P = {4 * m + 1 for m in range(8)}


def _quad_halfseg(V):
    """Per 128-sample half-segment quadratic coefs of V (cumsum weights).
    V: [SEQ, N] -> (A, Bc, C) each [SEQ, 128]."""
    H = N // 128
    A = V[:, 0::128][:, :H]
    C = (V[:, 2::128][:, :H] - 2 * V[:, 1::128][:, :H] + A) / 2
    Bc = V[:, 1::128][:, :H] - A - C
    return A, Bc, C


def _build_static():
    if "static" in _cache:
        return _cache["static"]
    # ---- interp weight cumsum V and quadratic coeffs ----
    pos = (np.arange(N, dtype=np.float64) + 0.5) * (SEQ / N) - 0.5
    pos = np.clip(pos, 0.0, SEQ - 1)
    i0 = np.floor(pos).astype(np.int64)
    i1 = np.minimum(i0 + 1, SEQ - 1)
    w = pos - i0
    U = np.zeros((SEQ, N))
    U[i0, np.arange(N)] += 1.0 - w
    U[i1, np.arange(N)] += w
    V = np.cumsum(U, axis=1)
    # harmonic chunk coefs: W64[s, 16m+3hs+k] (k=A,B,C) for chunk (m,hs)
    W64 = np.zeros((SEQ, 512))
    for m in range(32):
        for hs in range(4):
            t0 = 512 * m + 128 * hs
            A = V[:, t0]
            C = (V[:, t0 + 2] - 2 * V[:, t0 + 1] + V[:, t0]) / 2
            Bc = V[:, t0 + 1] - V[:, t0] - C
            W64[:, 16 * m + 3 * hs + 0] = A
            W64[:, 16 * m + 3 * hs + 1] = Bc
            W64[:, 16 * m + 3 * hs + 2] = C
    # fund: per half-seg quad coefs
    Aq, Bq, Cq = _quad_halfseg(V)          # [SEQ, 128]

    # ---- stage2 basis (f16 split): rows 64*rep + 4*br + sp ----
    j = np.arange(128, dtype=np.float64)
    b16s = np.zeros((16, 512))
    for hs in range(4):
        sl = slice(128 * hs, 128 * (hs + 1))
        b16s[4 * hs + 0, sl] = 1.0
        b16s[4 * hs + 1, sl] = j
        j2h = np.float16(j * j).astype(np.float64)
        b16s[4 * hs + 2, sl] = j2h
        b16s[4 * hs + 3, sl] = j * j - j2h
    basis64 = np.zeros((128, 512), np.float16)
    for rep in range(2):
        for br in range(16):
            for sp in range(3):
                basis64[64 * rep + 4 * br + sp] = np.float16(b16s[br])

    # ---- fund basis: rows 12*hh + 3*bf + sp; block-diag pair [64, 512] ----
    fbasis1 = np.zeros((32, 256), np.float16)
    for hh in range(2):
        sl = slice(128 * hh, 128 * (hh + 1))
        j2h = np.float16(j * j).astype(np.float64)
        for sp in range(3):
            fbasis1[12 * hh + 0 + sp, sl] = 1.0
            fbasis1[12 * hh + 3 + sp, sl] = np.float16(j)
            fbasis1[12 * hh + 6 + sp, sl] = np.float16(j2h)
            fbasis1[12 * hh + 9 + sp, sl] = np.float16(j * j - j2h)
    fbasis = np.zeros((64, 512), np.float16)
    fbasis[0:32, 0:256] = fbasis1
    fbasis[32:64, 256:512] = fbasis1

    # ---- DFT consts (win folded), 4 contraction groups ----
    t = np.arange(WS)
    f = np.arange(WS // 2 + 1)
    win = 0.5 - 0.5 * np.cos(2 * np.pi * t / WS)
    ang = 2 * np.pi * np.outer(t, f) / WS
    CwRe = np.cos(ang) * win[:, None]
    CwIm = -np.sin(ang) * win[:, None]
    wgt = np.full(WS // 2 + 1, 2.0)
    wgt[0] = 1.0
    wgt[-1] = 1.0
    ang2 = 2 * np.pi * np.outer(f, t) / WS
    DRe = wgt[:, None] * np.cos(ang2) / WS
    DIm = -wgt[:, None] * np.sin(ang2) / WS
    Cw = np.zeros((128, 2048))       # col = tc*512 + grp*128 + fcol
    Dc = np.zeros((128, 2048))       # col = grp*512 + tau
    freq4 = np.zeros((128, 4))
    for grp in range(4):
        if grp == 0:
            fidx, mats = np.arange(0, 128), CwRe
        elif grp == 1:
            fidx, mats = np.arange(128, 256), CwRe
        elif grp == 2:
            fidx, mats = np.arange(1, 129), CwIm
        else:
            fidx, mats = np.concatenate([np.arange(129, 256), [256]]), None
        for tc in range(4):
            trows = slice(128 * tc, 128 * (tc + 1))
            if grp < 3:
                Cw[:, tc * 512 + grp * 128: tc * 512 + grp * 128 + 128] = mats[trows][:, fidx]
            else:
                blockm = CwIm[trows][:, fidx[:-1]]
                Cw[:, tc * 512 + grp * 128: tc * 512 + grp * 128 + 127] = blockm
                Cw[:, tc * 512 + grp * 128 + 127] = CwRe[trows][:, 256]
        if grp < 3:
            Dc[:, grp * 512:(grp + 1) * 512] = (DRe if grp < 2 else DIm)[fidx]
            freq4[:, grp] = fidx / 256.0
        else:
            Dc[:127, grp * 512:(grp + 1) * 512] = DIm[fidx[:-1]]
            Dc[127, grp * 512:(grp + 1) * 512] = DRe[256]
            freq4[:127, grp] = fidx[:-1] / 256.0
            freq4[127, grp] = 1.0

    # ---- ramp consts ----
    wj = (np.arange(256) + 0.5) / 256.0
    Wc = np.zeros((128, 256))
    for p in range(128):
        Wc[p] = wj if (p // 16) % 2 == 1 else 1.0 - wj
    Wc63 = Wc.copy()
    Wc63[96:112] = np.concatenate([np.ones(128), np.zeros(128)])
    Wc63[112:128] = np.concatenate([np.zeros(128), np.ones(128)])
    W1m = np.tile(wj, (128, 1))
    W1m63 = W1m.copy()
    W1m63[112:128] = np.concatenate([np.zeros(128), np.ones(128)])
    P = np.zeros((128, 64))
    for p in range(128):
        P[p, 16 * (p // 32) + p % 16] = 1.0
    negI = (-np.eye(128)).astype(np.float64)

    bf = ml_dtypes.bfloat16
    static = dict(
        W64=W64, basis64=basis64, Aq=Aq, Bq=Bq, Cq=Cq, fbasis=fbasis,
        Cw=Cw.astype(bf), Dc=Dc.astype(bf), freq4=freq4.astype(np.float32),
        Wc=Wc.astype(bf), Wc63=Wc63.astype(bf),
        W1m=W1m.astype(bf), W1m63=W1m63.astype(bf),
        P=P.astype(bf), negI=negI.astype(bf),
    )
    _cache["static"] = static
    return static


def _split3(x):
    """3-way fp16 split of float64 array: returns list of 3 fp16 arrays."""
    h0 = x.astype(np.float16).astype(np.float64)
    h1 = (x - h0).astype(np.float16).astype(np.float64)
    h2 = (x - h0 - h1).astype(np.float16)
    return [h0.astype(np.float16), h1.astype(np.float16), h2]


def _build_nc():
    if "nc" in _cache:
        return _cache["nc"]
    from concourse import bacc
    import concourse.tile as tile
    from concourse import mybir
    from contextlib import ExitStack

    F32 = mybir.dt.float32
    F16 = mybir.dt.float16
    BF16 = mybir.dt.bfloat16
    AF = mybir.ActivationFunctionType
    OP = mybir.AluOpType
    PI = float(np.pi)

    nc = bacc.Bacc()
    nc._dbg = {}
    # data params
    coefT7 = nc.declare_dram_parameter("coefT7", [128, 16 * 640], F16, isOutput=False)
    selW = nc.declare_dram_parameter("selW", [128, 4 * 2048], BF16, isOutput=False)
    envsc = nc.declare_dram_parameter("envsc", [128, 48], F32, isOutput=False)
    fcoef = nc.declare_dram_parameter("fcoef", [64, 512], F16, isOutput=False)
    nfT = nc.declare_dram_parameter("nfT", [512, 1024], BF16, isOutput=False)
    meanb = nc.declare_dram_parameter("meanb", [128, 1056], BF16, isOutput=False)
    c2b = nc.declare_dram_parameter("c2b", [128, 1056], BF16, isOutput=False)
    # const params
    basis64 = nc.declare_dram_parameter("basis64", [128, 512], F16, isOutput=False)
    fbasis = nc.declare_dram_parameter("fbasis", [64, 512], F16, isOutput=False)
    Cw = nc.declare_dram_parameter("Cw", [128, 2048], BF16, isOutput=False)
    Dc = nc.declare_dram_parameter("Dc", [128, 2048], BF16, isOutput=False)
    freq4 = nc.declare_dram_parameter("freq4", [128, 4], F32, isOutput=False)
    Wc = nc.declare_dram_parameter("Wc", [128, 256], BF16, isOutput=False)
    Wc63 = nc.declare_dram_parameter("Wc63", [128, 256], BF16, isOutput=False)
    W1m = nc.declare_dram_parameter("W1m", [128, 256], BF16, isOutput=False)
    W1m63 = nc.declare_dram_parameter("W1m63", [128, 256], BF16, isOutput=False)
    P = nc.declare_dram_parameter("P", [128, 64], BF16, isOutput=False)
    negI = nc.declare_dram_parameter("negI", [128, 128], BF16, isOutput=False)
    out = nc.declare_dram_parameter("out", [128, 2048], BF16, isOutput=True)

    with tile.TileContext(nc) as tc, ExitStack() as ctx:
        cp = ctx.enter_context(tc.tile_pool(name="const", bufs=1))

        # ---- const DMAs (SP queue), ordered by first need ----
        meanb_sb = cp.tile([128, 1056], BF16, tag="meanb")
        nc.gpsimd.dma_start(meanb_sb[:], meanb[:])
        c2b_sb = cp.tile([128, 1056], BF16, tag="c2b")
        nc.gpsimd.dma_start(c2b_sb[:], c2b[:])
        basis64_sb = cp.tile([128, 512], F16, tag="basis64")
        nc.sync.dma_start(basis64_sb[:], basis64[:])
        freq4_sb = cp.tile([128, 4], F32, tag="freq4")
        nc.sync.dma_start(freq4_sb[:], freq4[:])
        coefT7_sb = cp.tile([128, 16 * 640], F16, tag="coefT7")
        nc.sync.dma_start(coefT7_sb[:, 0:1280], coefT7[:, 0:1280])
        negI_sb = cp.tile([128, 128], BF16, tag="negI")
        nc.sync.dma_start(negI_sb[:], negI[:])
        nc.sync.dma_start(coefT7_sb[:, 1280:2560], coefT7[:, 1280:2560])
        fbasis_sb = cp.tile([64, 512], F16, tag="fbasis")
        nc.sync.dma_start(fbasis_sb[:], fbasis[:])
        fcoef_sb = cp.tile([64, 512], F16, tag="fcoef")
        nc.sync.dma_start(fcoef_sb[:], fcoef[:])
        W1m_sb = cp.tile([128, 256], BF16, tag="W1m")
        nc.sync.dma_start(W1m_sb[:], W1m[:])
        W1m63_sb = cp.tile([128, 256], BF16, tag="W1m63")
        nc.sync.dma_start(W1m63_sb[:], W1m63[:])
        P_sb = cp.tile([128, 64], BF16, tag="P")
        nc.sync.dma_start(P_sb[:], P[:])
        envsc_sb = cp.tile([128, 48], F32, tag="envsc")
        nc.sync.dma_start(envsc_sb[:], envsc[:])
        Wc_sb = cp.tile([128, 256], BF16, tag="Wc")
        nc.sync.dma_start(Wc_sb[:], Wc[:])
        Wc63_sb = cp.tile([128, 256], BF16, tag="Wc63")
        nc.sync.dma_start(Wc63_sb[:], Wc63[:])
        # selW in slot-quarters so early selects unblock sooner
        selW_sb = cp.tile([128, 4 * 2048], BF16, tag="selW")
        for b4 in range(4):           # slots 0..15
            nc.sync.dma_start(selW_sb[:, 2048 * b4:2048 * b4 + 512],
                              selW[:, 2048 * b4:2048 * b4 + 512])
        nfT_sb = [cp.tile([128, 1024], BF16, tag=f"nfT{i}", name=f"nfT{i}") for i in range(4)]
        for i in range(4):
            nc.sync.dma_start(nfT_sb[i][:], nfT[128 * i:128 * (i + 1), :])
        Cw_sb = cp.tile([128, 2048], BF16, tag="Cw")
        nc.sync.dma_start(Cw_sb[:], Cw[:])
        Dc_sb = cp.tile([128, 2048], BF16, tag="Dc")
        nc.sync.dma_start(Dc_sb[:], Dc[:])
        for cch in (2, 3):
            nc.sync.dma_start(coefT7_sb[:, 1280 * cch:1280 * (cch + 1)],
                              coefT7[:, 1280 * cch:1280 * (cch + 1)])
        for b4 in range(4):           # slots 16..39
            nc.sync.dma_start(selW_sb[:, 2048 * b4 + 512:2048 * b4 + 1280],
                              selW[:, 2048 * b4 + 512:2048 * b4 + 1280])
        for cch in (4, 5):
            nc.sync.dma_start(coefT7_sb[:, 1280 * cch:1280 * (cch + 1)],
                              coefT7[:, 1280 * cch:1280 * (cch + 1)])
        for b4 in range(4):           # slots 40..63
            nc.sync.dma_start(selW_sb[:, 2048 * b4 + 1280:2048 * b4 + 2048],
                              selW[:, 2048 * b4 + 1280:2048 * b4 + 2048])
        for cch in (6, 7):
            nc.sync.dma_start(coefT7_sb[:, 1280 * cch:1280 * (cch + 1)],
                              coefT7[:, 1280 * cch:1280 * (cch + 1)])
        c23b = cp.tile([128, 1], F32, tag="c23b")
        nc.vector.memset(c23b[:], C23)

        spec_sb = [cp.tile([128, 1056], BF16, tag=f"spec{g}", name=f"spec{g}") for g in range(4)]
        noise_sb = [cp.tile([128, 256], BF16, tag=f"nz{t}", name=f"nz{t}") for t in range(8)]
        st4f = [cp.tile([128, 512], BF16, tag=f"st4fp{u}", name=f"st4fp{u}") for u in range(4)]
        stHT = cp.tile([128, 512], BF16, tag="stHT")

        # ---- pools ----
        php = ctx.enter_context(tc.tile_pool(name="phi", bufs=2))      # v1 fr tiles
        stp = ctx.enter_context(tc.tile_pool(name="st", bufs=3))
        rtp = ctx.enter_context(tc.tile_pool(name="rt", bufs=4))       # kt bf16 pairs
        ytp = ctx.enter_context(tc.tile_pool(name="yt", bufs=2))       # v3 yt f32 pairs
        awp = ctx.enter_context(tc.tile_pool(name="aw", bufs=2))
        fin = ctx.enter_context(tc.tile_pool(name="fin", bufs=2))
        fpool = ctx.enter_context(tc.tile_pool(name="fp", bufs=1))
        # PSUM: ph ring 3x[128,1024] (6 banks) + A-pair (1) + osc-pair (1)
        psPH = ctx.enter_context(tc.tile_pool(name="psPH", bufs=3, space="PSUM"))
        psSel = ctx.enter_context(tc.tile_pool(name="psSel", bufs=1, space="PSUM"))
        psOsc = ctx.enter_context(tc.tile_pool(name="psOsc", bufs=1, space="PSUM"))

        st_tiles = {}

        # ============ noise filter pre-pass (Act: exps before any sin) ======
        for g in range(4):
            nc.gpsimd.memset(spec_sb[g][:, 0:16], 0.0)
            nc.gpsimd.memset(spec_sb[g][:, 1040:1056], 0.0)
        filt_t = [None] * 4
        dts = {}
        for g in (0, 1):
            d = fpool.tile([128, 1056], BF16, tag=f"fdt{g}", name=f"fd{g}")
            nc.gpsimd.tensor_scalar(d[:], meanb_sb[:], freq4_sb[:, g:g + 1], None,
                                    OP.subtract)
            dts[g] = d
        for g in (2, 3):
            d = fpool.tile([128, 1056], BF16, tag=f"fdt{g}", name=f"fd{g}")
            nc.vector.tensor_scalar(d[:], meanb_sb[:], freq4_sb[:, g:g + 1], None,
                                    OP.subtract)
            dts[g] = d
        # PE p-state warm-up: dummy accumulations while DMAs stream in
        warm = psPH.tile([128, 1024], F32, tag="ph", name="warm")
        for i in range(18):
            nc.tensor.matmul(warm[:, 0:512], basis64_sb[0:64, 0:128],
                             basis64_sb[0:64, :], start=(i == 0), stop=(i == 17),
                             skip_group_check=True, tile_position=(0, 0))
        dd = {}
        for g in (0, 1):
            d2 = fpool.tile([128, 1056], BF16, tag=f"fd2t{g}", name=f"fd2{g}")
            nc.gpsimd.tensor_tensor(d2[:], dts[g][:], dts[g][:], OP.mult)
            dd[g] = d2
        for g in (2, 3, 0, 1):
            if g in (2, 3):
                d2 = fpool.tile([128, 1056], BF16, tag=f"fd2t{g}", name=f"fd2{g}")
                nc.vector.tensor_tensor(d2[:], dts[g][:], dts[g][:], OP.mult)
            else:
                d2 = dd[g]
            m2 = fpool.tile([128, 1056], BF16, tag=f"fm2t{g}", name=f"fm2{g}")
            nc.vector.tensor_tensor(m2[:], d2[:], c2b_sb[:], OP.mult)
            filt = fpool.tile([128, 1056], BF16, tag=f"filt{g}", name=f"filt{g}")
            nc.scalar.activation(filt[:], m2[:], AF.Exp)
            filt_t[g] = filt

        # ============ span machinery (pair-granular, software-pipelined) ====
        pair_state = {}

        def emit_F(m, b, q):
            """stage2 matmuls + round (kt) for pair q of block (m, b)."""
            var = _VAR[4 * m + b]
            if var == "v2" and q == 1 and (4 * m + b) in _V1P:
                var = "v1p"
            if q == 0:
                st = stp.tile([128, 2048], BF16, tag=f"st{b}", name=f"st{b}_{m}")
                st_tiles[(b, m)] = st
            pp = psPH.tile([128, 1024], F32, tag="ph", name=f"pp{m}_{b}_{q}")
            for h in range(2):
                k = 2 * q + h
                mm = 4 * m + k
                p2 = 64 * (mm % 2)
                cb = (mm // 2) * 640 + b * 128
                nc.tensor.matmul(pp[:, 512 * h:512 * (h + 1)],
                                 coefT7_sb[p2:p2 + 64, cb:cb + 128],
                                 basis64_sb[p2:p2 + 64, :], start=True, stop=True,
                                 skip_group_check=True, tile_position=(p2, 0))
            kt = rtp.tile([128, 1024], BF16, tag="kt", name=f"kt{m}_{b}_{q}")
            if var == "v3":
                yt = ytp.tile([128, 1024], F32, tag="yt", name=f"yt{m}_{b}_{q}")
                nc.scalar.activation(yt[:], pp[:], AF.Identity, bias=c23b[:, 0:1])
                nc.gpsimd.tensor_scalar(kt[:], yt[:], C23, None, OP.subtract)
            else:
                nc.vector.tensor_scalar(kt[:], pp[:], C23, C23, OP.add, OP.subtract)
            pair_state[(b, q)] = (var, pp, kt)

        def emit_D(m, b, q):
            """-I accumulate + Sin (or DVE sub for v1p) for pair q."""
            var, pp, kt = pair_state.pop((b, q))
            st = st_tiles[(b, m)]
            if var == "v1p":
                fr = php.tile([128, 1024], F32, tag="fr", name=f"fr{b}_{m}_{q}")
                nc.vector.tensor_tensor(fr[:], pp[:], kt[:], OP.subtract)
                nc.scalar.activation(st[:, 1024 * q:1024 * (q + 1)], fr[:],
                                     AF.Sin, scale=2 * PI)
            else:
                for h in range(2):
                    nc.tensor.matmul(pp[:, 512 * h:512 * (h + 1)], negI_sb[:],
                                     kt[:, 512 * h:512 * (h + 1)],
                                     start=False, stop=True,
                                     skip_group_check=True, tile_position=(0, 0))
                nc.scalar.activation(st[:, 1024 * q:1024 * (q + 1)], pp[:],
                                     AF.Sin, scale=2 * PI)

        def span_sched(m):
            ops = []
            pend = []
            for b in range(4):
                for q in range(2):
                    ops.append(("F", b, q))
                    pend.append((b, q))
                    if len(pend) > 2:
                        ops.append(("D",) + pend.pop(0))
            while pend:
                ops.append(("D",) + pend.pop(0))
            return ops

        def emit_fund_pair(u):
            pf = psPH.tile([128, 1024], F32, tag="ph", name=f"fph{u}")
            nc.tensor.matmul(pf[:, 0:512], fcoef_sb[:, 128 * u:128 * (u + 1)],
                             fbasis_sb[:], start=True, stop=True,
                             skip_group_check=True, tile_position=(0, 0))
            kt = rtp.tile([128, 1024], BF16, tag="kt", name=f"fkt{u}")
            nc.vector.tensor_scalar(kt[:, 0:512], pf[:, 0:512], C23, C23,
                                    OP.add, OP.subtract)
            nc.tensor.matmul(pf[:, 0:512], negI_sb[:], kt[:, 0:512],
                             start=False, stop=True, skip_group_check=True,
                             tile_position=(0, 0))
            nc.scalar.activation(st4f[u][:], pf[:, 0:512], AF.Sin, scale=2 * PI)

        def sel_windows(slot):
            t0 = 128 + 256 * slot
            m = t0 // 2048
            lo = t0 - 2048 * m
            if lo + 256 <= 2048:
                return [(m, lo, lo + 256, 0)]
            return [(m, lo, 2048, 0), (m + 1, 0, lo + 256 - 2048, 2048 - lo)]

        A_pairs = {}

        def emit_slot(slot):
            at4 = slot // 4
            v = at4 // 2
            if v not in A_pairs:
                A_pairs[v] = psSel.tile([128, 512], F32, tag="A", name=f"Ap{v}")
            ao = 256 * (at4 % 2)
            A = A_pairs[v]
            sl4 = slot % 4
            first = [True]
            if slot < 63:
                cws = sel_windows(slot)
                nmm = 4 * len(cws)
                i = 0
                for b in range(4):
                    for (mw, lo, hi, plo) in cws:
                        i += 1
                        nc.tensor.matmul(
                            A[32 * sl4:32 * sl4 + 32, ao + plo:ao + plo + hi - lo],
                            selW_sb[:, 2048 * b + 32 * slot: 2048 * b + 32 * slot + 32],
                            st_tiles[(b, mw)][:, lo:hi],
                            start=first[0], stop=(i == nmm),
                            skip_group_check=True, tile_position=(0, 32 * sl4))
                        first[0] = False
            else:
                for b in range(4):
                    nc.tensor.matmul(
                        A[96:128, ao:ao + 128],
                        selW_sb[:, 2048 * b + 32 * 63: 2048 * b + 32 * 63 + 32],
                        stHT[:, 128 * b:128 * (b + 1)],
                        start=(b == 0), stop=False,
                        skip_group_check=True, tile_position=(0, 96))
                    nc.tensor.matmul(
                        A[96:128, ao + 128:ao + 256],
                        selW_sb[:, 2048 * b + 32 * 63: 2048 * b + 32 * 63 + 32],
                        st_tiles[(b, 7)][:, 1920:2048],
                        start=False, stop=(b == 3),
                        skip_group_check=True, tile_position=(0, 96))
            if slot % 4 == 3:
                emit_atile_done(at4)

        osc_pairs = {}

        def emit_atile_done(at4):
            v = at4 // 2
            A = A_pairs[v]
            ao = 256 * (at4 % 2)
            if at4 % 2 == 1:
                A_pairs.pop(v)
            aw = awp.tile([128, 256], BF16, tag="aw")
            wc = Wc63_sb if at4 == 15 else Wc_sb
            nc.vector.tensor_tensor(aw[:], A[:, ao:ao + 256], wc[:], OP.mult)
            t = at4 // 2
            u = t // 2
            if u not in osc_pairs:
                osc_pairs[u] = psOsc.tile([128, 512], F32, tag="osc", name=f"oscp{u}")
            oo = 256 * (t % 2)
            nc.tensor.matmul(
                osc_pairs[u][64 * (at4 % 2):64 * (at4 % 2) + 64, oo:oo + 256],
                P_sb[:], aw[:], start=True, stop=True,
                skip_group_check=True, tile_position=(0, 64 * (at4 % 2)))

        def emit_rfft(g, h):
            fr_sl = slice(512 * h, 512 * (h + 1))
            sp = psPH.tile([128, 1024], F32, tag="ph", name=f"rf{g}_{h}")
            for tcx in range(4):
                nc.tensor.matmul(sp[:, 0:512],
                                 Cw_sb[:, tcx * 512 + g * 128: tcx * 512 + g * 128 + 128],
                                 nfT_sb[tcx][:, fr_sl],
                                 start=(tcx == 0), stop=(tcx == 3))
            nc.vector.tensor_tensor(spec_sb[g][:, 16 + 512 * h:16 + 512 * (h + 1)],
                                    sp[:, 0:512], filt_t[g][:, 16 + 512 * h:16 + 512 * (h + 1)],
                                    OP.mult)

        def emit_irfft(t):
            pz = psPH.tile([128, 1024], F32, tag="ph", name=f"nzps{t}")
            base = 16 + 16 * (8 * t)
            nslots = 7 if t == 7 else 8
            ncols = 16 * nslots
            for g in range(4):
                gD = Dc_sb[:, 512 * g: 512 * (g + 1)]
                sW = spec_sb[g]
                nc.tensor.matmul(pz[0:ncols, 0:256], sW[:, base:base + ncols],
                                 gD[:, 128:384], start=(g == 0), stop=False,
                                 skip_group_check=True)
                nc.tensor.matmul(pz[0:ncols, 0:128], sW[:, base - 16:base - 16 + ncols],
                                 gD[:, 384:512], start=False, stop=False,
                                 skip_group_check=True)
                nc.tensor.matmul(pz[0:ncols, 128:256], sW[:, base + 16:base + 16 + ncols],
                                 gD[:, 0:128], start=False,
                                 stop=(t < 7 and g == 3),
                                 skip_group_check=True)
            if t == 7:
                b63 = 16 + 16 * 63
                pz63 = psPH.tile([32, 512], F32, tag="ph", name="nz63")
                for g in range(4):
                    gD = Dc_sb[:, 512 * g: 512 * (g + 1)]
                    sW = spec_sb[g]
                    nc.tensor.matmul(pz63[0:16, 0:128], sW[:, 16:32],
                                     gD[:, 0:128], start=(g == 0), stop=False,
                                     skip_group_check=True, tile_position=(0, 0))
                    nc.tensor.matmul(pz63[0:16, 128:256], sW[:, b63:b63 + 16],
                                     gD[:, 128:256], start=False, stop=False,
                                     skip_group_check=True, tile_position=(0, 0))
                    nc.tensor.matmul(pz63[0:16, 128:256], sW[:, b63 - 16:b63],
                                     gD[:, 384:512], start=False, stop=(g == 3),
                                     skip_group_check=True, tile_position=(0, 0))
                nc.vector.tensor_copy(noise_sb[t][0:112, :], pz[0:112, 0:256])
                nz63s = fpool.tile([16, 256], BF16, tag="nz63s")
                nc.vector.tensor_copy(nz63s[:], pz63[0:16, 0:256])
                nc.sync.dma_start(noise_sb[t][112:128, :], nz63s[:])
            else:
                nc.vector.tensor_copy(noise_sb[t][:], pz[:, 0:256])

        def emit_combine(t):
            # r = osc*mixT + [fc*mixT + noise*(1-mixT)]; bracket has no osc dep
            u = t // 2
            osc = osc_pairs[u][:, 256 * (t % 2):256 * (t % 2) + 256]
            if t % 2 == 1:
                osc_pairs.pop(u)
            w1 = W1m63_sb if t == 7 else W1m_sb
            mixT = fin.tile([128, 256], BF16, tag="mixT")
            nc.gpsimd.tensor_scalar(mixT[:], w1[:], envsc_sb[:, 6 * t + 1:6 * t + 2],
                                    envsc_sb[:, 6 * t + 0:6 * t + 1], OP.mult, OP.add)
            mixT1 = fin.tile([128, 256], BF16, tag="mixT1")
            nc.gpsimd.tensor_scalar(mixT1[:], w1[:], envsc_sb[:, 6 * t + 5:6 * t + 6],
                                    envsc_sb[:, 6 * t + 4:6 * t + 5], OP.mult, OP.add)
            mixF = fin.tile([128, 256], BF16, tag="mixF")
            nc.gpsimd.tensor_scalar(mixF[:], w1[:], envsc_sb[:, 6 * t + 3:6 * t + 4],
                                    envsc_sb[:, 6 * t + 2:6 * t + 3], OP.mult, OP.add)
            fc = fin.tile([128, 256], BF16, tag="fc")
            nc.gpsimd.tensor_tensor(fc[:], st4f[t // 2][:, 256 * (t % 2):256 * (t % 2) + 256], mixF[:], OP.mult)
            fcm = fin.tile([128, 256], BF16, tag="fcm")
            nc.gpsimd.tensor_tensor(fcm[:], fc[:], mixT[:], OP.mult)
            nzm = fin.tile([128, 256], BF16, tag="nzm")
            nc.gpsimd.tensor_tensor(nzm[:], noise_sb[t][:], mixT1[:], OP.mult)
            pre = fin.tile([128, 256], BF16, tag="pre")
            nc.gpsimd.tensor_tensor(pre[:], fcm[:], nzm[:], OP.add)
            om = fin.tile([128, 256], BF16, tag="om")
            nc.vector.tensor_tensor(om[:], osc, mixT[:], OP.mult)
            r = fin.tile([128, 256], BF16, tag="r")
            nc._dbg[f"r{t}"] = r
            nc.gpsimd.tensor_tensor(r[:], om[:], pre[:], OP.add)
            nc.sync.dma_start(out[:, 256 * t:256 * (t + 1)], r[:])

        # ============ interleaved drive (global pipelined stream) ============
        def mkslot(sl):
            return lambda: emit_slot(sl)

        def stht_copy():
            for b in range(4):
                nc.gpsimd.tensor_copy(stHT[:, 128 * b:128 * (b + 1)],
                                      st_tiles[(b, 0)][:, 0:128])

        fillers = {m: [] for m in range(8)}
        fillers[0] = [stht_copy]
        fillers[1] = [lambda: emit_rfft(0, 0), lambda: emit_rfft(0, 1),
                      lambda: emit_rfft(1, 0), lambda: emit_rfft(1, 1),
                      mkslot(0), lambda: emit_rfft(2, 0), mkslot(1),
                      lambda: emit_rfft(2, 1), mkslot(2), lambda: emit_rfft(3, 0),
                      mkslot(3), lambda: emit_rfft(3, 1), mkslot(4),
                      lambda: emit_fund_pair(0), mkslot(5),
                      mkslot(6), mkslot(7), lambda: emit_irfft(0)]
        for m in range(2, 8):
            base_s = 8 * (m - 1)
            fl = [(lambda t: lambda: emit_combine(t))(m - 2)]
            if m in (3, 5, 7):
                fl.append((lambda u: lambda: emit_fund_pair(u))(m // 2))
            fl += [mkslot(base_s + i) for i in range(6)]
            fl.append((lambda t: lambda: emit_irfft(t))(m - 1))
            fl += [mkslot(base_s + 6), mkslot(base_s + 7)]
            fillers[m] = fl

        allp = [(m, b, q) for m in range(8) for b in range(4) for q in range(2)]
        ops = []
        pend = []
        for pr in allp:
            ops.append(("F",) + pr)
            pend.append(pr)
            if len(pend) > 3:
                ops.append(("D",) + pend.pop(0))
        while pend:
            ops.append(("D",) + pend.pop(0))

        from collections import Counter
        cnt = Counter(op[1] for op in ops)
        seen = Counter()
        fidx = {m: 0 for m in fillers}
        fm = [0]

        def pump():
            # strictly ordered filler groups; pace group fm by span-fm progress
            while fm[0] < 8:
                fl = fillers[fm[0]]
                if fidx[fm[0]] >= len(fl):
                    if seen[fm[0]] >= cnt[fm[0]]:
                        fm[0] += 1
                        continue
                    break
                frac = seen[fm[0]] / cnt[fm[0]]
                if (fidx[fm[0]] + 1) / len(fl) <= frac or seen[fm[0]] >= cnt[fm[0]]:
                    fl[fidx[fm[0]]]()
                    fidx[fm[0]] += 1
                else:
                    break

        for op in ops:
            if op[0] == "F":
                emit_F(op[1], op[2], op[3])
            else:
                emit_D(op[1], op[2], op[3])
            seen[op[1]] += 1
            pump()
        while fm[0] < 8:
            fl = fillers[fm[0]]
            if fidx[fm[0]] < len(fl):
                fl[fidx[fm[0]]]()
                fidx[fm[0]] += 1
            else:
                fm[0] += 1
        # epilogue: sel(7) + final combines (slots 56+ need all span-7 sins
        # emitted first; engine parallelism still overlaps them with the
        # last Act sins)
        emit_slot(56)
        emit_slot(57)
        emit_irfft(7)
        emit_slot(58)
        emit_slot(59)
        emit_combine(6)
        for sl in range(60, 64):
            emit_slot(sl)
        emit_combine(7)
    nc.finalize()
    _cache["nc"] = nc
    return nc


def _host_prep(inputs):
    st = _build_static()
    bf = ml_dtypes.bfloat16
    f0 = np.clip(np.asarray(inputs["f0"], np.float64), -0.5, 0.5)
    f0b = np.asarray(inputs["f0_baselines"], np.float64)
    erb = (0.108 * (f0b * NYQ) + 24.7) / NYQ
    f0v = np.clip(f0b + f0 * erb, 0.0, 1.0)
    f0n = MIN_F0 + f0v * F0_DIFF                                    # (B,16,64)

    # ---- harmonic rows: g' = 32*e + (o-1), o in 1..32, hfact = o+1 ----
    hfact = np.arange(2, 2 + NH, dtype=np.float64)                  # [2..33]
    frq = (f0n[:, :, None, :] * hfact[None, None, :, None] * 0.5)   # (B,16,32,64)
    frq = frq.reshape(B, NHRM, SEQ)

    coef = np.einsum("bgs,sc->bgc", frq, st["W64"])                 # (B,512,512)
    # reduce A (offset) and B (slope) coefs mod 1: basis funcs (1, j, j2hi,
    # j2lo) take integer values, so shifting A/B by integers moves the phase
    # by whole turns and leaves sin unchanged while keeping |phi| < 256.
    for hs in range(4):
        coef[:, :, (3 * hs + 0)::16] %= 1.0
        coef[:, :, (3 * hs + 1)::16] %= 1.0
    clog = np.zeros((B, NHRM, 32, 16))
    for m in range(32):
        for hs in range(4):
            base = 16 * m + 3 * hs
            clog[:, :, m, 4 * hs + 0] = coef[:, :, base + 0]
            clog[:, :, m, 4 * hs + 1] = coef[:, :, base + 1]
            clog[:, :, m, 4 * hs + 2] = coef[:, :, base + 2]
            clog[:, :, m, 4 * hs + 3] = coef[:, :, base + 2]
    splits = _split3(clog)
    # coefT7[64*(m%2) + 4*br + sp, (m//2)*640 + b*128 + glo]
    coefT7 = np.zeros((B, 128, 16 * 640), np.float16)
    for b4 in range(4):
        blkg = slice(128 * b4, 128 * (b4 + 1))
        for m in range(32):
            for sp in range(3):
                rows = 64 * (m % 2) + 4 * np.arange(16) + sp
                cols = (m // 2) * 640 + b4 * 128
                coefT7[:, rows, cols:cols + 128] = \
                    splits[sp][:, blkg, m, :].transpose(0, 2, 1)

    # ---- fund coefs in (sl,e) layout ----
    f0n_f = f0n * 0.5                                               # (B,16,64) turns
    fA = np.einsum("bes,sh->beh", f0n_f, st["Aq"])                  # (B,16,128)
    fB = np.einsum("bes,sh->beh", f0n_f, st["Bq"])
    fC = np.einsum("bes,sh->beh", f0n_f, st["Cq"])
    fA %= 1.0
    sA, sB, sC = _split3(fA), _split3(fB), _split3(fC)
    fcoef = np.zeros((B, 64, 512), np.float16)
    for t in range(8):
        ro = 32 * (t % 2)
        for sl in range(8):
            slot = 8 * t + sl
            p = 128 * (t // 2) + 16 * sl + np.arange(16)            # col index (per e)
            for hh in range(2):
                h = (1 + 2 * slot + hh) if slot < 63 else (0 if hh == 0 else 127)
                for sp in range(3):
                    fcoef[:, ro + 12 * hh + 0 + sp, p] = sA[sp][:, :, h]
                    fcoef[:, ro + 12 * hh + 3 + sp, p] = sB[sp][:, :, h]
                    fcoef[:, ro + 12 * hh + 6 + sp, p] = sC[sp][:, :, h]
                    fcoef[:, ro + 12 * hh + 9 + sp, p] = sC[sp][:, :, h]

    # ---- select weights (harmonics only) ----
    oe = np.clip(np.asarray(inputs["osc_env"], np.float64), 0, 1)   # (B,16,64)
    he = np.clip(np.asarray(inputs["harm_env"], np.float64), 0, 1)  # (B,16,32,64)
    env_node = (oe[:, :, None, :] * he).reshape(B, NHRM, SEQ)       # g'=32e+(o-1)
    selWh = np.zeros((B, 128, 4 * 2048), np.float32)
    eidx = np.arange(NHRM) // 32
    nodesL = np.concatenate([np.arange(63), [0]])
    nodesR = np.concatenate([np.arange(1, 64), [63]])
    for b4 in range(4):
        for glo in range(128):
            g = 128 * b4 + glo
            e = eidx[g]
            colsL = 2048 * b4 + 32 * np.arange(64) + e
            colsR = 2048 * b4 + 32 * np.arange(64) + 16 + e
            selWh[:, glo, colsL] = env_node[:, g, nodesL]
            selWh[:, glo, colsR] = env_node[:, g, nodesR]
    selWh = selWh.astype(bf)

    # ---- mix + fund env scalars ----
    ov = np.clip(np.asarray(inputs["overall_env"], np.float64), 0, 1)  # (B,16,64)
    envsc = np.zeros((B, 128, 48), np.float32)
    for t in range(8):
        for sl in range(8):
            slot = 8 * t + sl
            p = slice(16 * sl, 16 * (sl + 1))
            L, R = (slot, slot + 1) if slot < 63 else (0, 63)
            envsc[:, p, 6 * t + 0] = ov[:, :, L]
            envsc[:, p, 6 * t + 1] = ov[:, :, R] - ov[:, :, L]
            envsc[:, p, 6 * t + 2] = oe[:, :, L]
            envsc[:, p, 6 * t + 3] = oe[:, :, R] - oe[:, :, L]
            envsc[:, p, 6 * t + 4] = 1.0 - ov[:, :, L]
            envsc[:, p, 6 * t + 5] = ov[:, :, L] - ov[:, :, R]

    nf = np.asarray(inputs["noise_frames"], np.float32)             # (B,16,64,512)
    nfT = np.ascontiguousarray(
        nf.transpose(0, 3, 2, 1).reshape(B, WS, NFR)).astype(bf)    # [ws, s*16+e]

    nstd = np.clip(np.asarray(inputs["noise_std"], np.float64), 1e-12, 1.0) * F0_DIFF
    mean_fr = f0n.transpose(0, 2, 1).reshape(B, NFR)                # fr = s*16+e
    c2_fr = -0.5 / nstd.transpose(0, 2, 1).reshape(B, NFR) ** 2
    meanb = np.zeros((B, 128, 1056), np.float32)
    c2b = np.zeros((B, 128, 1056), np.float32)
    meanb[:, :, 16:1040] = mean_fr[:, None, :]
    c2b[:, :, 16:1040] = c2_fr[:, None, :]

    per_core = []
    for b in range(B):
        per_core.append(dict(
            coefT7=coefT7[b], selW=selWh[b], envsc=envsc[b], fcoef=fcoef[b],
            nfT=nfT[b], meanb=meanb[b].astype(bf), c2b=c2b[b].astype(bf),
            basis64=st["basis64"], fbasis=st["fbasis"],
            Cw=st["Cw"], Dc=st["Dc"], freq4=st["freq4"],
            Wc=st["Wc"], Wc63=st["Wc63"], W1m=st["W1m"], W1m63=st["W1m63"],
            P=st["P"], negI=st["negI"],
        ))
    return per_core


def _unshuffle(o):
    """[128, 2048] (sl,e)x(t,j) -> [16, 16384]."""
    full = np.zeros((NE, N), np.float32)
    for t in range(8):
        for sl in range(8):
            slot = 8 * t + sl
            r = o[16 * sl:16 * (sl + 1), 256 * t:256 * (t + 1)]
            if slot < 63:
                full[:, 128 + 256 * slot:128 + 256 * slot + 256] = r
            else:
                full[:, 0:128] = r[:, 0:128]
                full[:, 16256:16384] = r[:, 128:256]
    return full


def kernel(**inputs):
    from concourse.bass_utils import run_bass_kernel_spmd
    per_core = _host_prep(inputs)
    nc = _build_nc()
    trace = bool(os.environ.get("BASS_PROFILE"))
    res = run_bass_kernel_spmd(nc, per_core, list(range(B)), trace=trace)
    if trace and res.exec_time_ns is not None:
        print(f"HW exec time: {res.exec_time_ns} ns")
    out = np.stack([_unshuffle(np.asarray(r["out"], np.float32))
                    for r in res.results])
    return out


# revision 50
# speedup vs baseline: 1.8428x; 1.0144x over previous
"""Trainium2 Bass kernel v3 for nn_AudioEvent.

Per-core pipeline (batch-parallel over 8 cores):
  harmonics: host quadratic phase coeffs (A pre-reduced mod 1 so |phase|<256)
        -> stage2 matmuls (f16 3-split, exact) -> frac via dual-op round
        (kt=(phi+2^23)-2^23 in one DVE op, bf16) -> PE -I matmul accumulates
        -kt into psum -> Sin from psum (Act) -> env-folded select matmuls
        (512 harmonics = exactly 4 contraction blocks) -> ramp mult ->
        pairsum matmul -> osc psum in (slot,e) layout
  fundamentals: computed directly in (slot,e)-partition layout [128,256]
        per t-tile: tiny quad-coef matmul -> frac -> Sin -> env interp via
        dual-op tensor_scalar with per-partition node scalars
  noise: windowed rDFT matmuls -> gaussian filter -> irfft+overlap-add
        matmuls into (slot,e) layout
  mix:  dual-op piecewise-linear; out [128,2048] bf16, host unshuffles
"""
import os
import numpy as np
import ml_dtypes

B, NE, NH, SEQ, N, WS = 8, 16, 32, 64, 16384, 512
NYQ = 11025.0
MIN_F0 = np.float64(20.0 / NYQ)
MAX_F0 = np.float64(800.0 / NYQ)
F0_DIFF = MAX_F0 - MIN_F0
NHRM = 512                # 16 events x 32 harmonics
NFR = SEQ * NE            # 1024
C23 = float(1.5 * 2.0 ** 23)  # round magic: x+C in [2^23,2^24) ulp-1 zone

_cache = {}

# per-block frac variant: v2 = DVE kt + PE sub; v3 = Act yt + Pool kt2 + PE sub;
# v1 = DVE kt + DVE sub (sbuf fr, block sin)
_VAR = []
for _i in range(32):
    if _i in (2, 6, 10, 14, 18, 22, 26, 30) and not os.environ.get("ALL_V2"):
        _VAR.append("v3")
    else:
        _VAR.append("v2")
# pairs (block, q=1) computed DVE-sub + sbuf-sin instead of PE -I + psum-sin
_V1P = {4 * m + 1 for m in range(8)}


def _quad_halfseg(V):
    """Per 128-sample half-segment quadratic coefs of V (cumsum weights).
    V: [SEQ, N] -> (A, Bc, C) each [SEQ, 128]."""
    H = N // 128
    A = V[:, 0::128][:, :H]
    C = (V[:, 2::128][:, :H] - 2 * V[:, 1::128][:, :H] + A) / 2
    Bc = V[:, 1::128][:, :H] - A - C
    return A, Bc, C


def _build_static():
    if "static" in _cache:
        return _cache["static"]
    # ---- interp weight cumsum V and quadratic coeffs ----
    pos = (np.arange(N, dtype=np.float64) + 0.5) * (SEQ / N) - 0.5
    pos = np.clip(pos, 0.0, SEQ - 1)
    i0 = np.floor(pos).astype(np.int64)
    i1 = np.minimum(i0 + 1, SEQ - 1)
    w = pos - i0
    U = np.zeros((SEQ, N))
    U[i0, np.arange(N)] += 1.0 - w
    U[i1, np.arange(N)] += w
    V = np.cumsum(U, axis=1)
    # harmonic chunk coefs: W64[s, 16m+3hs+k] (k=A,B,C) for chunk (m,hs)
    W64 = np.zeros((SEQ, 512))
    for m in range(32):
        for hs in range(4):
            t0 = 512 * m + 128 * hs
            A = V[:, t0]
            C = (V[:, t0 + 2] - 2 * V[:, t0 + 1] + V[:, t0]) / 2
            Bc = V[:, t0 + 1] - V[:, t0] - C
            W64[:, 16 * m + 3 * hs + 0] = A
            W64[:, 16 * m + 3 * hs + 1] = Bc
            W64[:, 16 * m + 3 * hs + 2] = C
    # fund: per half-seg quad coefs
    Aq, Bq, Cq = _quad_halfseg(V)          # [SEQ, 128]

    # ---- stage2 basis (f16 split): rows 64*rep + 4*br + sp ----
    j = np.arange(128, dtype=np.float64)
    b16s = np.zeros((16, 512))
    for hs in range(4):
        sl = slice(128 * hs, 128 * (hs + 1))
        b16s[4 * hs + 0, sl] = 1.0
        b16s[4 * hs + 1, sl] = j
        j2h = np.float16(j * j).astype(np.float64)
        b16s[4 * hs + 2, sl] = j2h
        b16s[4 * hs + 3, sl] = j * j - j2h
    basis64 = np.zeros((128, 512), np.float16)
    for rep in range(2):
        for br in range(16):
            for sp in range(3):
                basis64[64 * rep + 4 * br + sp] = np.float16(b16s[br])

    # ---- fund basis: rows 12*hh + 3*bf + sp; block-diag pair [64, 512] ----
    fbasis1 = np.zeros((32, 256), np.float16)
    for hh in range(2):
        sl = slice(128 * hh, 128 * (hh + 1))
        j2h = np.float16(j * j).astype(np.float64)
        for sp in range(3):
            fbasis1[12 * hh + 0 + sp, sl] = 1.0
            fbasis1[12 * hh + 3 + sp, sl] = np.float16(j)
            fbasis1[12 * hh + 6 + sp, sl] = np.float16(j2h)
            fbasis1[12 * hh + 9 + sp, sl] = np.float16(j * j - j2h)
    fbasis = np.zeros((64, 512), np.float16)
    fbasis[0:32, 0:256] = fbasis1
    fbasis[32:64, 256:512] = fbasis1

    # ---- DFT consts (win folded), 4 contraction groups ----
    t = np.arange(WS)
    f = np.arange(WS // 2 + 1)
    win = 0.5 - 0.5 * np.cos(2 * np.pi * t / WS)
    ang = 2 * np.pi * np.outer(t, f) / WS
    CwRe = np.cos(ang) * win[:, None]
    CwIm = -np.sin(ang) * win[:, None]
    wgt = np.full(WS // 2 + 1, 2.0)
    wgt[0] = 1.0
    wgt[-1] = 1.0
    ang2 = 2 * np.pi * np.outer(f, t) / WS
    DRe = wgt[:, None] * np.cos(ang2) / WS
    DIm = -wgt[:, None] * np.sin(ang2) / WS
    Cw = np.zeros((128, 2048))       # col = tc*512 + grp*128 + fcol
    Dc = np.zeros((128, 2048))       # col = grp*512 + tau
    freq4 = np.zeros((128, 4))
    for grp in range(4):
        if grp == 0:
            fidx, mats = np.arange(0, 128), CwRe
        elif grp == 1:
            fidx, mats = np.arange(128, 256), CwRe
        elif grp == 2:
            fidx, mats = np.arange(1, 129), CwIm
        else:
            fidx, mats = np.concatenate([np.arange(129, 256), [256]]), None
        for tc in range(4):
            trows = slice(128 * tc, 128 * (tc + 1))
            if grp < 3:
                Cw[:, tc * 512 + grp * 128: tc * 512 + grp * 128 + 128] = mats[trows][:, fidx]
            else:
                blockm = CwIm[trows][:, fidx[:-1]]
                Cw[:, tc * 512 + grp * 128: tc * 512 + grp * 128 + 127] = blockm
                Cw[:, tc * 512 + grp * 128 + 127] = CwRe[trows][:, 256]
        if grp < 3:
            Dc[:, grp * 512:(grp + 1) * 512] = (DRe if grp < 2 else DIm)[fidx]
            freq4[:, grp] = fidx / 256.0
        else:
            Dc[:127, grp * 512:(grp + 1) * 512] = DIm[fidx[:-1]]
            Dc[127, grp * 512:(grp + 1) * 512] = DRe[256]
            freq4[:127, grp] = fidx[:-1] / 256.0
            freq4[127, grp] = 1.0

    # ---- ramp consts ----
    wj = (np.arange(256) + 0.5) / 256.0
    Wc = np.zeros((128, 256))
    for p in range(128):
        Wc[p] = wj if (p // 16) % 2 == 1 else 1.0 - wj
    Wc63 = Wc.copy()
    Wc63[96:112] = np.concatenate([np.ones(128), np.zeros(128)])
    Wc63[112:128] = np.concatenate([np.zeros(128), np.ones(128)])
    W1m = np.tile(wj, (128, 1))
    W1m63 = W1m.copy()
    W1m63[112:128] = np.concatenate([np.zeros(128), np.ones(128)])
    P = np.zeros((128, 64))
    for p in range(128):
        P[p, 16 * (p // 32) + p % 16] = 1.0
    negI = (-np.eye(128)).astype(np.float64)

    bf = ml_dtypes.bfloat16
    static = dict(
        W64=W64, basis64=basis64, Aq=Aq, Bq=Bq, Cq=Cq, fbasis=fbasis,
        Cw=Cw.astype(bf), Dc=Dc.astype(bf), freq4=freq4.astype(np.float32),
        Wc=Wc.astype(bf), Wc63=Wc63.astype(bf),
        W1m=W1m.astype(bf), W1m63=W1m63.astype(bf),
        P=P.astype(bf), negI=negI.astype(bf),
    )
    _cache["static"] = static
    return static


def _split3(x):
    """3-way fp16 split of float64 array: returns list of 3 fp16 arrays."""
    h0 = x.astype(np.float16).astype(np.float64)
    h1 = (x - h0).astype(np.float16).astype(np.float64)
    h2 = (x - h0 - h1).astype(np.float16)
    return [h0.astype(np.float16), h1.astype(np.float16), h2]


def _build_nc():
    if "nc" in _cache:
        return _cache["nc"]
    from concourse import bacc
    import concourse.tile as tile
    from concourse import mybir
    from contextlib import ExitStack

    F32 = mybir.dt.float32
    F16 = mybir.dt.float16
    BF16 = mybir.dt.bfloat16
    AF = mybir.ActivationFunctionType
    OP = mybir.AluOpType
    PI = float(np.pi)

    nc = bacc.Bacc()
    nc._dbg = {}
    # data params
    coefT7 = nc.declare_dram_parameter("coefT7", [128, 16 * 640], F16, isOutput=False)
    selW = nc.declare_dram_parameter("selW", [128, 4 * 2048], BF16, isOutput=False)
    envsc = nc.declare_dram_parameter("envsc", [128, 48], F32, isOutput=False)
    fcoef = nc.declare_dram_parameter("fcoef", [64, 512], F16, isOutput=False)
    nfT = nc.declare_dram_parameter("nfT", [512, 1024], BF16, isOutput=False)
    meanb = nc.declare_dram_parameter("meanb", [128, 1056], BF16, isOutput=False)
    c2b = nc.declare_dram_parameter("c2b", [128, 1056], BF16, isOutput=False)
    # const params
    basis64 = nc.declare_dram_parameter("basis64", [128, 512], F16, isOutput=False)
    fbasis = nc.declare_dram_parameter("fbasis", [64, 512], F16, isOutput=False)
    Cw = nc.declare_dram_parameter("Cw", [128, 2048], BF16, isOutput=False)
    Dc = nc.declare_dram_parameter("Dc", [128, 2048], BF16, isOutput=False)
    freq4 = nc.declare_dram_parameter("freq4", [128, 4], F32, isOutput=False)
    Wc = nc.declare_dram_parameter("Wc", [128, 256], BF16, isOutput=False)
    Wc63 = nc.declare_dram_parameter("Wc63", [128, 256], BF16, isOutput=False)
    W1m = nc.declare_dram_parameter("W1m", [128, 256], BF16, isOutput=False)
    W1m63 = nc.declare_dram_parameter("W1m63", [128, 256], BF16, isOutput=False)
    P = nc.declare_dram_parameter("P", [128, 64], BF16, isOutput=False)
    negI = nc.declare_dram_parameter("negI", [128, 128], BF16, isOutput=False)
    out = nc.declare_dram_parameter("out", [128, 2048], BF16, isOutput=True)

    with tile.TileContext(nc) as tc, ExitStack() as ctx:
        cp = ctx.enter_context(tc.tile_pool(name="const", bufs=1))

        # ---- const DMAs (SP queue), ordered by first need ----
        meanb_sb = cp.tile([128, 1056], BF16, tag="meanb")
        nc.gpsimd.dma_start(meanb_sb[:], meanb[:])
        c2b_sb = cp.tile([128, 1056], BF16, tag="c2b")
        nc.gpsimd.dma_start(c2b_sb[:], c2b[:])
        basis64_sb = cp.tile([128, 512], F16, tag="basis64")
        nc.sync.dma_start(basis64_sb[:], basis64[:])
        freq4_sb = cp.tile([128, 4], F32, tag="freq4")
        nc.sync.dma_start(freq4_sb[:], freq4[:])
        coefT7_sb = cp.tile([128, 16 * 640], F16, tag="coefT7")
        nc.sync.dma_start(coefT7_sb[:, 0:1280], coefT7[:, 0:1280])
        negI_sb = cp.tile([128, 128], BF16, tag="negI")
        nc.sync.dma_start(negI_sb[:], negI[:])
        nc.sync.dma_start(coefT7_sb[:, 1280:2560], coefT7[:, 1280:2560])
        fbasis_sb = cp.tile([64, 512], F16, tag="fbasis")
        nc.sync.dma_start(fbasis_sb[:], fbasis[:])
        fcoef_sb = cp.tile([64, 512], F16, tag="fcoef")
        nc.sync.dma_start(fcoef_sb[:], fcoef[:])
        W1m_sb = cp.tile([128, 256], BF16, tag="W1m")
        nc.sync.dma_start(W1m_sb[:], W1m[:])
        W1m63_sb = cp.tile([128, 256], BF16, tag="W1m63")
        nc.sync.dma_start(W1m63_sb[:], W1m63[:])
        P_sb = cp.tile([128, 64], BF16, tag="P")
        nc.sync.dma_start(P_sb[:], P[:])
        envsc_sb = cp.tile([128, 48], F32, tag="envsc")
        nc.sync.dma_start(envsc_sb[:], envsc[:])
        Wc_sb = cp.tile([128, 256], BF16, tag="Wc")
        nc.sync.dma_start(Wc_sb[:], Wc[:])
        Wc63_sb = cp.tile([128, 256], BF16, tag="Wc63")
        nc.sync.dma_start(Wc63_sb[:], Wc63[:])
        # selW in slot-quarters so early selects unblock sooner
        selW_sb = cp.tile([128, 4 * 2048], BF16, tag="selW")
        for b4 in range(4):           # slots 0..15
            nc.sync.dma_start(selW_sb[:, 2048 * b4:2048 * b4 + 512],
                              selW[:, 2048 * b4:2048 * b4 + 512])
        nfT_sb = [cp.tile([128, 1024], BF16, tag=f"nfT{i}", name=f"nfT{i}") for i in range(4)]
        for i in range(4):
            nc.sync.dma_start(nfT_sb[i][:], nfT[128 * i:128 * (i + 1), :])
        Cw_sb = cp.tile([128, 2048], BF16, tag="Cw")
        nc.sync.dma_start(Cw_sb[:], Cw[:])
        Dc_sb = cp.tile([128, 2048], BF16, tag="Dc")
        nc.sync.dma_start(Dc_sb[:], Dc[:])
        for cch in (2, 3):
            nc.sync.dma_start(coefT7_sb[:, 1280 * cch:1280 * (cch + 1)],
                              coefT7[:, 1280 * cch:1280 * (cch + 1)])
        for b4 in range(4):           # slots 16..39
            nc.sync.dma_start(selW_sb[:, 2048 * b4 + 512:2048 * b4 + 1280],
                              selW[:, 2048 * b4 + 512:2048 * b4 + 1280])
        for cch in (4, 5):
            nc.sync.dma_start(coefT7_sb[:, 1280 * cch:1280 * (cch + 1)],
                              coefT7[:, 1280 * cch:1280 * (cch + 1)])
        for b4 in range(4):           # slots 40..63
            nc.sync.dma_start(selW_sb[:, 2048 * b4 + 1280:2048 * b4 + 2048],
                              selW[:, 2048 * b4 + 1280:2048 * b4 + 2048])
        for cch in (6, 7):
            nc.sync.dma_start(coefT7_sb[:, 1280 * cch:1280 * (cch + 1)],
                              coefT7[:, 1280 * cch:1280 * (cch + 1)])
        c23b = cp.tile([128, 1], F32, tag="c23b")
        nc.vector.memset(c23b[:], C23)

        spec_sb = [cp.tile([128, 1056], BF16, tag=f"spec{g}", name=f"spec{g}") for g in range(4)]
        noise_sb = [cp.tile([128, 256], BF16, tag=f"nz{t}", name=f"nz{t}") for t in range(8)]
        st4f = [cp.tile([128, 512], BF16, tag=f"st4fp{u}", name=f"st4fp{u}") for u in range(4)]
        stHT = cp.tile([128, 512], BF16, tag="stHT")

        # ---- pools ----
        php = ctx.enter_context(tc.tile_pool(name="phi", bufs=2))      # v1 fr tiles
        stp = ctx.enter_context(tc.tile_pool(name="st", bufs=3))
        rtp = ctx.enter_context(tc.tile_pool(name="rt", bufs=4))       # kt bf16 pairs
        ytp = ctx.enter_context(tc.tile_pool(name="yt", bufs=2))       # v3 yt f32 pairs
        awp = ctx.enter_context(tc.tile_pool(name="aw", bufs=2))
        fin = ctx.enter_context(tc.tile_pool(name="fin", bufs=2))
        fpool = ctx.enter_context(tc.tile_pool(name="fp", bufs=1))
        # PSUM: ph ring 3x[128,1024] (6 banks) + A-pair (1) + osc-pair (1)
        psPH = ctx.enter_context(tc.tile_pool(name="psPH", bufs=3, space="PSUM"))
        psSel = ctx.enter_context(tc.tile_pool(name="psSel", bufs=1, space="PSUM"))
        psOsc = ctx.enter_context(tc.tile_pool(name="psOsc", bufs=1, space="PSUM"))

        st_tiles = {}

        # ============ noise filter pre-pass (Act: exps before any sin) ======
        for g in range(4):
            nc.gpsimd.memset(spec_sb[g][:, 0:16], 0.0)
            nc.gpsimd.memset(spec_sb[g][:, 1040:1056], 0.0)
        filt_t = [None] * 4
        dts = {}
        for g in (0, 1):
            d = fpool.tile([128, 1056], BF16, tag=f"fdt{g}", name=f"fd{g}")
            nc.gpsimd.tensor_scalar(d[:], meanb_sb[:], freq4_sb[:, g:g + 1], None,
                                    OP.subtract)
            dts[g] = d
        for g in (2, 3):
            d = fpool.tile([128, 1056], BF16, tag=f"fdt{g}", name=f"fd{g}")
            nc.vector.tensor_scalar(d[:], meanb_sb[:], freq4_sb[:, g:g + 1], None,
                                    OP.subtract)
            dts[g] = d
        # PE p-state warm-up: dummy accumulations while DMAs stream in
        warm = psPH.tile([128, 1024], F32, tag="ph", name="warm")
        for i in range(18):
            nc.tensor.matmul(warm[:, 0:512], basis64_sb[0:64, 0:128],
                             basis64_sb[0:64, :], start=(i == 0), stop=(i == 17),
                             skip_group_check=True, tile_position=(0, 0))
        dd = {}
        for g in (0, 1):
            d2 = fpool.tile([128, 1056], BF16, tag=f"fd2t{g}", name=f"fd2{g}")
            nc.gpsimd.tensor_tensor(d2[:], dts[g][:], dts[g][:], OP.mult)
            dd[g] = d2
        for g in (2, 3, 0, 1):
            if g in (2, 3):
                d2 = fpool.tile([128, 1056], BF16, tag=f"fd2t{g}", name=f"fd2{g}")
                nc.vector.tensor_tensor(d2[:], dts[g][:], dts[g][:], OP.mult)
            else:
                d2 = dd[g]
            m2 = fpool.tile([128, 1056], BF16, tag=f"fm2t{g}", name=f"fm2{g}")
            nc.vector.tensor_tensor(m2[:], d2[:], c2b_sb[:], OP.mult)
            filt = fpool.tile([128, 1056], BF16, tag=f"filt{g}", name=f"filt{g}")
            nc.scalar.activation(filt[:], m2[:], AF.Exp)
            filt_t[g] = filt

        # ============ span machinery (pair-granular, software-pipelined) ====
        pair_state = {}

        def emit_F(m, b, q):
            """stage2 matmuls + round (kt) for pair q of block (m, b)."""
            var = _VAR[4 * m + b]
            if var == "v2" and q == 1 and (4 * m + b) in _V1P:
                var = "v1p"
            if q == 0:
                st = stp.tile([128, 2048], BF16, tag=f"st{b}", name=f"st{b}_{m}")
                st_tiles[(b, m)] = st
            pp = psPH.tile([128, 1024], F32, tag="ph", name=f"pp{m}_{b}_{q}")
            for h in range(2):
                k = 2 * q + h
                mm = 4 * m + k
                p2 = 64 * (mm % 2)
                cb = (mm // 2) * 640 + b * 128
                nc.tensor.matmul(pp[:, 512 * h:512 * (h + 1)],
                                 coefT7_sb[p2:p2 + 64, cb:cb + 128],
                                 basis64_sb[p2:p2 + 64, :], start=True, stop=True,
                                 skip_group_check=True, tile_position=(p2, 0))
            kt = rtp.tile([128, 1024], BF16, tag="kt", name=f"kt{m}_{b}_{q}")
            if var == "v3":
                yt = ytp.tile([128, 1024], F32, tag="yt", name=f"yt{m}_{b}_{q}")
                nc.scalar.activation(yt[:], pp[:], AF.Identity, bias=c23b[:, 0:1])
                nc.gpsimd.tensor_scalar(kt[:], yt[:], C23, None, OP.subtract)
            else:
                nc.vector.tensor_scalar(kt[:], pp[:], C23, C23, OP.add, OP.subtract)
            pair_state[(b, q)] = (var, pp, kt)

        def emit_D(m, b, q):
            """-I accumulate + Sin (or DVE sub for v1p) for pair q."""
            var, pp, kt = pair_state.pop((b, q))
            st = st_tiles[(b, m)]
            if var == "v1p":
                fr = php.tile([128, 1024], F32, tag="fr", name=f"fr{b}_{m}_{q}")
                nc.vector.tensor_tensor(fr[:], pp[:], kt[:], OP.subtract)
                nc.scalar.activation(st[:, 1024 * q:1024 * (q + 1)], fr[:],
                                     AF.Sin, scale=2 * PI)
            else:
                for h in range(2):
                    nc.tensor.matmul(pp[:, 512 * h:512 * (h + 1)], negI_sb[:],
                                     kt[:, 512 * h:512 * (h + 1)],
                                     start=False, stop=True,
                                     skip_group_check=True, tile_position=(0, 0))
                nc.scalar.activation(st[:, 1024 * q:1024 * (q + 1)], pp[:],
                                     AF.Sin, scale=2 * PI)

        def span_sched(m):
            ops = []
            pend = []
            for b in range(4):
                for q in range(2):
                    ops.append(("F", b, q))
                    pend.append((b, q))
                    if len(pend) > 2:
                        ops.append(("D",) + pend.pop(0))
            while pend:
                ops.append(("D",) + pend.pop(0))
            return ops

        def emit_fund_pair(u):
            pf = psPH.tile([128, 1024], F32, tag="ph", name=f"fph{u}")
            nc.tensor.matmul(pf[:, 0:512], fcoef_sb[:, 128 * u:128 * (u + 1)],
                             fbasis_sb[:], start=True, stop=True,
                             skip_group_check=True, tile_position=(0, 0))
            kt = rtp.tile([128, 1024], BF16, tag="kt", name=f"fkt{u}")
            nc.vector.tensor_scalar(kt[:, 0:512], pf[:, 0:512], C23, C23,
                                    OP.add, OP.subtract)
            nc.tensor.matmul(pf[:, 0:512], negI_sb[:], kt[:, 0:512],
                             start=False, stop=True, skip_group_check=True,
                             tile_position=(0, 0))
            nc.scalar.activation(st4f[u][:], pf[:, 0:512], AF.Sin, scale=2 * PI)

        def sel_windows(slot):
            t0 = 128 + 256 * slot
            m = t0 // 2048
            lo = t0 - 2048 * m
            if lo + 256 <= 2048:
                return [(m, lo, lo + 256, 0)]
            return [(m, lo, 2048, 0), (m + 1, 0, lo + 256 - 2048, 2048 - lo)]

        A_pairs = {}

        def emit_slot(slot):
            at4 = slot // 4
            v = at4 // 2
            if v not in A_pairs:
                A_pairs[v] = psSel.tile([128, 512], F32, tag="A", name=f"Ap{v}")
            ao = 256 * (at4 % 2)
            A = A_pairs[v]
            sl4 = slot % 4
            first = [True]
            if slot < 63:
                cws = sel_windows(slot)
                nmm = 4 * len(cws)
                i = 0
                for b in range(4):
                    for (mw, lo, hi, plo) in cws:
                        i += 1
                        nc.tensor.matmul(
                            A[32 * sl4:32 * sl4 + 32, ao + plo:ao + plo + hi - lo],
                            selW_sb[:, 2048 * b + 32 * slot: 2048 * b + 32 * slot + 32],
                            st_tiles[(b, mw)][:, lo:hi],
                            start=first[0], stop=(i == nmm),
                            skip_group_check=True, tile_position=(0, 32 * sl4))
                        first[0] = False
            else:
                for b in range(4):
                    nc.tensor.matmul(
                        A[96:128, ao:ao + 128],
                        selW_sb[:, 2048 * b + 32 * 63: 2048 * b + 32 * 63 + 32],
                        stHT[:, 128 * b:128 * (b + 1)],
                        start=(b == 0), stop=False,
                        skip_group_check=True, tile_position=(0, 96))
                    nc.tensor.matmul(
                        A[96:128, ao + 128:ao + 256],
                        selW_sb[:, 2048 * b + 32 * 63: 2048 * b + 32 * 63 + 32],
                        st_tiles[(b, 7)][:, 1920:2048],
                        start=False, stop=(b == 3),
                        skip_group_check=True, tile_position=(0, 96))
            if slot % 4 == 3:
                emit_atile_done(at4)

        osc_pairs = {}

        def emit_atile_done(at4):
            v = at4 // 2
            A = A_pairs[v]
            ao = 256 * (at4 % 2)
            if at4 % 2 == 1:
                A_pairs.pop(v)
            aw = awp.tile([128, 256], BF16, tag="aw")
            wc = Wc63_sb if at4 == 15 else Wc_sb
            nc.vector.tensor_tensor(aw[:], A[:, ao:ao + 256], wc[:], OP.mult)
            t = at4 // 2
            u = t // 2
            if u not in osc_pairs:
                osc_pairs[u] = psOsc.tile([128, 512], F32, tag="osc", name=f"oscp{u}")
            oo = 256 * (t % 2)
            nc.tensor.matmul(
                osc_pairs[u][64 * (at4 % 2):64 * (at4 % 2) + 64, oo:oo + 256],
                P_sb[:], aw[:], start=True, stop=True,
                skip_group_check=True, tile_position=(0, 64 * (at4 % 2)))

        def emit_rfft(g, h):
            fr_sl = slice(512 * h, 512 * (h + 1))
            sp = psPH.tile([128, 1024], F32, tag="ph", name=f"rf{g}_{h}")
            for tcx in range(4):
                nc.tensor.matmul(sp[:, 0:512],
                                 Cw_sb[:, tcx * 512 + g * 128: tcx * 512 + g * 128 + 128],
                                 nfT_sb[tcx][:, fr_sl],
                                 start=(tcx == 0), stop=(tcx == 3))
            nc.vector.tensor_tensor(spec_sb[g][:, 16 + 512 * h:16 + 512 * (h + 1)],
                                    sp[:, 0:512], filt_t[g][:, 16 + 512 * h:16 + 512 * (h + 1)],
                                    OP.mult)

        def emit_irfft(t):
            pz = psPH.tile([128, 1024], F32, tag="ph", name=f"nzps{t}")
            base = 16 + 16 * (8 * t)
            nslots = 7 if t == 7 else 8
            ncols = 16 * nslots
            for g in range(4):
                gD = Dc_sb[:, 512 * g: 512 * (g + 1)]
                sW = spec_sb[g]
                nc.tensor.matmul(pz[0:ncols, 0:256], sW[:, base:base + ncols],
                                 gD[:, 128:384], start=(g == 0), stop=False,
                                 skip_group_check=True)
                nc.tensor.matmul(pz[0:ncols, 0:128], sW[:, base - 16:base - 16 + ncols],
                                 gD[:, 384:512], start=False, stop=False,
                                 skip_group_check=True)
                nc.tensor.matmul(pz[0:ncols, 128:256], sW[:, base + 16:base + 16 + ncols],
                                 gD[:, 0:128], start=False,
                                 stop=(t < 7 and g == 3),
                                 skip_group_check=True)
            if t == 7:
                b63 = 16 + 16 * 63
                pz63 = psPH.tile([32, 512], F32, tag="ph", name="nz63")
                for g in range(4):
                    gD = Dc_sb[:, 512 * g: 512 * (g + 1)]
                    sW = spec_sb[g]
                    nc.tensor.matmul(pz63[0:16, 0:128], sW[:, 16:32],
                                     gD[:, 0:128], start=(g == 0), stop=False,
                                     skip_group_check=True, tile_position=(0, 0))
                    nc.tensor.matmul(pz63[0:16, 128:256], sW[:, b63:b63 + 16],
                                     gD[:, 128:256], start=False, stop=False,
                                     skip_group_check=True, tile_position=(0, 0))
                    nc.tensor.matmul(pz63[0:16, 128:256], sW[:, b63 - 16:b63],
                                     gD[:, 384:512], start=False, stop=(g == 3),
                                     skip_group_check=True, tile_position=(0, 0))
                nc.vector.tensor_copy(noise_sb[t][0:112, :], pz[0:112, 0:256])
                nz63s = fpool.tile([16, 256], BF16, tag="nz63s")
                nc.vector.tensor_copy(nz63s[:], pz63[0:16, 0:256])
                nc.sync.dma_start(noise_sb[t][112:128, :], nz63s[:])
            else:
                nc.vector.tensor_copy(noise_sb[t][:], pz[:, 0:256])

        def emit_combine(t):
            # r = osc*mixT + [fc*mixT + noise*(1-mixT)]; bracket has no osc dep
            u = t // 2
            osc = osc_pairs[u][:, 256 * (t % 2):256 * (t % 2) + 256]
            if t % 2 == 1:
                osc_pairs.pop(u)
            w1 = W1m63_sb if t == 7 else W1m_sb
            mixT = fin.tile([128, 256], BF16, tag="mixT")
            nc.gpsimd.tensor_scalar(mixT[:], w1[:], envsc_sb[:, 6 * t + 1:6 * t + 2],
                                    envsc_sb[:, 6 * t + 0:6 * t + 1], OP.mult, OP.add)
            mixT1 = fin.tile([128, 256], BF16, tag="mixT1")
            nc.gpsimd.tensor_scalar(mixT1[:], w1[:], envsc_sb[:, 6 * t + 5:6 * t + 6],
                                    envsc_sb[:, 6 * t + 4:6 * t + 5], OP.mult, OP.add)
            mixF = fin.tile([128, 256], BF16, tag="mixF")
            nc.gpsimd.tensor_scalar(mixF[:], w1[:], envsc_sb[:, 6 * t + 3:6 * t + 4],
                                    envsc_sb[:, 6 * t + 2:6 * t + 3], OP.mult, OP.add)
            fc = fin.tile([128, 256], BF16, tag="fc")
            nc.gpsimd.tensor_tensor(fc[:], st4f[t // 2][:, 256 * (t % 2):256 * (t % 2) + 256], mixF[:], OP.mult)
            fcm = fin.tile([128, 256], BF16, tag="fcm")
            nc.gpsimd.tensor_tensor(fcm[:], fc[:], mixT[:], OP.mult)
            nzm = fin.tile([128, 256], BF16, tag="nzm")
            nc.gpsimd.tensor_tensor(nzm[:], noise_sb[t][:], mixT1[:], OP.mult)
            pre = fin.tile([128, 256], BF16, tag="pre")
            nc.gpsimd.tensor_tensor(pre[:], fcm[:], nzm[:], OP.add)
            om = fin.tile([128, 256], BF16, tag="om")
            nc.vector.tensor_tensor(om[:], osc, mixT[:], OP.mult)
            r = fin.tile([128, 256], BF16, tag="r")
            nc._dbg[f"r{t}"] = r
            nc.gpsimd.tensor_tensor(r[:], om[:], pre[:], OP.add)
            nc.sync.dma_start(out[:, 256 * t:256 * (t + 1)], r[:])

        # ============ interleaved drive (global pipelined stream) ============
        def mkslot(sl):
            return lambda: emit_slot(sl)

        def stht_copy():
            for b in range(4):
                nc.gpsimd.tensor_copy(stHT[:, 128 * b:128 * (b + 1)],
                                      st_tiles[(b, 0)][:, 0:128])

        fillers = {m: [] for m in range(8)}
        fillers[0] = [stht_copy]
        fillers[1] = [lambda: emit_rfft(0, 0), lambda: emit_rfft(0, 1),
                      lambda: emit_rfft(1, 0), lambda: emit_rfft(1, 1),
                      mkslot(0), lambda: emit_rfft(2, 0), mkslot(1),
                      lambda: emit_rfft(2, 1), mkslot(2), lambda: emit_rfft(3, 0),
                      mkslot(3), lambda: emit_rfft(3, 1), mkslot(4),
                      lambda: emit_fund_pair(0), mkslot(5),
                      mkslot(6), mkslot(7), lambda: emit_irfft(0)]
        for m in range(2, 8):
            base_s = 8 * (m - 1)
            fl = [(lambda t: lambda: emit_combine(t))(m - 2),
                  (lambda t: lambda: emit_irfft(t))(m - 1)]
            if m in (3, 5, 7):
                fl.append((lambda u: lambda: emit_fund_pair(u))(m // 2))
            fl += [mkslot(base_s + i) for i in range(8)]
            fillers[m] = fl

        allp = [(m, b, q) for m in range(8) for b in range(4) for q in range(2)]
        ops = []
        pend = []
        for pr in allp:
            ops.append(("F",) + pr)
            pend.append(pr)
            if len(pend) > 3:
                ops.append(("D",) + pend.pop(0))
        while pend:
            ops.append(("D",) + pend.pop(0))

        from collections import Counter
        cnt = Counter(op[1] for op in ops)
        seen = Counter()
        fidx = {m: 0 for m in fillers}
        fm = [0]

        def pump():
            # strictly ordered filler groups; pace group fm by span-fm progress
            while fm[0] < 8:
                fl = fillers[fm[0]]
                if fidx[fm[0]] >= len(fl):
                    if seen[fm[0]] >= cnt[fm[0]]:
                        fm[0] += 1
                        continue
                    break
                frac = seen[fm[0]] / cnt[fm[0]]
                if (fidx[fm[0]] + 1) / len(fl) <= frac or seen[fm[0]] >= cnt[fm[0]]:
                    fl[fidx[fm[0]]]()
                    fidx[fm[0]] += 1
                else:
                    break

        for op in ops:
            if op[0] == "F":
                emit_F(op[1], op[2], op[3])
            else:
                emit_D(op[1], op[2], op[3])
            seen[op[1]] += 1
            pump()
        while fm[0] < 8:
            fl = fillers[fm[0]]
            if fidx[fm[0]] < len(fl):
                fl[fidx[fm[0]]]()
                fidx[fm[0]] += 1
            else:
                fm[0] += 1
        # epilogue: sel(7) + final combines (slots 56+ need all span-7 sins
        # emitted first; engine parallelism still overlaps them with the
        # last Act sins)
        emit_slot(56)
        emit_slot(57)
        emit_irfft(7)
        emit_slot(58)
        emit_slot(59)
        emit_combine(6)
        for sl in range(60, 64):
            emit_slot(sl)
        emit_combine(7)
    nc.finalize()
    _cache["nc"] = nc
    return nc


def _host_prep(inputs):
    st = _build_static()
    bf = ml_dtypes.bfloat16
    f0 = np.clip(np.asarray(inputs["f0"], np.float64), -0.5, 0.5)
    f0b = np.asarray(inputs["f0_baselines"], np.float64)
    erb = (0.108 * (f0b * NYQ) + 24.7) / NYQ
    f0v = np.clip(f0b + f0 * erb, 0.0, 1.0)
    f0n = MIN_F0 + f0v * F0_DIFF                                    # (B,16,64)

    # ---- harmonic rows: g' = 32*e + (o-1), o in 1..32, hfact = o+1 ----
    hfact = np.arange(2, 2 + NH, dtype=np.float64)                  # [2..33]
    frq = (f0n[:, :, None, :] * hfact[None, None, :, None] * 0.5)   # (B,16,32,64)
    frq = frq.reshape(B, NHRM, SEQ)

    coef = np.einsum("bgs,sc->bgc", frq, st["W64"])                 # (B,512,512)
    # reduce A (offset) and B (slope) coefs mod 1: basis funcs (1, j, j2hi,
    # j2lo) take integer values, so shifting A/B by integers moves the phase
    # by whole turns and leaves sin unchanged while keeping |phi| < 256.
    for hs in range(4):
        coef[:, :, (3 * hs + 0)::16] %= 1.0
        coef[:, :, (3 * hs + 1)::16] %= 1.0
    clog = np.zeros((B, NHRM, 32, 16))
    for m in range(32):
        for hs in range(4):
            base = 16 * m + 3 * hs
            clog[:, :, m, 4 * hs + 0] = coef[:, :, base + 0]
            clog[:, :, m, 4 * hs + 1] = coef[:, :, base + 1]
            clog[:, :, m, 4 * hs + 2] = coef[:, :, base + 2]
            clog[:, :, m, 4 * hs + 3] = coef[:, :, base + 2]
    splits = _split3(clog)
    # coefT7[64*(m%2) + 4*br + sp, (m//2)*640 + b*128 + glo]
    coefT7 = np.zeros((B, 128, 16 * 640), np.float16)
    for b4 in range(4):
        blkg = slice(128 * b4, 128 * (b4 + 1))
        for m in range(32):
            for sp in range(3):
                rows = 64 * (m % 2) + 4 * np.arange(16) + sp
                cols = (m // 2) * 640 + b4 * 128
                coefT7[:, rows, cols:cols + 128] = \
                    splits[sp][:, blkg, m, :].transpose(0, 2, 1)

    # ---- fund coefs in (sl,e) layout ----
    f0n_f = f0n * 0.5                                               # (B,16,64) turns
    fA = np.einsum("bes,sh->beh", f0n_f, st["Aq"])                  # (B,16,128)
    fB = np.einsum("bes,sh->beh", f0n_f, st["Bq"])
    fC = np.einsum("bes,sh->beh", f0n_f, st["Cq"])
    fA %= 1.0
    sA, sB, sC = _split3(fA), _split3(fB), _split3(fC)
    fcoef = np.zeros((B, 64, 512), np.float16)
    for t in range(8):
        ro = 32 * (t % 2)
        for sl in range(8):
            slot = 8 * t + sl
            p = 128 * (t // 2) + 16 * sl + np.arange(16)            # col index (per e)
            for hh in range(2):
                h = (1 + 2 * slot + hh) if slot < 63 else (0 if hh == 0 else 127)
                for sp in range(3):
                    fcoef[:, ro + 12 * hh + 0 + sp, p] = sA[sp][:, :, h]
                    fcoef[:, ro + 12 * hh + 3 + sp, p] = sB[sp][:, :, h]
                    fcoef[:, ro + 12 * hh + 6 + sp, p] = sC[sp][:, :, h]
                    fcoef[:, ro + 12 * hh + 9 + sp, p] = sC[sp][:, :, h]

    # ---- select weights (harmonics only) ----
    oe = np.clip(np.asarray(inputs["osc_env"], np.float64), 0, 1)   # (B,16,64)
    he = np.clip(np.asarray(inputs["harm_env"], np.float64), 0, 1)  # (B,16,32,64)
    env_node = (oe[:, :, None, :] * he).reshape(B, NHRM, SEQ)       # g'=32e+(o-1)
    selWh = np.zeros((B, 128, 4 * 2048), np.float32)
    eidx = np.arange(NHRM) // 32
    nodesL = np.concatenate([np.arange(63), [0]])
    nodesR = np.concatenate([np.arange(1, 64), [63]])
    for b4 in range(4):
        for glo in range(128):
            g = 128 * b4 + glo
            e = eidx[g]
            colsL = 2048 * b4 + 32 * np.arange(64) + e
            colsR = 2048 * b4 + 32 * np.arange(64) + 16 + e
            selWh[:, glo, colsL] = env_node[:, g, nodesL]
            selWh[:, glo, colsR] = env_node[:, g, nodesR]
    selWh = selWh.astype(bf)

    # ---- mix + fund env scalars ----
    ov = np.clip(np.asarray(inputs["overall_env"], np.float64), 0, 1)  # (B,16,64)
    envsc = np.zeros((B, 128, 48), np.float32)
    for t in range(8):
        for sl in range(8):
            slot = 8 * t + sl
            p = slice(16 * sl, 16 * (sl + 1))
            L, R = (slot, slot + 1) if slot < 63 else (0, 63)
            envsc[:, p, 6 * t + 0] = ov[:, :, L]
            envsc[:, p, 6 * t + 1] = ov[:, :, R] - ov[:, :, L]
            envsc[:, p, 6 * t + 2] = oe[:, :, L]
            envsc[:, p, 6 * t + 3] = oe[:, :, R] - oe[:, :, L]
            envsc[:, p, 6 * t + 4] = 1.0 - ov[:, :, L]
            envsc[:, p, 6 * t + 5] = ov[:, :, L] - ov[:, :, R]

    nf = np.asarray(inputs["noise_frames"], np.float32)             # (B,16,64,512)
    nfT = np.ascontiguousarray(
        nf.transpose(0, 3, 2, 1).reshape(B, WS, NFR)).astype(bf)    # [ws, s*16+e]

    nstd = np.clip(np.asarray(inputs["noise_std"], np.float64), 1e-12, 1.0) * F0_DIFF
    mean_fr = f0n.transpose(0, 2, 1).reshape(B, NFR)                # fr = s*16+e
    c2_fr = -0.5 / nstd.transpose(0, 2, 1).reshape(B, NFR) ** 2
    meanb = np.zeros((B, 128, 1056), np.float32)
    c2b = np.zeros((B, 128, 1056), np.float32)
    meanb[:, :, 16:1040] = mean_fr[:, None, :]
    c2b[:, :, 16:1040] = c2_fr[:, None, :]

    per_core = []
    for b in range(B):
        per_core.append(dict(
            coefT7=coefT7[b], selW=selWh[b], envsc=envsc[b], fcoef=fcoef[b],
            nfT=nfT[b], meanb=meanb[b].astype(bf), c2b=c2b[b].astype(bf),
            basis64=st["basis64"], fbasis=st["fbasis"],
            Cw=st["Cw"], Dc=st["Dc"], freq4=st["freq4"],
            Wc=st["Wc"], Wc63=st["Wc63"], W1m=st["W1m"], W1m63=st["W1m63"],
            P=st["P"], negI=st["negI"],
        ))
    return per_core


def _unshuffle(o):
    """[128, 2048] (sl,e)x(t,j) -> [16, 16384]."""
    full = np.zeros((NE, N), np.float32)
    for t in range(8):
        for sl in range(8):
            slot = 8 * t + sl
            r = o[16 * sl:16 * (sl + 1), 256 * t:256 * (t + 1)]
            if slot < 63:
                full[:, 128 + 256 * slot:128 + 256 * slot + 256] = r
            else:
                full[:, 0:128] = r[:, 0:128]
                full[:, 16256:16384] = r[:, 128:256]
    return full


def kernel(**inputs):
    from concourse.bass_utils import run_bass_kernel_spmd
    per_core = _host_prep(inputs)
    nc = _build_nc()
    trace = bool(os.environ.get("BASS_PROFILE"))
    res = run_bass_kernel_spmd(nc, per_core, list(range(B)), trace=trace)
    if trace and res.exec_time_ns is not None:
        print(f"HW exec time: {res.exec_time_ns} ns")
    out = np.stack([_unshuffle(np.asarray(r["out"], np.float32))
                    for r in res.results])
    return out
